# revision 1
# baseline (speedup 1.0000x reference)
"""Trainium2 Bass kernel for the AxisMDTA dense-transformer block.

Shapes (hardcoded): x (4, 256, 64, 256) fp32 -> out (4, 256, 64, 256) fp32.
The reference treats x as 256 independent samples of shape (seq f=256,
chan c=256): LN -> qkv matmul -> depthwise conv3 along f -> 8-head attn
(L2-normed q/k scaled by temperature) -> proj + residual -> LN ->
MLP(gelu) + residual.

Strategy: data-parallel over the 256 (b,t) samples across 8 NeuronCores
(32 samples per core). On-chip layout is channel-major (c on partitions,
(sample, f) on the free dim) so every activation@weight matmul runs as
out = W.T @ actT with weights stationary, and elementwise work batches
samples along the free dimension. The depthwise conv is folded into the
qkv matmul as 3 tap-scaled weight matrices applied to f-shifted rhs views
of a zero-guarded xn layout. All rsqrt (LN + L2 norm) run on densely
packed stats with an integer Newton seed, so the only ACT table sets
needed are exp (softmax) and gelu.
"""

import contextlib

import numpy as np
import ml_dtypes

import concourse.bass as bass
import concourse.mybir as mybir
import concourse.tile as tile
from concourse.vector_clock import ScopedClock
from concourse.bass_utils import run_bass_kernel_spmd

AF = mybir.ActivationFunctionType
ALU = mybir.AluOpType
DT = mybir.dt
BF16 = ml_dtypes.bfloat16

# Problem constants
B, C, T, Fd = 4, 256, 64, 256
H, Dh = 8, 32
HID = 512
NCORES = 8
SPC = (B * T) // NCORES  # 32 samples per core
LN_EPS = 1e-5
RSQRT_MAGIC = 0x5F3759DF


class _TileContext(tile.TileContext):
    """Walrus in this container caps sync-wait commands per CTRL-class
    instruction; spread the exit drain's waits across single-wait nops."""

    def _drain_and_barrier(self, tick_clock, wait_clock):
        drain_inst = self.nc.sync.drain()
        wait_clock.add_sem_waits(
            drain_inst.ins, ScopedClock({None: tick_clock.global_clock})
        )
        si = drain_inst.ins.sync_info
        waits = list(si.on_wait or []) if si else []
        if len(waits) > 1:
            si.on_wait = waits[:1]
            for w in waits[1:]:
                n = self.nc.sync.nop(nofuse=True).ins
                n.sync_info = mybir.SyncInfo(on_wait=[w], on_update=[])
        self.nc.all_engine_barrier()
        assert self.sems is not None
        popped = self.nc._tile_sem_poison_stack.pop()
        assert popped is self._sem_poison
        self.nc.clear_and_free_semaphores(list(self.sems.allocated().values()))
        self.nc.all_engine_barrier()


def _f32r(ap):
    return ap.bitcast(DT.float32r)


def _split_excess_waits(nc, max_waits=2):
    """Walrus in this container caps sync-wait commands per instruction.
    Move excess waits onto same-engine NoOps inserted just before."""
    for f in nc.m.functions:
        for bb in f.blocks:
            new_insts = []
            for inst in bb.instructions:
                si = inst.sync_info
                waits = list(si.on_wait) if si and si.on_wait else []
                if len(waits) > max_waits:
                    si.on_wait = waits[:max_waits]
                    rest = waits[max_waits:]
                    for i in range(0, len(rest), max_waits):
                        nop = mybir.InstEventSemaphore(
                            name=f"I-ws{nc.next_id()}", ins=[], outs=[])
                        nop.engine = inst.engine
                        nop.sync_info = mybir.SyncInfo(
                            on_wait=rest[i:i + max_waits], on_update=[])
                        nc.register_instruction(nop)
                        new_insts.append(nop)
                new_insts.append(inst)
            bb.instructions[:] = new_insts


def build_nc(n_blocks=4, S=8):
    """Build the per-core Bass module (same program on all 8 cores)."""
    W = S * Fd                # free width per block
    PITCH = Fd + 2            # guarded pitch for conv rhs views
    NCH = W // 512            # 512-col psum chunks
    nsamp = n_blocks * S

    nc = bass.Bass()

    # ---- DRAM I/O ----
    x_in = nc.dram_tensor("x", [C, nsamp, Fd], DT.float32, kind="ExternalInput")
    out_d = nc.dram_tensor("out", [C, nsamp, Fd], DT.float32, kind="ExternalOutput")
    wqkv3_d = nc.dram_tensor("wqkv3", [128, 2, 3, 3 * C], DT.bfloat16, kind="ExternalInput")
    wproj_d = nc.dram_tensor("wproj", [128, 2, C], DT.bfloat16, kind="ExternalInput")
    w1_d = nc.dram_tensor("w1", [128, 2, HID], DT.bfloat16, kind="ExternalInput")
    w2_d = nc.dram_tensor("w2", [128, 4, C], DT.bfloat16, kind="ExternalInput")
    vec_d = nc.dram_tensor("vecs", [128, 2, 5], DT.float32, kind="ExternalInput")
    b1_d = nc.dram_tensor("b1v", [128, 4], DT.float32, kind="ExternalInput")
    temp_d = nc.dram_tensor("temp128", [128, 1], DT.float32, kind="ExternalInput")
    ones_b_d = nc.dram_tensor("ones_b", [128, 128], DT.bfloat16, kind="ExternalInput")
    bandh0_d = nc.dram_tensor("bandh0", [128, 128], DT.bfloat16, kind="ExternalInput")
    bandh1_d = nc.dram_tensor("bandh1", [128, 128], DT.bfloat16, kind="ExternalInput")
    rowind4_d = nc.dram_tensor("rowind4", [4, 128], DT.bfloat16, kind="ExternalInput")
    ident_d = nc.dram_tensor("identb", [128, 128], DT.bfloat16, kind="ExternalInput")

    with _TileContext(nc) as tc, contextlib.ExitStack() as ctx:
        cpool = ctx.enter_context(tc.tile_pool(name="consts", bufs=1))
        xpool = ctx.enter_context(tc.tile_pool(name="xp", bufs=2))
        xnpool = ctx.enter_context(tc.tile_pool(name="xnp", bufs=2))
        qcp = ctx.enter_context(tc.tile_pool(name="qcp", bufs=6))
        ohp = ctx.enter_context(tc.tile_pool(name="ohp", bufs=2))
        sc2 = ctx.enter_context(tc.tile_pool(name="sc2", bufs=3))
        php = ctx.enter_context(tc.tile_pool(name="php", bufs=2))
        gelp = ctx.enter_context(tc.tile_pool(name="gelp", bufs=4))
        rowsp = ctx.enter_context(tc.tile_pool(name="rowsp", bufs=2))
        rows16 = ctx.enter_context(tc.tile_pool(name="rows16", bufs=2))
        dense = ctx.enter_context(tc.tile_pool(name="dense", bufs=2))
        o1pool = ctx.enter_context(tc.tile_pool(name="o1p", bufs=2))
        xhpool = ctx.enter_context(tc.tile_pool(name="xhp", bufs=2))
        opool = ctx.enter_context(tc.tile_pool(name="outp", bufs=1))
        ps = ctx.enter_context(tc.tile_pool(name="ps", bufs=2, space="PSUM"))

        # ---- load constants ----
        def cload(name, shape, dt, dram):
            t = cpool.tile(shape, dt, tag=name)
            nc.sync.dma_start(t[:], dram[:])
            return t

        wqkv_sb = cload("wqkv", [128, 2, 3, 3 * C], DT.bfloat16, wqkv3_d)
        wproj_sb = cload("wproj", [128, 2, C], DT.bfloat16, wproj_d)
        w1_sb = cload("w1", [128, 2, HID], DT.bfloat16, w1_d)
        w2_sb = cload("w2", [128, 4, C], DT.bfloat16, w2_d)
        vecs = cload("vecs", [128, 2, 5], DT.float32, vec_d)
        b1v = cload("b1v", [128, 4], DT.float32, b1_d)
        temp128 = cload("temp", [128, 1], DT.float32, temp_d)
        ones_b = cload("ones_b", [128, 128], DT.bfloat16, ones_b_d)
        bandh = [cload("bandh0", [128, 128], DT.bfloat16, bandh0_d),
                 cload("bandh1", [128, 128], DT.bfloat16, bandh1_d)]
        rowind4 = cload("rowind4", [4, 128], DT.bfloat16, rowind4_d)
        identb = cload("identb", [128, 128], DT.bfloat16, ident_d)

        # v' (f-major v with per-head ones column) - static, ones set once
        vp = [cpool.tile([128, S, H, Dh + 1], DT.bfloat16,
                         name=f"vp{b2}", tag=f"vp{b2}")
              for b2 in range(2)]
        for b2 in range(2):
            nc.vector.memset(vp[b2][:], 0.0)
            nc.vector.memset(vp[b2][:, :, :, Dh:Dh + 1], 1.0)

        def rsqrt_dense(x_ap, out_ap, iters):
            """out = 1/sqrt(x) on a small dense fp32 tile (Newton)."""
            shape = list(x_ap.shape)
            s1 = dense.tile(shape, DT.int32, tag="rsq_i1")
            s2 = dense.tile(shape, DT.int32, tag="rsq_i2")
            nc.vector.tensor_scalar(s1[:], x_ap.bitcast(DT.int32), 1, None,
                                    ALU.arith_shift_right)
            nc.vector.tensor_scalar(s2[:], s1[:], -1, None, ALU.bitwise_xor)
            nc.vector.tensor_scalar(s2[:], s2[:], RSQRT_MAGIC + 1, None,
                                    ALU.add)
            y = s2[:].bitcast(DT.float32)
            for it in range(iters):
                t = dense.tile(shape, DT.float32, tag="rsq_t")
                u = dense.tile(shape, DT.float32, tag="rsq_u")
                nc.vector.tensor_mul(t[:], y, y)
                nc.vector.scalar_tensor_tensor(u[:], t[:], -0.5, x_ap,
                                               ALU.mult, ALU.mult)
                last = (it == iters - 1)
                ynew = out_ap if last else dense.tile(
                    shape, DT.float32, name="rsq_y", tag="rsq_y")
                yap = ynew if last else ynew[:]
                nc.vector.scalar_tensor_tensor(yap, u[:], 1.5, y,
                                               ALU.add, ALU.mult)
                y = yap

        def recip_dense(x_ap, out_ap, iters=3):
            """out = 1/x on a small dense fp32 tile (Newton, bf16 out ok)."""
            shape = list(x_ap.shape)
            s1 = dense.tile(shape, DT.int32, tag="rcp_i1")
            s2 = dense.tile(shape, DT.int32, tag="rcp_i2")
            nc.vector.tensor_scalar(s1[:], x_ap.bitcast(DT.int32), -1, None,
                                    ALU.bitwise_xor)
            nc.vector.tensor_scalar(s2[:], s1[:], 0x7EF127EA + 1, None,
                                    ALU.add)
            y = s2[:].bitcast(DT.float32)
            for it in range(iters):
                u = dense.tile(shape, DT.float32, tag="rcp_u")
                nc.vector.tensor_mul(u[:], x_ap, y)
                v = dense.tile(shape, DT.float32, tag="rcp_v")
                nc.vector.tensor_scalar(v[:], u[:], -1.0, 2.0,
                                        ALU.mult, ALU.add)
                last = (it == iters - 1)
                ynew = out_ap if last else dense.tile(
                    shape, DT.float32, name="rcp_y", tag="rcp_y")
                yap = ynew if last else ynew[:]
                nc.vector.tensor_mul(yap, v[:], y)
                y = yap

        def layernorm(x_tiles, g_cols, b_cols, out_aps):
            """x_tiles: two fp32 (128, W) APs (channel-major).  Writes bf16
            out_aps.  LN over the channel (partition) axis."""
            ps_s = ps.tile([128, W], DT.float32, tag="mm")
            ps_q = ps.tile([128, W], DT.float32, tag="mm")
            for kt in range(2):
                x16 = sc2.tile([128, W], DT.bfloat16, tag="lns")
                nc.vector.tensor_copy(x16[:], x_tiles[kt])
                x2 = sc2.tile([128, W], DT.bfloat16, tag="lns")
                nc.scalar.activation(x2[:], x16[:], AF.Square)
                for ch in range(NCH):
                    nc.tensor.matmul(
                        ps_s[:, ch * 512:(ch + 1) * 512],
                        ones_b[:], x16[:, ch * 512:(ch + 1) * 512],
                        start=(kt == 0), stop=(kt == 1),
                        skip_group_check=True)
                for ch in range(NCH):
                    nc.tensor.matmul(
                        ps_q[:, ch * 512:(ch + 1) * 512],
                        ones_b[:], x2[:, ch * 512:(ch + 1) * 512],
                        start=(kt == 0), stop=(kt == 1),
                        skip_group_check=True)
            # copy one replicated stat row to SBUF, then dense-pack via DMA
            nj = W // 128
            srow = rowsp.tile([1, W], DT.float32, name="srow", tag="rows")
            qrow = rowsp.tile([1, W], DT.float32, name="qrow", tag="rows")
            nc.vector.tensor_copy(srow[:], ps_s[0:1, :])
            nc.vector.tensor_copy(qrow[:], ps_q[0:1, :])
            d_s = dense.tile([nj, 128], DT.float32, tag="dls")
            d_q = dense.tile([nj, 128], DT.float32, tag="dlq")
            nc.sync.dma_start(
                d_s[:], srow[:].rearrange("o (j c) -> o j c", j=nj))
            nc.sync.dma_start(
                d_q[:], qrow[:].rearrange("o (j c) -> o j c", j=nj))
            mu_d = dense.tile([nj, 128], DT.float32, tag="dmu")
            nc.vector.tensor_scalar(mu_d[:], d_s[:], 1.0 / C, None, ALU.mult)
            m2 = dense.tile([nj, 128], DT.float32, tag="dm2")
            nc.vector.tensor_mul(m2[:], mu_d[:], mu_d[:])
            var_d = dense.tile([nj, 128], DT.float32, tag="dvar")
            nc.vector.scalar_tensor_tensor(var_d[:], d_q[:], 1.0 / C, m2[:],
                                           ALU.mult, ALU.subtract)
            nc.vector.tensor_scalar(var_d[:], var_d[:], LN_EPS, None, ALU.add)
            rsig_d = dense.tile([nj, 128], DT.bfloat16, tag="drs")
            rsqrt_dense(var_d[:], rsig_d[:], iters=2)
            rrow = rows16.tile([1, W], DT.bfloat16, name="rrow", tag="rowsb")
            nc.sync.dma_start(
                rrow[:].rearrange("o (j c) -> o j c", j=nj), rsig_d[:])
            ps_r = ps.tile([128, W], DT.float32, tag="mm")
            for ch in range(NCH):
                nc.tensor.matmul(
                    ps_r[:, ch * 512:(ch + 1) * 512],
                    ones_b[0:1, :], rrow[:, ch * 512:(ch + 1) * 512],
                    start=True, stop=True, skip_group_check=True)
            # apply: out = ((x - mu) * g) * rsig + b
            for kt in range(2):
                w1t = sc2.tile([128, W], DT.bfloat16, tag="lns")
                nc.vector.scalar_tensor_tensor(
                    w1t[:], ps_s[:], -1.0 / C, x_tiles[kt],
                    ALU.mult, ALU.add)
                w2t = sc2.tile([128, W], DT.bfloat16, tag="lns")
                nc.vector.scalar_tensor_tensor(
                    w2t[:], w1t[:], g_cols[kt], ps_r[:],
                    ALU.mult, ALU.mult)
                nc.gpsimd.tensor_scalar(out_aps[kt], w2t[:], b_cols[kt],
                                        None, ALU.add)

        out1_tiles = {}
        xh_tiles = {}

        def attn_part(blk):
            # ---- load x ----
            x32 = [xpool.tile([128, S, Fd], DT.float32, name=f"x32_{kt}",
                              tag="x32") for kt in range(2)]
            for kt in range(2):
                nc.sync.dma_start(
                    x32[kt][:],
                    x_in[kt * 128:(kt + 1) * 128, blk * S:(blk + 1) * S, :])
            x32f = [t[:].rearrange("p s f -> p (s f)") for t in x32]

            # ---- LN1 -> xn16 (guarded layout for the folded conv) ----
            xn16 = [xnpool.tile([128, S * PITCH], DT.bfloat16,
                         name=f"xn{kt}", tag="xn") for kt in range(2)]
            for kt in range(2):
                nc.vector.memset(xn16[kt][:], 0.0)
            xn_data = [
                xn16[kt][:].rearrange("p (s q) -> p s q", q=PITCH)[:, :, 1:1 + Fd]
                for kt in range(2)]
            layernorm(x32f,
                      [vecs[:, kt, 0:1] for kt in range(2)],
                      [vecs[:, kt, 1:2] for kt in range(2)],
                      xn_data)

            # ---- qkv with folded depthwise conv3 ----
            qc = [qcp.tile([128, W], DT.bfloat16, name=f"qc{m}", tag="qc")
                  for m in range(6)]
            xn3 = [xn16[kt][:].rearrange("p (s q) -> p s q", q=PITCH)
                   for kt in range(2)]
            for m in range(6):
                ps_m = ps.tile([128, W], DT.float32, tag="mm")
                for kt in range(2):
                    for tap in range(3):
                        for sp in range(S // 2):
                            # two samples per matmul: strided rhs, N=512
                            nc.tensor.matmul(
                                ps_m[:, sp * 512:(sp + 1) * 512],
                                wqkv_sb[:, kt, tap, m * 128:(m + 1) * 128],
                                xn3[kt][:, 2 * sp:2 * sp + 2, tap:tap + Fd],
                                start=(kt == 0 and tap == 0),
                                stop=(kt == 1 and tap == 2),
                                skip_group_check=True)
                nc.any.tensor_copy(qc[m][:], ps_m[:])

            # ---- L2 normalize q, k (temperature folded into k) ----
            for vi, base in (("q", 0), ("k", 2)):
                ps_n = ps.tile([128, W], DT.float32, tag="mm")
                for ti in range(2):
                    m = base + ti
                    sq = sc2.tile([128, W], DT.bfloat16, tag="lns")
                    nc.gpsimd.tensor_mul(sq[:], qc[m][:], qc[m][:])
                    for ch in range(NCH):
                        nc.tensor.matmul(
                            ps_n[:, ch * 512:(ch + 1) * 512],
                            bandh[ti][:], sq[:, ch * 512:(ch + 1) * 512],
                            start=(ti == 0), stop=(ti == 1),
                            skip_group_check=True)
                # rows 0..7 of ps_n now hold per-head sumsq; copy + dense-pack
                nsb = rowsp.tile([8, W], DT.float32, name="nsb", tag="rows")
                nc.vector.tensor_copy(nsb[:], ps_n[0:8, :])
                d_n = dense.tile([128, 128], DT.float32, tag="dn")
                nc.sync.dma_start(
                    d_n[:], nsb[:].rearrange("h (j c) -> h j c", c=128))
                r_n = dense.tile([128, 128], DT.bfloat16, tag="dr")
                rsqrt_dense(d_n[:], r_n[:], iters=2)
                if vi == "k":
                    nc.vector.tensor_scalar(r_n[:], r_n[:], temp128[:, 0:1],
                                            None, ALU.mult)
                # scale q / k in place via band-replicated broadcast
                for ti in range(2):
                    rows = rows16.tile([4, W], DT.bfloat16, name="rows",
                                       tag="rowsb")
                    for b4 in range(4):
                        h4 = ti * 4 + b4
                        nc.sync.dma_start(
                            rows[b4:b4 + 1, :].rearrange(
                                "o (j c) -> o j c", c=128),
                            r_n[h4 * 16:(h4 + 1) * 16, :])
                    ps_b = ps.tile([128, W], DT.float32, tag="mm")
                    for ch in range(NCH):
                        nc.tensor.matmul(
                            ps_b[:, ch * 512:(ch + 1) * 512],
                            rowind4[:], rows[:, ch * 512:(ch + 1) * 512],
                            start=True, stop=True, skip_group_check=True)
                    m = base + ti
                    nc.vector.tensor_mul(qc[m][:], qc[m][:], ps_b[:])

            # ---- transpose v into v' (f-major, with ones column) ----
            for ti in range(2):
                vt = qc[4 + ti]
                for b2 in range(2):
                    ps_tp = ps.tile([128, S * 128], DT.bfloat16, tag="mm")
                    for s in range(S):
                        nc.tensor.transpose(
                            ps_tp[:, s * 128:(s + 1) * 128],
                            vt[:, s * Fd + b2 * 128:s * Fd + b2 * 128 + 128],
                            identb[:])
                    nc.vector.tensor_copy(
                        vp[b2][:, :, 4 * ti:4 * ti + 4, 0:Dh],
                        ps_tp[:].rearrange("p (s hb d) -> p s hb d",
                                           s=S, hb=4))

            # ---- attention per head ----
            rs16 = [rows16.tile([4, W], DT.bfloat16, name=f"rs16_{t_}",
                                tag=f"rows16_{t_}") for t_ in range(2)]
            oh16 = [ohp.tile([128, W], DT.bfloat16, name=f"oh{t_}", tag="oh")
                    for t_ in range(2)]
            for h in range(H):
                ti, band = h // 4, (h % 4) * 32
                phat = []
                for jt in range(2):
                    ps_S = ps.tile([128, W], DT.float32, tag="mm")
                    for s in range(S):
                        nc.tensor.matmul(
                            ps_S[:, s * Fd:(s + 1) * Fd],
                            qc[2 + ti][band:band + 32,
                                       s * Fd + jt * 128:s * Fd + jt * 128 + 128],
                            qc[ti][band:band + 32, s * Fd:(s + 1) * Fd],
                            start=True, stop=True, skip_group_check=True,
                            tile_position=(band, 0))
                    pj = php.tile([128, W], DT.bfloat16, tag="phat")
                    nc.scalar.activation(pj[:], ps_S[:], AF.Exp)
                    phat.append(pj)
                ps_O = ps.tile([Dh + 1, W], DT.float32, tag="mm")
                for s in range(S):
                    for ktj in range(2):
                        nc.tensor.matmul(
                            ps_O[:, s * Fd:(s + 1) * Fd],
                            vp[ktj][:, s, h, :],
                            phat[ktj][:, s * Fd:(s + 1) * Fd],
                            start=(ktj == 0), stop=(ktj == 1),
                            skip_group_check=True)
                # fp32 psum -> bf16 (engine), then partition move (DMA)
                ostg = sc2.tile([Dh + 1, W], DT.bfloat16, tag="lns")
                nc.any.tensor_copy(ostg[:], ps_O[:])
                nc.sync.dma_start(oh16[ti][band:band + 32, :],
                                    ostg[0:Dh, :])
                nc.sync.dma_start(rs16[ti][h % 4:h % 4 + 1, :],
                                    ostg[Dh:Dh + 1, :])

            # softmax denominators: oh16 *= 1/rowsum (band-broadcast)
            d_rs = dense.tile([128, 128], DT.float32, tag="dnr")
            for ti in range(2):
                rs32 = rowsp.tile([4, W], DT.float32, name="rs32", tag="rows")
                nc.vector.tensor_copy(rs32[:], rs16[ti][:])
                nc.sync.dma_start(
                    d_rs[ti * 64:(ti + 1) * 64, :],
                    rs32[:].rearrange("b (j c) -> b j c", c=128))
            d_ri = dense.tile([128, 128], DT.bfloat16, tag="dri")
            recip_dense(d_rs[:], d_ri[:])
            for ti in range(2):
                rinvb = rows16.tile([4, W], DT.bfloat16, name="rinvb",
                                    tag="rowsb")
                for b4 in range(4):
                    h4 = ti * 4 + b4
                    nc.sync.dma_start(
                        rinvb[b4:b4 + 1, :].rearrange(
                            "o (j c) -> o j c", c=128),
                        d_ri[h4 * 16:(h4 + 1) * 16, :])
                ps_b = ps.tile([128, W], DT.float32, tag="mm")
                for ch in range(NCH):
                    nc.tensor.matmul(
                        ps_b[:, ch * 512:(ch + 1) * 512],
                        rowind4[:], rinvb[:, ch * 512:(ch + 1) * 512],
                        start=True, stop=True, skip_group_check=True)
                nc.vector.tensor_mul(oh16[ti][:], oh16[ti][:], ps_b[:])

            # ---- proj + residual -> out1 (fp32) ----
            o1 = [o1pool.tile([128, W], DT.float32, name=f"o1_{m_}",
                              tag="out1") for m_ in range(2)]
            for m2 in range(2):
                ps_y = ps.tile([128, W], DT.float32, tag="mm")
                for kt in range(2):
                    for ch in range(NCH):
                        nc.tensor.matmul(
                            ps_y[:, ch * 512:(ch + 1) * 512],
                            wproj_sb[:, kt, m2 * 128:(m2 + 1) * 128],
                            oh16[kt][:, ch * 512:(ch + 1) * 512],
                            start=(kt == 0), stop=(kt == 1),
                            skip_group_check=True)
                nc.vector.scalar_tensor_tensor(
                    o1[m2][:], ps_y[:], 1.0, x32f[m2], ALU.mult, ALU.add)
            out1_tiles[blk] = o1

            # ---- LN2 -> xh16 ----
            xh = [xhpool.tile([128, W], DT.bfloat16, name=f"xh{t_}",
                             tag="xh") for t_ in range(2)]
            layernorm([t[:] for t in o1],
                      [vecs[:, kt, 2:3] for kt in range(2)],
                      [vecs[:, kt, 3:4] for kt in range(2)],
                      [t[:] for t in xh])
            xh_tiles[blk] = xh

        def mlp_part(blk):
            xh = xh_tiles.pop(blk)
            o1 = out1_tiles.pop(blk)
            g16 = [gelp.tile([128, W], DT.bfloat16, name=f"gel{t_}",
                          tag="gel") for t_ in range(4)]
            for mh in range(4):
                ps_h = ps.tile([128, W], DT.float32, tag="mm")
                for kt in range(2):
                    for ch in range(NCH):
                        nc.tensor.matmul(
                            ps_h[:, ch * 512:(ch + 1) * 512],
                            w1_sb[:, kt, mh * 128:(mh + 1) * 128],
                            xh[kt][:, ch * 512:(ch + 1) * 512],
                            start=(kt == 0), stop=(kt == 1),
                            skip_group_check=True)
                nc.scalar.activation(g16[mh][:], ps_h[:], AF.Gelu,
                                     bias=b1v[:, mh:mh + 1], scale=1.0)
            for m2 in range(2):
                ps_y = ps.tile([128, W], DT.float32, tag="mm")
                for kt in range(4):
                    for ch in range(NCH):
                        nc.tensor.matmul(
                            ps_y[:, ch * 512:(ch + 1) * 512],
                            w2_sb[:, kt, m2 * 128:(m2 + 1) * 128],
                            g16[kt][:, ch * 512:(ch + 1) * 512],
                            start=(kt == 0), stop=(kt == 3),
                            skip_group_check=True)
                o2 = opool.tile([128, S, Fd], DT.float32, tag="o2")
                nc.vector.scalar_tensor_tensor(
                    o2[:].rearrange("p s f -> p (s f)"),
                    ps_y[:], vecs[:, m2, 4:5], o1[m2][:],
                    ALU.add, ALU.add)
                nc.sync.dma_start(
                    out_d[m2 * 128:(m2 + 1) * 128, blk * S:(blk + 1) * S, :],
                    o2[:])

        for blk in range(n_blocks):
            attn_part(blk)
            mlp_part(blk)

    _split_excess_waits(nc, max_waits=1)
    return nc


def _host_prep(inputs):
    """Precompute device-side constant tensors from the full inputs."""
    Wqkv = np.asarray(inputs["Wqkv"], np.float32)        # (C, 3C)
    dw_w = np.asarray(inputs["dw_w"], np.float32)        # (3C, 1, 3)
    taps = dw_w[:, 0, :]                                 # (3C, 3)
    # wqkv3[p, kt, tap, ch] = Wqkv[kt*128+p, ch] * taps[ch, tap]
    wqkv3 = np.einsum("cx,xt->ctx", Wqkv, taps)          # (C, 3, 3C)
    wqkv3 = np.ascontiguousarray(
        wqkv3.reshape(2, 128, 3, 3 * C).transpose(1, 0, 2, 3)).astype(BF16)

    def kt_major(w, nkt):
        # (K, N) -> (128, nkt, N) with kt-major K split
        K, N = w.shape
        return np.ascontiguousarray(
            w.reshape(nkt, 128, N).transpose(1, 0, 2)).astype(BF16)

    wproj = kt_major(np.asarray(inputs["Wproj"], np.float32), 2)
    w1 = kt_major(np.asarray(inputs["W1"], np.float32), 2)
    w2 = kt_major(np.asarray(inputs["W2"], np.float32), 4)

    vecs = np.zeros((128, 2, 5), np.float32)
    for kt in range(2):
        sl = slice(kt * 128, (kt + 1) * 128)
        vecs[:, kt, 0] = np.asarray(inputs["norm1_g"], np.float32)[sl]
        vecs[:, kt, 1] = np.asarray(inputs["norm1_b"], np.float32)[sl]
        vecs[:, kt, 2] = np.asarray(inputs["norm2_g"], np.float32)[sl]
        vecs[:, kt, 3] = np.asarray(inputs["norm2_b"], np.float32)[sl]
        vecs[:, kt, 4] = np.asarray(inputs["b2"], np.float32)[sl]
    b1v = np.ascontiguousarray(
        np.asarray(inputs["b1"], np.float32).reshape(4, 128).T)

    temp = np.asarray(inputs["temperature"], np.float32).reshape(H)
    temp128 = np.repeat(temp, 16).reshape(128, 1).astype(np.float32)

    bandh0 = np.zeros((128, 128), np.float32)
    bandh1 = np.zeros((128, 128), np.float32)
    for d in range(128):
        for m in range(128):
            if m % 8 == d // 32:
                bandh0[d, m] = 1.0
            if m % 8 == 4 + d // 32:
                bandh1[d, m] = 1.0
    rowind4 = np.zeros((4, 128), np.float32)
    for m in range(128):
        rowind4[m // 32, m] = 1.0

    return dict(
        wqkv3=wqkv3, wproj=wproj, w1=w1, w2=w2, vecs=vecs, b1v=b1v,
        temp128=temp128,
        ones_b=np.ones((128, 128), BF16),
        bandh0=bandh0.astype(BF16),
        bandh1=bandh1.astype(BF16),
        rowind4=rowind4.astype(BF16),
        identb=np.eye(128).astype(BF16),
    )


_NC_CACHE = {}


def get_nc():
    if "nc" not in _NC_CACHE:
        _NC_CACHE["nc"] = build_nc()
    return _NC_CACHE["nc"]


def make_in_maps(inputs):
    consts = _host_prep(inputs)
    x = np.asarray(inputs["x"], np.float32)  # (B, C, T, Fd)
    in_maps = []
    for core in range(NCORES):
        b, t0 = core // 2, (core % 2) * SPC
        m = dict(consts)
        m["x"] = np.ascontiguousarray(x[b, :, t0:t0 + SPC, :])
        in_maps.append(m)
    return in_maps


def assemble_out(results):
    out = np.zeros((B, C, T, Fd), np.float32)
    for core in range(NCORES):
        b, t0 = core // 2, (core % 2) * SPC
        out[b, :, t0:t0 + SPC, :] = results[core]["out"]
    return out


def kernel(**inputs):
    nc = get_nc()
    in_maps = make_in_maps(inputs)
    res = run_bass_kernel_spmd(nc, in_maps, core_ids=list(range(NCORES)))
    return assemble_out(res.results)



# revision 2
# speedup vs baseline: 1.7000x; 1.7000x over previous
"""Trainium2 Bass kernel for the AxisMDTA dense-transformer block (v2).

Shapes (hardcoded): x (4, 256, 64, 256) fp32 -> out (4, 256, 64, 256) fp32.
256 independent samples of (f=256, c=256): LN -> qkv -> depthwise conv3
along f -> 8-head attn (L2-normed q/k, temperature) -> proj + residual ->
LN -> MLP(gelu) + residual.

Data-parallel over the 256 (b,t) samples across 8 cores (32 each, in 4
blocks of 8). Channel-major on-chip layout (c on partitions, (sample, f)
on the free dim).  Key cost choices vs v1:
  - depthwise conv runs on DVE (4x-rate scalar_tensor_tensor on bf16
    guarded-pitch views) instead of tripling the qkv matmul work;
  - v' (f-major v for attn@v) comes straight from 3-tap matmuls with
    xn slices as the stationary operand - no PE transposes;
  - LN/L2 statistics matmuls emit band-replicated rows so rsqrt runs
    directly on the replicated tiles (Ln+Exp on ACT) - no dense-pack
    DMAs, no Newton iterations, no broadcast matmuls;
  - attn@v processes head pairs with an in-psum ones-block that yields
    the softmax denominators, normalized during the psum drain;
  - x is pre-cast to bf16 on the host, halving DMA and enabling 4x DVE.
"""

import contextlib

import numpy as np
import ml_dtypes

import concourse.bass as bass
import concourse.mybir as mybir
import concourse.tile as tile
from concourse.vector_clock import ScopedClock
from concourse.bass_utils import run_bass_kernel_spmd

AF = mybir.ActivationFunctionType
ALU = mybir.AluOpType
DT = mybir.dt
BF16 = ml_dtypes.bfloat16

# Problem constants
B, C, T, Fd = 4, 256, 64, 256
H, Dh = 8, 32
HID = 512
NCORES = 8
SPC = (B * T) // NCORES  # 32 samples per core
LN_EPS = 1e-5
S = 8                    # samples per block
NBLK = SPC // S
W = S * Fd               # 2048 free columns per block
PITCH = Fd + 2           # guarded pitch for conv views
NCH = W // 512           # 512-col psum chunks


class _TileContext(tile.TileContext):
    """Walrus in this container caps sync-wait commands per CTRL-class
    instruction; spread the exit drain's waits across single-wait nops."""

    def _drain_and_barrier(self, tick_clock, wait_clock):
        drain_inst = self.nc.sync.drain()
        wait_clock.add_sem_waits(
            drain_inst.ins, ScopedClock({None: tick_clock.global_clock})
        )
        si = drain_inst.ins.sync_info
        waits = list(si.on_wait or []) if si else []
        if len(waits) > 1:
            si.on_wait = waits[:1]
            for w in waits[1:]:
                n = self.nc.sync.nop(nofuse=True).ins
                n.sync_info = mybir.SyncInfo(on_wait=[w], on_update=[])
        self.nc.all_engine_barrier()
        assert self.sems is not None
        popped = self.nc._tile_sem_poison_stack.pop()
        assert popped is self._sem_poison
        self.nc.clear_and_free_semaphores(list(self.sems.allocated().values()))
        self.nc.all_engine_barrier()


def _split_excess_waits(nc, max_waits=1):
    """Walrus in this container caps sync-wait commands per instruction.
    Move excess waits onto same-engine NoOps inserted just before."""
    for f in nc.m.functions:
        for bb in f.blocks:
            new_insts = []
            for inst in bb.instructions:
                si = inst.sync_info
                waits = list(si.on_wait) if si and si.on_wait else []
                if len(waits) > max_waits:
                    si.on_wait = waits[:max_waits]
                    rest = waits[max_waits:]
                    for i in range(0, len(rest), max_waits):
                        nop = mybir.InstEventSemaphore(
                            name=f"I-ws{nc.next_id()}", ins=[], outs=[])
                        nop.engine = inst.engine
                        nop.sync_info = mybir.SyncInfo(
                            on_wait=rest[i:i + max_waits], on_update=[])
                        nc.register_instruction(nop)
                        new_insts.append(nop)
                new_insts.append(inst)
            bb.instructions[:] = new_insts


def build_nc(skip_b1=True, skip_b2=True, skip_lnb=True,
             skip_lng=True):
    nc = bass.Bass()

    x_in = nc.dram_tensor("x16", [128, 2, SPC, Fd], DT.bfloat16,
                          kind="ExternalInput")
    out_d = nc.dram_tensor("out", [128, 2, SPC, Fd], DT.float32,
                           kind="ExternalOutput")
    wqk_d = nc.dram_tensor("wqk", [128, 2, 512], DT.bfloat16, kind="ExternalInput")
    wv3_d = nc.dram_tensor("wv3", [128, 2, 3, 256], DT.bfloat16, kind="ExternalInput")
    wproj_d = nc.dram_tensor("wproj", [128, 2, 256], DT.bfloat16, kind="ExternalInput")
    w1_d = nc.dram_tensor("w1", [128, 2, HID], DT.bfloat16, kind="ExternalInput")
    w2_d = nc.dram_tensor("w2", [128, 4, 256], DT.bfloat16, kind="ExternalInput")
    vec_d = nc.dram_tensor("vecs", [128, 2, 6], DT.float32, kind="ExternalInput")
    b1_d = nc.dram_tensor("b1v", [128, 4], DT.float32, kind="ExternalInput")
    taps_d = nc.dram_tensor("tapsqk", [128, 4, 3], DT.float32, kind="ExternalInput")
    ones_d = nc.dram_tensor("ones_b", [128, 128], DT.bfloat16, kind="ExternalInput")
    bandq_d = nc.dram_tensor("bandq", [128, 128], DT.bfloat16, kind="ExternalInput")
    bandk_d = nc.dram_tensor("bandk", [128, 2, 128], DT.bfloat16, kind="ExternalInput")

    with _TileContext(nc) as tc, contextlib.ExitStack() as ctx:
        cpool = ctx.enter_context(tc.tile_pool(name="consts", bufs=1))
        xpool = ctx.enter_context(tc.tile_pool(name="xp", bufs=2))
        xnpool = ctx.enter_context(tc.tile_pool(name="xnp", bufs=2))
        qrawp = ctx.enter_context(tc.tile_pool(name="qraw", bufs=2))
        qcp = ctx.enter_context(tc.tile_pool(name="qcp", bufs=1))
        wp = ctx.enter_context(tc.tile_pool(name="wp", bufs=3))
        php = ctx.enter_context(tc.tile_pool(name="php", bufs=4))
        ohp = ctx.enter_context(tc.tile_pool(name="ohp", bufs=1))
        o1p = ctx.enter_context(tc.tile_pool(name="o1p", bufs=1))
        xhp = ctx.enter_context(tc.tile_pool(name="xhp", bufs=1))
        gp = ctx.enter_context(tc.tile_pool(name="gp", bufs=1))
        op = ctx.enter_context(tc.tile_pool(name="op", bufs=1))
        ps = ctx.enter_context(tc.tile_pool(name="ps", bufs=2, space="PSUM"))

        def cload(name, shape, dt, dram):
            t = cpool.tile(shape, dt, tag=name)
            nc.sync.dma_start(t[:], dram[:])
            return t

        wqk = cload("wqk", [128, 2, 512], DT.bfloat16, wqk_d)
        wv3 = cload("wv3", [128, 2, 3, 256], DT.bfloat16, wv3_d)
        wproj = cload("wproj", [128, 2, 256], DT.bfloat16, wproj_d)
        w1 = cload("w1", [128, 2, HID], DT.bfloat16, w1_d)
        w2 = cload("w2", [128, 4, 256], DT.bfloat16, w2_d)
        vecs = cload("vecs", [128, 2, 6], DT.float32, vec_d)
        b1v = cload("b1v", [128, 4], DT.float32, b1_d)
        tapsqk = cload("tapsqk", [128, 4, 3], DT.float32, taps_d)
        ones_b = cload("ones_b", [128, 128], DT.bfloat16, ones_d)
        bandq = cload("bandq", [128, 128], DT.bfloat16, bandq_d)
        bandk = cload("bandk", [128, 2, 128], DT.bfloat16, bandk_d)

        # v' composite lhsT tiles (single set; the framework serializes
        # next-block drains behind this block's attn@v reads):
        # layout (128 fk, S, pair, slot, 128); slot0 = [vA |0| 1 |0],
        # slot1 = [0| vB |0| 1].
        vpx = [cpool.tile([128, S, 4, 2, 128], DT.bfloat16,
                          name=f"vpx_{fc}", tag=f"vpx_{fc}")
               for fc in range(2)]
        for fc in range(2):
            nc.vector.memset(vpx[fc][:], 0.0)
            nc.vector.memset(vpx[fc][:, :, :, 0, 64:96], 1.0)
            nc.vector.memset(vpx[fc][:, :, :, 1, 96:128], 1.0)

        def rsqrt_rep(src_ap, out_ap, eps_ap):
            """out = 1/sqrt(src + eps), elementwise, via ACT Ln + Exp.
            src may be PSUM fp32; out is bf16 SBUF (same partitions)."""
            t = wp.tile([128, W], DT.bfloat16, name="lnt", tag="wk")
            nc.scalar.activation(t[: src_ap.shape[0]], src_ap, AF.Ln,
                                 bias=eps_ap, scale=1.0)
            nc.scalar.activation(out_ap, t[: src_ap.shape[0]], AF.Exp,
                                 scale=-0.5)

        def layernorm(src_f, g_cols, b_cols, out_aps, skip_b, skip_g,
                      relaxed):
            ve = nc.vector
            """src_f: two bf16 (128, 2048) APs (channel-major, kt split).
            Writes bf16 out_aps. LN over the 256 channels (both kt tiles).
            Fast-evicts the psum stat tiles to bf16 so psum frees early."""
            ps_s = ps.tile([128, W], DT.float32, tag="mm")
            ps_q = ps.tile([128, W], DT.float32, tag="mm")
            for kt in range(2):
                for ch in range(NCH):
                    sl = slice(ch * 512, (ch + 1) * 512)
                    nc.tensor.matmul(
                        ps_s[:, sl], ones_b[:], src_f[kt][:, sl],
                        start=(kt == 0), stop=(kt == 1), skip_group_check=True)
            for kt in range(2):
                sq = wp.tile([128, W], DT.bfloat16, name="lnsq", tag="lnsq",
                             bufs=1)
                ve.tensor_mul(sq[:], src_f[kt], src_f[kt])
                for ch in range(NCH):
                    sl = slice(ch * 512, (ch + 1) * 512)
                    nc.tensor.matmul(
                        ps_q[:, sl], ones_b[:], sq[:, sl],
                        start=(kt == 0), stop=(kt == 1), skip_group_check=True)
            mu16 = wp.tile([128, W], DT.bfloat16, name="mu16", tag="lnmu",
                           bufs=2)
            ve.tensor_scalar(mu16[:], ps_s[:], 1.0 / C, None, ALU.mult)
            q16 = wp.tile([128, W], DT.bfloat16, name="q16", tag="wk")
            nc.scalar.activation(q16[:], ps_q[:], AF.Copy, bias=0.0,
                                 scale=1.0 / C)
            mu2 = wp.tile([128, W], DT.bfloat16, name="mu2", tag="wk")
            ve.tensor_mul(mu2[:], mu16[:], mu16[:])
            var = wp.tile([128, W], DT.bfloat16, name="var", tag="wk")
            ve.tensor_sub(var[:], q16[:], mu2[:])
            rsig = wp.tile([128, W], DT.bfloat16, name="rsg1", tag="lnrs",
                           bufs=2)
            rsqrt_rep(var[:], rsig[:], vecs[:, 0, 5:6])
            for kt in range(2):
                if skip_g:
                    gr = rsig
                else:
                    gr = wp.tile([128, W], DT.bfloat16, name="lngr", tag="wk")
                    nc.vector.tensor_scalar(gr[:], rsig[:], g_cols[kt], None,
                                            ALU.mult)
                a = wp.tile([128, W], DT.bfloat16, name="lna", tag="wk")
                ve.tensor_sub(a[:], src_f[kt], mu16[:])
                if skip_b:
                    nc.vector.tensor_mul(out_aps[kt], a[:], gr[:])
                else:
                    t = wp.tile([128, W], DT.bfloat16, name="lnb", tag="wk")
                    nc.vector.tensor_mul(t[:], a[:], gr[:])
                    nc.vector.tensor_scalar(
                        out_aps[kt], t[:], b_cols[kt], None, ALU.add)

        # ---------------- staged, software-pipelined blocks ----------------
        st = [dict() for _ in range(NBLK)]

        def dma_x(blk):
            d = st[blk]
            d["x16"] = [xpool.tile([128, S, Fd], DT.bfloat16,
                                   name=f"x16_{kt}", tag=f"x16_{kt}")
                        for kt in range(2)]
            for kt in range(2):
                nc.sync.dma_start(
                    d["x16"][kt][:], x_in[:, kt, blk * S:(blk + 1) * S, :])
            d["xf"] = [t[:].rearrange("p s f -> p (s f)") for t in d["x16"]]

        def s1_ln1(blk):
            d = st[blk]
            xn = [xnpool.tile([128, S, PITCH], DT.bfloat16, name=f"xn{kt}",
                              tag=f"xn{kt}") for kt in range(2)]
            for kt in range(2):
                nc.vector.memset(xn[kt][:, :, 0:1], 0.0)
                nc.vector.memset(xn[kt][:, :, PITCH - 1:PITCH], 0.0)
            xn_data = [xn[kt][:, :, 1:1 + Fd] for kt in range(2)]
            layernorm(d["xf"], [vecs[:, kt, 0:1] for kt in range(2)],
                      [vecs[:, kt, 1:2] for kt in range(2)],
                      xn_data, skip_lnb, skip_lng, relaxed=True)
            d["xn"] = xn
            d["xn_rhs"] = xn_data

        def s2_qkv(blk):
            d = st[blk]
            d["qc"] = []
            for m in range(4):
                ps_m = ps.tile([128, W], DT.float32, tag="mm")
                for kt in range(2):
                    for ch in range(NCH):
                        nc.tensor.matmul(
                            ps_m[:, ch * 512:(ch + 1) * 512],
                            wqk[:, kt, m * 128:(m + 1) * 128],
                            d["xn_rhs"][kt][:, 2 * ch:2 * ch + 2, :],
                            start=(kt == 0), stop=(kt == 1),
                            skip_group_check=True)
                qr = qrawp.tile([128, S, PITCH], DT.bfloat16, name=f"qr{m}",
                                tag="qraw")
                nc.gpsimd.memset(qr[:, :, 0:1], 0.0)
                nc.gpsimd.memset(qr[:, :, PITCH - 1:PITCH], 0.0)
                if m % 2 == 0:
                    nc.scalar.copy(qr[:, :, 1:1 + Fd],
                                   ps_m[:].rearrange("p (s f) -> p s f", s=S))
                else:
                    nc.vector.tensor_copy(
                        qr[:, :, 1:1 + Fd],
                        ps_m[:].rearrange("p (s f) -> p s f", s=S))
                t0 = wp.tile([128, S, Fd], DT.bfloat16, name="cv0", tag="wk")
                nc.vector.tensor_scalar(
                    t0[:], qr[:, :, 0:Fd], tapsqk[:, m, 0:1], None, ALU.mult)
                t1 = wp.tile([128, S, Fd], DT.bfloat16, name="cv1", tag="wk")
                nc.vector.tensor_scalar(
                    t1[:], qr[:, :, 1:1 + Fd], tapsqk[:, m, 1:2], None,
                    ALU.mult)
                t2 = wp.tile([128, S, Fd], DT.bfloat16, name="cv2", tag="wk")
                nc.vector.tensor_scalar(
                    t2[:], qr[:, :, 2:2 + Fd], tapsqk[:, m, 2:3], None,
                    ALU.mult)
                nc.vector.tensor_add(t0[:], t0[:], t1[:])
                q = qcp.tile([128, S, Fd], DT.bfloat16, name=f"qc{m}",
                             tag=f"qc{m}")
                nc.vector.tensor_add(q[:], t0[:], t2[:])
                d["qc"].append(q)

        def s3_vprime(blk):
            d = st[blk]
            for fc in range(2):
                vp_ps = ps.tile([128, S, 256], DT.float32, tag="mm")
                for s in range(S):
                    first = True
                    for kt in range(2):
                        for tap in range(3):
                            nc.tensor.matmul(
                                vp_ps[:, s, :],
                                d["xn"][kt][:, s,
                                            fc * 128 + tap:fc * 128 + tap + 128],
                                wv3[:, kt, tap, :],
                                start=first, stop=(kt == 1 and tap == 2),
                                skip_group_check=True)
                            first = False
                vsrc = vp_ps[:].rearrange("p s (h2 two d) -> p s h2 two d",
                                          two=2, d=Dh)
                nc.scalar.copy(vpx[fc][:, :, :, 0, 0:32],
                               vsrc[:, :, :, 0, :])
                nc.vector.tensor_copy(vpx[fc][:, :, :, 1, 32:64],
                                      vsrc[:, :, :, 1, :])

        def s4_l2(blk):
            d = st[blk]
            for m in (0, 2, 1, 3):
                qf = d["qc"][m][:].rearrange("p s f -> p (s f)")
                sqm = wp.tile([128, W], DT.bfloat16, name="sqm",
                              tag="l2sq", bufs=2)
                nc.vector.tensor_mul(sqm[:], qf, qf)
                ps_n = ps.tile([128, W], DT.float32, tag="mm")
                band = bandq[:] if m < 2 else bandk[:, m - 2, :]
                for ch in range(NCH):
                    sl = slice(ch * 512, (ch + 1) * 512)
                    nc.tensor.matmul(ps_n[:, sl], band, sqm[:, sl],
                                     start=True, stop=True,
                                     skip_group_check=True)
                rsig = wp.tile([128, W], DT.bfloat16, name="rsg2", tag="lnrs",
                               bufs=2)
                if m % 2 == 0:
                    rsqrt_rep(ps_n[:], rsig[:], vecs[:, 1, 5:6])
                else:
                    sig = wp.tile([128, W], DT.bfloat16, name="sig", tag="wk")
                    nc.scalar.activation(sig[:], ps_n[:], AF.Sqrt,
                                         bias=vecs[:, 1, 5:6], scale=1.0)
                    with nc.allow_low_precision(reason="l2 rsig bf16"):
                        nc.vector.reciprocal(rsig[:], sig[:])
                nc.vector.tensor_mul(qf, qf, rsig[:])

        def s5_attn(blk):
            d = st[blk]
            qc = d["qc"]
            phat = {}
            oh = [ohp.tile([128, W], DT.bfloat16, name=f"oh{kt}",
                           tag=f"oh{kt}") for kt in range(2)]
            for h in range(H):
                ti, band = h // 4, (h % 4) * 32
                for jt in range(2):
                    ps_S = ps.tile([128, W], DT.float32, tag="mm")
                    for s in range(S):
                        nc.tensor.matmul(
                            ps_S[:, s * Fd:(s + 1) * Fd],
                            qc[2 + ti][band:band + 32, s,
                                       jt * 128:jt * 128 + 128],
                            qc[ti][band:band + 32, s, :],
                            start=True, stop=True, skip_group_check=True,
                            tile_position=(band, 0))
                    pj = php.tile([128, W], DT.bfloat16, tag="phat")
                    nc.scalar.activation(pj[:, 0:W // 2], ps_S[:, 0:W // 2],
                                         AF.Exp)
                    nc.scalar.activation(pj[:, W // 2:W], ps_S[:, W // 2:W],
                                         AF.Exp)
                    phat[(h, jt)] = pj
                if h % 2 == 1:
                    p = h // 2
                    pair_ps = ps.tile([128, W], DT.float32, tag="mm")
                    for s in range(S):
                        osl = pair_ps[:, s * Fd:(s + 1) * Fd]
                        first = True
                        for sl, hh in ((0, h - 1), (1, h)):
                            for fc in range(2):
                                nc.tensor.matmul(
                                    osl, vpx[fc][:, s, p, sl, :],
                                    phat[(hh, fc)][:, s * Fd:(s + 1) * Fd],
                                    start=first,
                                    stop=(sl == 1 and fc == 1),
                                    skip_group_check=True)
                                first = False
                    pnum = wp.tile([128, W], DT.bfloat16, name="pnum",
                                   tag="lnsq", bufs=1)
                    nc.vector.tensor_copy(pnum[:], pair_ps[:])
                    rinv = wp.tile([64, W], DT.bfloat16, tag="rinv", bufs=1)
                    with nc.allow_low_precision(reason="softmax denom bf16"):
                        nc.vector.reciprocal(rinv[:], pnum[64:128, :])
                    nc.vector.tensor_mul(
                        oh[p // 2][(p % 2) * 64:(p % 2) * 64 + 64, :],
                        pnum[0:64, :], rinv[:])
                    del phat[(h - 1, 0)], phat[(h - 1, 1)]
                    del phat[(h, 0)], phat[(h, 1)]
            d["oh"] = oh

        def s6_proj(blk):
            d = st[blk]
            d["o1"] = []
            for m2 in range(2):
                ps_y = ps.tile([128, W], DT.float32, tag="mm")
                for kt in range(2):
                    for ch in range(NCH):
                        sl = slice(ch * 512, (ch + 1) * 512)
                        nc.tensor.matmul(
                            ps_y[:, sl],
                            wproj[:, kt, m2 * 128:(m2 + 1) * 128],
                            d["oh"][kt][:, sl], start=(kt == 0),
                            stop=(kt == 1), skip_group_check=True)
                o1t = o1p.tile([128, W], DT.bfloat16, name=f"o1_{m2}",
                               tag=f"o1_{m2}")
                nc.vector.tensor_add(o1t[:], ps_y[:], d["xf"][m2])
                d["o1"].append(o1t)

        def s7_ln2(blk):
            d = st[blk]
            xh = [xhp.tile([128, W], DT.bfloat16, name=f"xh{kt}",
                           tag=f"xh{kt}") for kt in range(2)]
            layernorm([t[:] for t in d["o1"]],
                      [vecs[:, kt, 2:3] for kt in range(2)],
                      [vecs[:, kt, 3:4] for kt in range(2)],
                      [t[:] for t in xh], skip_lnb, skip_lng, relaxed=False)
            d["xh"] = xh

        def s8_mlp1(blk):
            d = st[blk]
            d["g16"] = []
            for mh in range(4):
                ps_h = ps.tile([128, W], DT.float32, tag="mm")
                for kt in range(2):
                    for ch in range(NCH):
                        sl = slice(ch * 512, (ch + 1) * 512)
                        nc.tensor.matmul(
                            ps_h[:, sl], w1[:, kt, mh * 128:(mh + 1) * 128],
                            d["xh"][kt][:][:, sl], start=(kt == 0),
                            stop=(kt == 1), skip_group_check=True)
                g = gp.tile([128, W], DT.bfloat16, name=f"g{mh}", tag=f"g{mh}")
                if skip_b1:
                    nc.scalar.activation(g[:], ps_h[:], AF.Gelu)
                else:
                    nc.scalar.activation(g[:], ps_h[:], AF.Gelu,
                                         bias=b1v[:, mh:mh + 1], scale=1.0)
                d["g16"].append(g)

        def s9_mlp2(blk):
            d = st[blk]
            for m2 in range(2):
                ps_o = ps.tile([128, W], DT.float32, tag="mm")
                for kt in range(4):
                    for ch in range(NCH):
                        sl = slice(ch * 512, (ch + 1) * 512)
                        nc.tensor.matmul(
                            ps_o[:, sl], w2[:, kt, m2 * 128:(m2 + 1) * 128],
                            d["g16"][kt][:][:, sl], start=(kt == 0),
                            stop=(kt == 3), skip_group_check=True)
                o2 = op.tile([128, S, Fd], DT.float32, name="o2", tag="o2",
                             bufs=1)
                o2f = o2[:].rearrange("p s f -> p (s f)")
                if skip_b2:
                    nc.vector.tensor_add(o2f, ps_o[:], d["o1"][m2][:])
                else:
                    t = op.tile([128, W], DT.float32, name="o2t", tag="o2t")
                    nc.vector.tensor_add(t[:], ps_o[:], d["o1"][m2][:])
                    nc.vector.tensor_scalar(
                        o2f, t[:], vecs[:, m2, 4:5], None, ALU.add)
                nc.gpsimd.dma_start(
                    out_d[:, m2, blk * S:(blk + 1) * S, :], o2[:])

        # prologue (pipeline fill: LN1 runs two blocks ahead)
        dma_x(0)
        dma_x(1)
        s1_ln1(0)
        s1_ln1(1)
        s3_vprime(0)
        s2_qkv(0)
        s4_l2(0)
        for b in range(NBLK):
            s5_attn(b)
            s6_proj(b)
            s7_ln2(b)
            if b + 1 < NBLK:
                s3_vprime(b + 1)
                s2_qkv(b + 1)
            s8_mlp1(b)
            s9_mlp2(b)
            if b + 1 < NBLK:
                s4_l2(b + 1)
                if b + 2 < NBLK:
                    dma_x(b + 2)
                    s1_ln1(b + 2)

    _split_excess_waits(nc, max_waits=1)
    return nc


def _host_prep(inputs):
    Wqkv = np.asarray(inputs["Wqkv"], np.float32)        # (C, 3C)
    dw_w = np.asarray(inputs["dw_w"], np.float32)        # (3C, 1, 3)
    taps = dw_w[:, 0, :]                                 # (3C, 3)

    wqk = np.ascontiguousarray(
        Wqkv[:, :512].reshape(2, 128, 512).transpose(1, 0, 2)).astype(BF16)
    # wv3[p, kt, tap, c_out] = Wv[kt*128+p, c_out] * taps_v[c_out, tap]
    Wv = Wqkv[:, 512:]                                   # (C, 256)
    wv3 = np.einsum("co,ot->cto", Wv, taps[512:])        # (C, 3, 256)
    wv3 = np.ascontiguousarray(
        wv3.reshape(2, 128, 3, 256).transpose(1, 0, 2, 3)).astype(BF16)

    def kt_major(w, nkt):
        K, N = w.shape
        return np.ascontiguousarray(
            w.reshape(nkt, 128, N).transpose(1, 0, 2)).astype(BF16)

    wproj = kt_major(np.asarray(inputs["Wproj"], np.float32), 2)
    w1 = kt_major(np.asarray(inputs["W1"], np.float32), 2)
    w2 = kt_major(np.asarray(inputs["W2"], np.float32), 4)

    vecs = np.zeros((128, 2, 6), np.float32)
    for kt in range(2):
        sl = slice(kt * 128, (kt + 1) * 128)
        vecs[:, kt, 0] = np.asarray(inputs["norm1_g"], np.float32)[sl]
        vecs[:, kt, 1] = np.asarray(inputs["norm1_b"], np.float32)[sl]
        vecs[:, kt, 2] = np.asarray(inputs["norm2_g"], np.float32)[sl]
        vecs[:, kt, 3] = np.asarray(inputs["norm2_b"], np.float32)[sl]
        vecs[:, kt, 4] = np.asarray(inputs["b2"], np.float32)[sl]
    vecs[:, 0, 5] = LN_EPS
    vecs[:, 1, 5] = 1e-24
    b1v = np.ascontiguousarray(
        np.asarray(inputs["b1"], np.float32).reshape(4, 128).T)

    # conv taps for the q,k channel tiles: tapsqk[p, m, tap]
    tapsqk = np.ascontiguousarray(
        taps[:512].reshape(4, 128, 3).transpose(1, 0, 2)).astype(np.float32)

    temp = np.asarray(inputs["temperature"], np.float32).reshape(H)
    bandq = np.zeros((128, 128), np.float32)
    bandk = np.zeros((128, 2, 128), np.float32)
    for k in range(128):
        for m in range(128):
            if k // 32 == m // 32:
                bandq[k, m] = 1.0
                for ti in range(2):
                    h = ti * 4 + m // 32
                    bandk[k, ti, m] = 1.0 / (temp[h] * temp[h])

    return dict(
        wqk=wqk, wv3=wv3, wproj=wproj, w1=w1, w2=w2, vecs=vecs, b1v=b1v,
        tapsqk=tapsqk,
        ones_b=np.ones((128, 128), BF16),
        bandq=bandq.astype(BF16),
        bandk=bandk.astype(BF16),
    )


_NC_CACHE = {}


def get_nc(flags=(True, True, True, True)):
    if flags not in _NC_CACHE:
        _NC_CACHE[flags] = build_nc(*flags)
    return _NC_CACHE[flags]


def _flags_from(inputs):
    z = lambda k: bool(np.all(np.asarray(inputs[k]) == 0.0))
    o = lambda k: bool(np.all(np.asarray(inputs[k]) == 1.0))
    return (z("b1"), z("b2"), z("norm1_b") and z("norm2_b"),
            o("norm1_g") and o("norm2_g"))


def make_in_maps(inputs):
    consts = _host_prep(inputs)
    x = np.asarray(inputs["x"], np.float32)  # (B, C, T, Fd)
    in_maps = []
    for core in range(NCORES):
        b, t0 = core // 2, (core % 2) * SPC
        m = dict(consts)
        xs = x[b, :, t0:t0 + SPC, :]          # (C, SPC, Fd)
        m["x16"] = np.ascontiguousarray(
            xs.reshape(2, 128, SPC, Fd).transpose(1, 0, 2, 3)).astype(BF16)
        in_maps.append(m)
    return in_maps


def assemble_out(results):
    out = np.zeros((B, C, T, Fd), np.float32)
    for core in range(NCORES):
        b, t0 = core // 2, (core % 2) * SPC
        r = results[core]["out"]              # (128, 2, SPC, Fd)
        out[b, :, t0:t0 + SPC, :] = r.transpose(1, 0, 2, 3).reshape(
            C, SPC, Fd)
    return out


def kernel(**inputs):
    nc = get_nc(_flags_from(inputs))
    in_maps = make_in_maps(inputs)
    res = run_bass_kernel_spmd(nc, in_maps, core_ids=list(range(NCORES)))
    return assemble_out(res.results)


# revision 3
# speedup vs baseline: 1.7527x; 1.0310x over previous
"""Trainium2 Bass kernel for the AxisMDTA dense-transformer block (v2).

Shapes (hardcoded): x (4, 256, 64, 256) fp32 -> out (4, 256, 64, 256) fp32.
256 independent samples of (f=256, c=256): LN -> qkv -> depthwise conv3
along f -> 8-head attn (L2-normed q/k, temperature) -> proj + residual ->
LN -> MLP(gelu) + residual.

Data-parallel over the 256 (b,t) samples across 8 cores (32 each, in 4
blocks of 8). Channel-major on-chip layout (c on partitions, (sample, f)
on the free dim).  Key cost choices vs v1:
  - depthwise conv runs on DVE (4x-rate scalar_tensor_tensor on bf16
    guarded-pitch views) instead of tripling the qkv matmul work;
  - v' (f-major v for attn@v) comes straight from 3-tap matmuls with
    xn slices as the stationary operand - no PE transposes;
  - LN/L2 statistics matmuls emit band-replicated rows so rsqrt runs
    directly on the replicated tiles (Ln+Exp on ACT) - no dense-pack
    DMAs, no Newton iterations, no broadcast matmuls;
  - attn@v processes head pairs with an in-psum ones-block that yields
    the softmax denominators, normalized during the psum drain;
  - x is pre-cast to bf16 on the host, halving DMA and enabling 4x DVE.
"""

import contextlib

import numpy as np
import ml_dtypes

import concourse.bass as bass
import concourse.mybir as mybir
import concourse.tile as tile
from concourse.vector_clock import ScopedClock
from concourse.bass_utils import run_bass_kernel_spmd

AF = mybir.ActivationFunctionType
ALU = mybir.AluOpType
DT = mybir.dt
BF16 = ml_dtypes.bfloat16

# Problem constants
B, C, T, Fd = 4, 256, 64, 256
H, Dh = 8, 32
HID = 512
NCORES = 8
SPC = (B * T) // NCORES  # 32 samples per core
LN_EPS = 1e-5
S = 8                    # samples per block
NBLK = SPC // S
W = S * Fd               # 2048 free columns per block
PITCH = Fd + 2           # guarded pitch for conv views
NCH = W // 512           # 512-col psum chunks


class _TileContext(tile.TileContext):
    """Walrus in this container caps sync-wait commands per CTRL-class
    instruction; spread the exit drain's waits across single-wait nops."""

    def _drain_and_barrier(self, tick_clock, wait_clock):
        drain_inst = self.nc.sync.drain()
        wait_clock.add_sem_waits(
            drain_inst.ins, ScopedClock({None: tick_clock.global_clock})
        )
        si = drain_inst.ins.sync_info
        waits = list(si.on_wait or []) if si else []
        if len(waits) > 1:
            si.on_wait = waits[:1]
            for w in waits[1:]:
                n = self.nc.sync.nop(nofuse=True).ins
                n.sync_info = mybir.SyncInfo(on_wait=[w], on_update=[])
        self.nc.all_engine_barrier()
        assert self.sems is not None
        popped = self.nc._tile_sem_poison_stack.pop()
        assert popped is self._sem_poison
        self.nc.clear_and_free_semaphores(list(self.sems.allocated().values()))
        self.nc.all_engine_barrier()


def _split_excess_waits(nc, max_waits=1):
    """Walrus in this container caps sync-wait commands per instruction.
    Move excess waits onto same-engine NoOps inserted just before."""
    for f in nc.m.functions:
        for bb in f.blocks:
            new_insts = []
            for inst in bb.instructions:
                si = inst.sync_info
                waits = list(si.on_wait) if si and si.on_wait else []
                if len(waits) > max_waits:
                    si.on_wait = waits[:max_waits]
                    rest = waits[max_waits:]
                    for i in range(0, len(rest), max_waits):
                        nop = mybir.InstEventSemaphore(
                            name=f"I-ws{nc.next_id()}", ins=[], outs=[])
                        nop.engine = inst.engine
                        nop.sync_info = mybir.SyncInfo(
                            on_wait=rest[i:i + max_waits], on_update=[])
                        nc.register_instruction(nop)
                        new_insts.append(nop)
                new_insts.append(inst)
            bb.instructions[:] = new_insts


def build_nc(skip_b1=True, skip_b2=True, skip_lnb=True,
             skip_lng=True):
    nc = bass.Bass()

    x_in = nc.dram_tensor("x16", [128, 2, SPC, Fd], DT.bfloat16,
                          kind="ExternalInput")
    out_d = nc.dram_tensor("out", [128, 2, SPC, Fd], DT.float32,
                           kind="ExternalOutput")
    wqk_d = nc.dram_tensor("wqk", [128, 2, 512], DT.bfloat16, kind="ExternalInput")
    wv3_d = nc.dram_tensor("wv3", [128, 2, 3, 256], DT.bfloat16, kind="ExternalInput")
    wproj_d = nc.dram_tensor("wproj", [128, 2, 256], DT.bfloat16, kind="ExternalInput")
    w1_d = nc.dram_tensor("w1", [128, 2, HID], DT.bfloat16, kind="ExternalInput")
    w2_d = nc.dram_tensor("w2", [128, 4, 256], DT.bfloat16, kind="ExternalInput")
    vec_d = nc.dram_tensor("vecs", [128, 2, 6], DT.float32, kind="ExternalInput")
    b1_d = nc.dram_tensor("b1v", [128, 4], DT.float32, kind="ExternalInput")
    taps_d = nc.dram_tensor("tapsqk", [128, 4, 3], DT.float32, kind="ExternalInput")
    ones_d = nc.dram_tensor("ones_b", [128, 128], DT.bfloat16, kind="ExternalInput")
    bandq_d = nc.dram_tensor("bandq", [128, 128], DT.bfloat16, kind="ExternalInput")
    bandk_d = nc.dram_tensor("bandk", [128, 2, 128], DT.bfloat16, kind="ExternalInput")

    with _TileContext(nc) as tc, contextlib.ExitStack() as ctx:
        cpool = ctx.enter_context(tc.tile_pool(name="consts", bufs=1))
        xpool = ctx.enter_context(tc.tile_pool(name="xp", bufs=2))
        xnpool = ctx.enter_context(tc.tile_pool(name="xnp", bufs=2))
        qrawp = ctx.enter_context(tc.tile_pool(name="qraw", bufs=2))
        qcp = ctx.enter_context(tc.tile_pool(name="qcp", bufs=1))
        wp = ctx.enter_context(tc.tile_pool(name="wp", bufs=3))
        php = ctx.enter_context(tc.tile_pool(name="php", bufs=4))
        ohp = ctx.enter_context(tc.tile_pool(name="ohp", bufs=1))
        o1p = ctx.enter_context(tc.tile_pool(name="o1p", bufs=1))
        xhp = ctx.enter_context(tc.tile_pool(name="xhp", bufs=1))
        gp = ctx.enter_context(tc.tile_pool(name="gp", bufs=1))
        op = ctx.enter_context(tc.tile_pool(name="op", bufs=1))
        ps = ctx.enter_context(tc.tile_pool(name="ps", bufs=2, space="PSUM"))

        def cload(name, shape, dt, dram):
            t = cpool.tile(shape, dt, tag=name)
            nc.sync.dma_start(t[:], dram[:])
            return t

        wqk = cload("wqk", [128, 2, 512], DT.bfloat16, wqk_d)
        wv3 = cload("wv3", [128, 2, 3, 256], DT.bfloat16, wv3_d)
        wproj = cload("wproj", [128, 2, 256], DT.bfloat16, wproj_d)
        w1 = cload("w1", [128, 2, HID], DT.bfloat16, w1_d)
        w2 = cload("w2", [128, 4, 256], DT.bfloat16, w2_d)
        vecs = cload("vecs", [128, 2, 6], DT.float32, vec_d)
        b1v = cload("b1v", [128, 4], DT.float32, b1_d)
        tapsqk = cload("tapsqk", [128, 4, 3], DT.float32, taps_d)
        ones_b = cload("ones_b", [128, 128], DT.bfloat16, ones_d)
        bandq = cload("bandq", [128, 128], DT.bfloat16, bandq_d)
        bandk = cload("bandk", [128, 2, 128], DT.bfloat16, bandk_d)

        # v' composite lhsT tiles (single set; the framework serializes
        # next-block drains behind this block's attn@v reads):
        # layout (128 fk, S, pair, slot, 128); slot0 = [vA |0| 1 |0],
        # slot1 = [0| vB |0| 1].
        vpx = [cpool.tile([128, S, 4, 2, 128], DT.bfloat16,
                          name=f"vpx_{fc}", tag=f"vpx_{fc}")
               for fc in range(2)]
        for fc in range(2):
            nc.vector.memset(vpx[fc][:], 0.0)
            nc.vector.memset(vpx[fc][:, :, :, 0, 64:96], 1.0)
            nc.vector.memset(vpx[fc][:, :, :, 1, 96:128], 1.0)

        def rsqrt_rep(src_ap, out_ap, eps_ap):
            """out = 1/sqrt(src + eps), elementwise, via ACT Ln + Exp.
            src may be PSUM fp32; out is bf16 SBUF (same partitions)."""
            t = wp.tile([128, W], DT.bfloat16, name="lnt", tag="wk")
            nc.scalar.activation(t[: src_ap.shape[0]], src_ap, AF.Ln,
                                 bias=eps_ap, scale=1.0)
            nc.scalar.activation(out_ap, t[: src_ap.shape[0]], AF.Exp,
                                 scale=-0.5)

        def layernorm(src_f, g_cols, b_cols, out_aps, skip_b, skip_g,
                      relaxed):
            ve = nc.vector
            """src_f: two bf16 (128, 2048) APs (channel-major, kt split).
            Writes bf16 out_aps. LN over the 256 channels (both kt tiles).
            Fast-evicts the psum stat tiles to bf16 so psum frees early."""
            ps_s = ps.tile([128, W], DT.float32, tag="mm")
            ps_q = ps.tile([128, W], DT.float32, tag="mm")
            for kt in range(2):
                for ch in range(NCH):
                    sl = slice(ch * 512, (ch + 1) * 512)
                    nc.tensor.matmul(
                        ps_s[:, sl], ones_b[:], src_f[kt][:, sl],
                        start=(kt == 0), stop=(kt == 1), skip_group_check=True)
            for kt in range(2):
                sq = wp.tile([128, W], DT.bfloat16, name="lnsq", tag="lnsq",
                             bufs=1)
                ve.tensor_mul(sq[:], src_f[kt], src_f[kt])
                for ch in range(NCH):
                    sl = slice(ch * 512, (ch + 1) * 512)
                    nc.tensor.matmul(
                        ps_q[:, sl], ones_b[:], sq[:, sl],
                        start=(kt == 0), stop=(kt == 1), skip_group_check=True)
            mu16 = wp.tile([128, W], DT.bfloat16, name="mu16", tag="lnmu",
                           bufs=2)
            ve.tensor_scalar(mu16[:], ps_s[:], 1.0 / C, None, ALU.mult)
            q16 = wp.tile([128, W], DT.bfloat16, name="q16", tag="wk")
            nc.scalar.activation(q16[:], ps_q[:], AF.Copy, bias=0.0,
                                 scale=1.0 / C)
            mu2 = wp.tile([128, W], DT.bfloat16, name="mu2", tag="wk")
            ve.tensor_mul(mu2[:], mu16[:], mu16[:])
            var = wp.tile([128, W], DT.bfloat16, name="var", tag="wk")
            ve.tensor_sub(var[:], q16[:], mu2[:])
            rsig = wp.tile([128, W], DT.bfloat16, name="rsg1", tag="lnrs",
                           bufs=2)
            rsqrt_rep(var[:], rsig[:], vecs[:, 0, 5:6])
            for kt in range(2):
                if skip_g:
                    gr = rsig
                else:
                    gr = wp.tile([128, W], DT.bfloat16, name="lngr", tag="wk")
                    nc.vector.tensor_scalar(gr[:], rsig[:], g_cols[kt], None,
                                            ALU.mult)
                a = wp.tile([128, W], DT.bfloat16, name="lna", tag="wk")
                ve.tensor_sub(a[:], src_f[kt], mu16[:])
                if skip_b:
                    nc.vector.tensor_mul(out_aps[kt], a[:], gr[:])
                else:
                    t = wp.tile([128, W], DT.bfloat16, name="lnb", tag="wk")
                    nc.vector.tensor_mul(t[:], a[:], gr[:])
                    nc.vector.tensor_scalar(
                        out_aps[kt], t[:], b_cols[kt], None, ALU.add)

        # ---------------- staged, software-pipelined blocks ----------------
        st = [dict() for _ in range(NBLK)]

        def dma_x(blk):
            d = st[blk]
            d["x16"] = [xpool.tile([128, S, Fd], DT.bfloat16,
                                   name=f"x16_{kt}", tag=f"x16_{kt}")
                        for kt in range(2)]
            for kt in range(2):
                nc.sync.dma_start(
                    d["x16"][kt][:], x_in[:, kt, blk * S:(blk + 1) * S, :])
            d["xf"] = [t[:].rearrange("p s f -> p (s f)") for t in d["x16"]]

        def s1_ln1(blk):
            d = st[blk]
            xn = [xnpool.tile([128, S, PITCH], DT.bfloat16, name=f"xn{kt}",
                              tag=f"xn{kt}") for kt in range(2)]
            for kt in range(2):
                nc.vector.memset(xn[kt][:, :, 0:1], 0.0)
                nc.vector.memset(xn[kt][:, :, PITCH - 1:PITCH], 0.0)
            xn_data = [xn[kt][:, :, 1:1 + Fd] for kt in range(2)]
            layernorm(d["xf"], [vecs[:, kt, 0:1] for kt in range(2)],
                      [vecs[:, kt, 1:2] for kt in range(2)],
                      xn_data, skip_lnb, skip_lng, relaxed=True)
            d["xn"] = xn
            d["xn_rhs"] = xn_data

        def s2_qkv(blk):
            d = st[blk]
            d["qc"] = []
            for m in range(4):
                ps_m = ps.tile([128, W], DT.float32, tag="mm")
                for kt in range(2):
                    for ch in range(NCH):
                        nc.tensor.matmul(
                            ps_m[:, ch * 512:(ch + 1) * 512],
                            wqk[:, kt, m * 128:(m + 1) * 128],
                            d["xn_rhs"][kt][:, 2 * ch:2 * ch + 2, :],
                            start=(kt == 0), stop=(kt == 1),
                            skip_group_check=True)
                qr = qrawp.tile([128, S, PITCH], DT.bfloat16, name=f"qr{m}",
                                tag="qraw")
                nc.gpsimd.memset(qr[:, :, 0:1], 0.0)
                nc.gpsimd.memset(qr[:, :, PITCH - 1:PITCH], 0.0)
                nc.scalar.copy(qr[:, :, 1:1 + Fd],
                               ps_m[:].rearrange("p (s f) -> p s f", s=S))
                t0 = wp.tile([128, S, Fd], DT.bfloat16, name="cv0", tag="wk")
                nc.vector.tensor_scalar(
                    t0[:], qr[:, :, 0:Fd], tapsqk[:, m, 0:1], None, ALU.mult)
                t1 = wp.tile([128, S, Fd], DT.bfloat16, name="cv1", tag="wk")
                nc.vector.tensor_scalar(
                    t1[:], qr[:, :, 1:1 + Fd], tapsqk[:, m, 1:2], None,
                    ALU.mult)
                t2 = wp.tile([128, S, Fd], DT.bfloat16, name="cv2", tag="wk")
                nc.vector.tensor_scalar(
                    t2[:], qr[:, :, 2:2 + Fd], tapsqk[:, m, 2:3], None,
                    ALU.mult)
                nc.vector.tensor_add(t0[:], t0[:], t1[:])
                q = qcp.tile([128, S, Fd], DT.bfloat16, name=f"qc{m}",
                             tag=f"qc{m}")
                nc.vector.tensor_add(q[:], t0[:], t2[:])
                d["qc"].append(q)

        def s3_vprime(blk):
            d = st[blk]
            for fc in range(2):
                vp_ps = ps.tile([128, S, 256], DT.float32, tag="mm")
                for s in range(S):
                    first = True
                    for kt in range(2):
                        for tap in range(3):
                            nc.tensor.matmul(
                                vp_ps[:, s, :],
                                d["xn"][kt][:, s,
                                            fc * 128 + tap:fc * 128 + tap + 128],
                                wv3[:, kt, tap, :],
                                start=first, stop=(kt == 1 and tap == 2),
                                skip_group_check=True)
                            first = False
                vsrc = vp_ps[:].rearrange("p s (h2 two d) -> p s h2 two d",
                                          two=2, d=Dh)
                nc.scalar.copy(vpx[fc][:, :, :, 0, 0:32],
                               vsrc[:, :, :, 0, :])
                nc.vector.tensor_copy(vpx[fc][:, :, :, 1, 32:64],
                                      vsrc[:, :, :, 1, :])

        def s4_l2(blk):
            d = st[blk]
            for m in (0, 2, 1, 3):
                qf = d["qc"][m][:].rearrange("p s f -> p (s f)")
                sqm = wp.tile([128, W], DT.bfloat16, name="sqm",
                              tag="l2sq", bufs=2)
                nc.vector.tensor_mul(sqm[:], qf, qf)
                ps_n = ps.tile([128, W], DT.float32, tag="mm")
                band = bandq[:] if m < 2 else bandk[:, m - 2, :]
                for ch in range(NCH):
                    sl = slice(ch * 512, (ch + 1) * 512)
                    nc.tensor.matmul(ps_n[:, sl], band, sqm[:, sl],
                                     start=True, stop=True,
                                     skip_group_check=True)
                rsig = wp.tile([128, W], DT.bfloat16, name="rsg2", tag="lnrs",
                               bufs=2)
                if m % 2 == 0:
                    rsqrt_rep(ps_n[:], rsig[:], vecs[:, 1, 5:6])
                else:
                    sig = wp.tile([128, W], DT.bfloat16, name="sig", tag="wk")
                    nc.scalar.activation(sig[:], ps_n[:], AF.Sqrt,
                                         bias=vecs[:, 1, 5:6], scale=1.0)
                    with nc.allow_low_precision(reason="l2 rsig bf16"):
                        nc.vector.reciprocal(rsig[:], sig[:])
                nc.vector.tensor_mul(qf, qf, rsig[:])

        def s5_attn(blk):
            d = st[blk]
            qc = d["qc"]
            phat = {}
            oh = [ohp.tile([128, W], DT.bfloat16, name=f"oh{kt}",
                           tag=f"oh{kt}") for kt in range(2)]
            for h in range(H):
                ti, band = h // 4, (h % 4) * 32
                for jt in range(2):
                    ps_S = ps.tile([128, W], DT.float32, tag="mm")
                    for s in range(S):
                        nc.tensor.matmul(
                            ps_S[:, s * Fd:(s + 1) * Fd],
                            qc[2 + ti][band:band + 32, s,
                                       jt * 128:jt * 128 + 128],
                            qc[ti][band:band + 32, s, :],
                            start=True, stop=True, skip_group_check=True,
                            tile_position=(band, 0))
                    pj = php.tile([128, W], DT.bfloat16, tag="phat")
                    nc.scalar.activation(pj[:, 0:W // 2], ps_S[:, 0:W // 2],
                                         AF.Exp)
                    nc.scalar.activation(pj[:, W // 2:W], ps_S[:, W // 2:W],
                                         AF.Exp)
                    phat[(h, jt)] = pj
                if h % 2 == 1:
                    p = h // 2
                    pair_ps = ps.tile([128, W], DT.float32, tag="mm")
                    for s in range(S):
                        osl = pair_ps[:, s * Fd:(s + 1) * Fd]
                        first = True
                        for sl, hh in ((0, h - 1), (1, h)):
                            for fc in range(2):
                                nc.tensor.matmul(
                                    osl, vpx[fc][:, s, p, sl, :],
                                    phat[(hh, fc)][:, s * Fd:(s + 1) * Fd],
                                    start=first,
                                    stop=(sl == 1 and fc == 1),
                                    skip_group_check=True)
                                first = False
                    pnum = wp.tile([128, W], DT.bfloat16, name="pnum",
                                   tag="lnsq", bufs=1)
                    nc.vector.tensor_copy(pnum[:], pair_ps[:])
                    rinv = wp.tile([64, W], DT.bfloat16, tag="rinv", bufs=1)
                    with nc.allow_low_precision(reason="softmax denom bf16"):
                        nc.vector.reciprocal(rinv[:], pnum[64:128, :])
                    nc.vector.tensor_mul(
                        oh[p // 2][(p % 2) * 64:(p % 2) * 64 + 64, :],
                        pnum[0:64, :], rinv[:])
                    del phat[(h - 1, 0)], phat[(h - 1, 1)]
                    del phat[(h, 0)], phat[(h, 1)]
            d["oh"] = oh

        def s6_proj(blk):
            d = st[blk]
            d["o1"] = []
            for m2 in range(2):
                ps_y = ps.tile([128, W], DT.float32, tag="mm")
                for kt in range(2):
                    for ch in range(NCH):
                        sl = slice(ch * 512, (ch + 1) * 512)
                        nc.tensor.matmul(
                            ps_y[:, sl],
                            wproj[:, kt, m2 * 128:(m2 + 1) * 128],
                            d["oh"][kt][:, sl], start=(kt == 0),
                            stop=(kt == 1), skip_group_check=True)
                o1t = o1p.tile([128, W], DT.bfloat16, name=f"o1_{m2}",
                               tag=f"o1_{m2}")
                nc.vector.tensor_add(o1t[:], ps_y[:], d["xf"][m2])
                d["o1"].append(o1t)

        def s7_ln2(blk):
            d = st[blk]
            xh = [xhp.tile([128, W], DT.bfloat16, name=f"xh{kt}",
                           tag=f"xh{kt}") for kt in range(2)]
            layernorm([t[:] for t in d["o1"]],
                      [vecs[:, kt, 2:3] for kt in range(2)],
                      [vecs[:, kt, 3:4] for kt in range(2)],
                      [t[:] for t in xh], skip_lnb, skip_lng, relaxed=False)
            d["xh"] = xh

        def s8_mlp1(blk):
            d = st[blk]
            d["g16"] = []
            for mh in range(4):
                ps_h = ps.tile([128, W], DT.float32, tag="mm")
                for kt in range(2):
                    for ch in range(NCH):
                        sl = slice(ch * 512, (ch + 1) * 512)
                        nc.tensor.matmul(
                            ps_h[:, sl], w1[:, kt, mh * 128:(mh + 1) * 128],
                            d["xh"][kt][:][:, sl], start=(kt == 0),
                            stop=(kt == 1), skip_group_check=True)
                g = gp.tile([128, W], DT.bfloat16, name=f"g{mh}", tag=f"g{mh}")
                if skip_b1:
                    nc.scalar.activation(g[:], ps_h[:], AF.Gelu)
                else:
                    nc.scalar.activation(g[:], ps_h[:], AF.Gelu,
                                         bias=b1v[:, mh:mh + 1], scale=1.0)
                d["g16"].append(g)

        def s9_mlp2(blk):
            d = st[blk]
            for m2 in range(2):
                ps_o = ps.tile([128, W], DT.float32, tag="mm")
                for kt in range(4):
                    for ch in range(NCH):
                        sl = slice(ch * 512, (ch + 1) * 512)
                        nc.tensor.matmul(
                            ps_o[:, sl], w2[:, kt, m2 * 128:(m2 + 1) * 128],
                            d["g16"][kt][:][:, sl], start=(kt == 0),
                            stop=(kt == 3), skip_group_check=True)
                o2 = op.tile([128, S, Fd], DT.float32, name="o2", tag="o2",
                             bufs=1)
                o2f = o2[:].rearrange("p s f -> p (s f)")
                if skip_b2:
                    nc.vector.tensor_add(o2f, ps_o[:], d["o1"][m2][:])
                else:
                    t = op.tile([128, W], DT.float32, name="o2t", tag="o2t")
                    nc.vector.tensor_add(t[:], ps_o[:], d["o1"][m2][:])
                    nc.vector.tensor_scalar(
                        o2f, t[:], vecs[:, m2, 4:5], None, ALU.add)
                nc.gpsimd.dma_start(
                    out_d[:, m2, blk * S:(blk + 1) * S, :], o2[:])

        # prologue (pipeline fill: LN1 runs two blocks ahead)
        dma_x(0)
        dma_x(1)
        s1_ln1(0)
        s1_ln1(1)
        s3_vprime(0)
        s2_qkv(0)
        s4_l2(0)
        for b in range(NBLK):
            s5_attn(b)
            s6_proj(b)
            s7_ln2(b)
            if b + 1 < NBLK:
                s3_vprime(b + 1)
                s2_qkv(b + 1)
            s8_mlp1(b)
            s9_mlp2(b)
            if b + 1 < NBLK:
                s4_l2(b + 1)
                if b + 2 < NBLK:
                    dma_x(b + 2)
                    s1_ln1(b + 2)

    _split_excess_waits(nc, max_waits=1)
    return nc


def _host_prep(inputs):
    Wqkv = np.asarray(inputs["Wqkv"], np.float32)        # (C, 3C)
    dw_w = np.asarray(inputs["dw_w"], np.float32)        # (3C, 1, 3)
    taps = dw_w[:, 0, :]                                 # (3C, 3)

    wqk = np.ascontiguousarray(
        Wqkv[:, :512].reshape(2, 128, 512).transpose(1, 0, 2)).astype(BF16)
    # wv3[p, kt, tap, c_out] = Wv[kt*128+p, c_out] * taps_v[c_out, tap]
    Wv = Wqkv[:, 512:]                                   # (C, 256)
    wv3 = np.einsum("co,ot->cto", Wv, taps[512:])        # (C, 3, 256)
    wv3 = np.ascontiguousarray(
        wv3.reshape(2, 128, 3, 256).transpose(1, 0, 2, 3)).astype(BF16)

    def kt_major(w, nkt):
        K, N = w.shape
        return np.ascontiguousarray(
            w.reshape(nkt, 128, N).transpose(1, 0, 2)).astype(BF16)

    wproj = kt_major(np.asarray(inputs["Wproj"], np.float32), 2)
    w1 = kt_major(np.asarray(inputs["W1"], np.float32), 2)
    w2 = kt_major(np.asarray(inputs["W2"], np.float32), 4)

    vecs = np.zeros((128, 2, 6), np.float32)
    for kt in range(2):
        sl = slice(kt * 128, (kt + 1) * 128)
        vecs[:, kt, 0] = np.asarray(inputs["norm1_g"], np.float32)[sl]
        vecs[:, kt, 1] = np.asarray(inputs["norm1_b"], np.float32)[sl]
        vecs[:, kt, 2] = np.asarray(inputs["norm2_g"], np.float32)[sl]
        vecs[:, kt, 3] = np.asarray(inputs["norm2_b"], np.float32)[sl]
        vecs[:, kt, 4] = np.asarray(inputs["b2"], np.float32)[sl]
    vecs[:, 0, 5] = LN_EPS
    vecs[:, 1, 5] = 1e-24
    b1v = np.ascontiguousarray(
        np.asarray(inputs["b1"], np.float32).reshape(4, 128).T)

    # conv taps for the q,k channel tiles: tapsqk[p, m, tap]
    tapsqk = np.ascontiguousarray(
        taps[:512].reshape(4, 128, 3).transpose(1, 0, 2)).astype(np.float32)

    temp = np.asarray(inputs["temperature"], np.float32).reshape(H)
    bandq = np.zeros((128, 128), np.float32)
    bandk = np.zeros((128, 2, 128), np.float32)
    for k in range(128):
        for m in range(128):
            if k // 32 == m // 32:
                bandq[k, m] = 1.0
                for ti in range(2):
                    h = ti * 4 + m // 32
                    bandk[k, ti, m] = 1.0 / (temp[h] * temp[h])

    return dict(
        wqk=wqk, wv3=wv3, wproj=wproj, w1=w1, w2=w2, vecs=vecs, b1v=b1v,
        tapsqk=tapsqk,
        ones_b=np.ones((128, 128), BF16),
        bandq=bandq.astype(BF16),
        bandk=bandk.astype(BF16),
    )


_NC_CACHE = {}


def get_nc(flags=(True, True, True, True)):
    if flags not in _NC_CACHE:
        _NC_CACHE[flags] = build_nc(*flags)
    return _NC_CACHE[flags]


def _flags_from(inputs):
    z = lambda k: bool(np.all(np.asarray(inputs[k]) == 0.0))
    o = lambda k: bool(np.all(np.asarray(inputs[k]) == 1.0))
    return (z("b1"), z("b2"), z("norm1_b") and z("norm2_b"),
            o("norm1_g") and o("norm2_g"))


def make_in_maps(inputs):
    consts = _host_prep(inputs)
    x = np.asarray(inputs["x"], np.float32)  # (B, C, T, Fd)
    in_maps = []
    for core in range(NCORES):
        b, t0 = core // 2, (core % 2) * SPC
        m = dict(consts)
        xs = x[b, :, t0:t0 + SPC, :]          # (C, SPC, Fd)
        m["x16"] = np.ascontiguousarray(
            xs.reshape(2, 128, SPC, Fd).transpose(1, 0, 2, 3)).astype(BF16)
        in_maps.append(m)
    return in_maps


def assemble_out(results):
    out = np.zeros((B, C, T, Fd), np.float32)
    for core in range(NCORES):
        b, t0 = core // 2, (core % 2) * SPC
        r = results[core]["out"]              # (128, 2, SPC, Fd)
        out[b, :, t0:t0 + SPC, :] = r.transpose(1, 0, 2, 3).reshape(
            C, SPC, Fd)
    return out


def kernel(**inputs):
    nc = get_nc(_flags_from(inputs))
    in_maps = make_in_maps(inputs)
    res = run_bass_kernel_spmd(nc, in_maps, core_ids=list(range(NCORES)))
    return assemble_out(res.results)


# revision 4
# speedup vs baseline: 1.7529x; 1.0001x over previous
"""Trainium2 Bass kernel for the AxisMDTA dense-transformer block (v2).

Shapes (hardcoded): x (4, 256, 64, 256) fp32 -> out (4, 256, 64, 256) fp32.
256 independent samples of (f=256, c=256): LN -> qkv -> depthwise conv3
along f -> 8-head attn (L2-normed q/k, temperature) -> proj + residual ->
LN -> MLP(gelu) + residual.

Data-parallel over the 256 (b,t) samples across 8 cores (32 each, in 4
blocks of 8). Channel-major on-chip layout (c on partitions, (sample, f)
on the free dim).  Key cost choices vs v1:
  - depthwise conv runs on DVE (4x-rate scalar_tensor_tensor on bf16
    guarded-pitch views) instead of tripling the qkv matmul work;
  - v' (f-major v for attn@v) comes straight from 3-tap matmuls with
    xn slices as the stationary operand - no PE transposes;
  - LN/L2 statistics matmuls emit band-replicated rows so rsqrt runs
    directly on the replicated tiles (Ln+Exp on ACT) - no dense-pack
    DMAs, no Newton iterations, no broadcast matmuls;
  - attn@v processes head pairs with an in-psum ones-block that yields
    the softmax denominators, normalized during the psum drain;
  - x is pre-cast to bf16 on the host, halving DMA and enabling 4x DVE.
"""

import contextlib

import numpy as np
import ml_dtypes

import concourse.bass as bass
import concourse.mybir as mybir
import concourse.tile as tile
from concourse.vector_clock import ScopedClock
from concourse.bass_utils import run_bass_kernel_spmd

AF = mybir.ActivationFunctionType
ALU = mybir.AluOpType
DT = mybir.dt
BF16 = ml_dtypes.bfloat16

# Problem constants
B, C, T, Fd = 4, 256, 64, 256
H, Dh = 8, 32
HID = 512
NCORES = 8
SPC = (B * T) // NCORES  # 32 samples per core
LN_EPS = 1e-5
S = 8                    # samples per block
NBLK = SPC // S
W = S * Fd               # 2048 free columns per block
PITCH = Fd + 2           # guarded pitch for conv views
NCH = W // 512           # 512-col psum chunks


class _TileContext(tile.TileContext):
    """Walrus in this container caps sync-wait commands per CTRL-class
    instruction; spread the exit drain's waits across single-wait nops."""

    def _drain_and_barrier(self, tick_clock, wait_clock):
        drain_inst = self.nc.sync.drain()
        wait_clock.add_sem_waits(
            drain_inst.ins, ScopedClock({None: tick_clock.global_clock})
        )
        si = drain_inst.ins.sync_info
        waits = list(si.on_wait or []) if si else []
        if len(waits) > 1:
            si.on_wait = waits[:1]
            for w in waits[1:]:
                n = self.nc.sync.nop(nofuse=True).ins
                n.sync_info = mybir.SyncInfo(on_wait=[w], on_update=[])
        self.nc.all_engine_barrier()
        assert self.sems is not None
        popped = self.nc._tile_sem_poison_stack.pop()
        assert popped is self._sem_poison
        self.nc.clear_and_free_semaphores(list(self.sems.allocated().values()))
        self.nc.all_engine_barrier()


def _split_excess_waits(nc, max_waits=1):
    """Walrus in this container caps sync-wait commands per instruction.
    Move excess waits onto same-engine NoOps inserted just before."""
    for f in nc.m.functions:
        for bb in f.blocks:
            new_insts = []
            for inst in bb.instructions:
                si = inst.sync_info
                waits = list(si.on_wait) if si and si.on_wait else []
                if len(waits) > max_waits:
                    si.on_wait = waits[:max_waits]
                    rest = waits[max_waits:]
                    for i in range(0, len(rest), max_waits):
                        nop = mybir.InstEventSemaphore(
                            name=f"I-ws{nc.next_id()}", ins=[], outs=[])
                        nop.engine = inst.engine
                        nop.sync_info = mybir.SyncInfo(
                            on_wait=rest[i:i + max_waits], on_update=[])
                        nc.register_instruction(nop)
                        new_insts.append(nop)
                new_insts.append(inst)
            bb.instructions[:] = new_insts


def build_nc(skip_b1=True, skip_b2=True, skip_lnb=True,
             skip_lng=True):
    nc = bass.Bass()

    x_in = nc.dram_tensor("x16", [128, 2, SPC, Fd], DT.bfloat16,
                          kind="ExternalInput")
    out_d = nc.dram_tensor("out", [128, 2, SPC, Fd], DT.float32,
                           kind="ExternalOutput")
    wqk_d = nc.dram_tensor("wqk", [128, 2, 512], DT.bfloat16, kind="ExternalInput")
    wv3_d = nc.dram_tensor("wv3", [128, 2, 3, 256], DT.bfloat16, kind="ExternalInput")
    wproj_d = nc.dram_tensor("wproj", [128, 2, 256], DT.bfloat16, kind="ExternalInput")
    w1_d = nc.dram_tensor("w1", [128, 2, HID], DT.bfloat16, kind="ExternalInput")
    w2_d = nc.dram_tensor("w2", [128, 4, 256], DT.bfloat16, kind="ExternalInput")
    vec_d = nc.dram_tensor("vecs", [128, 2, 6], DT.float32, kind="ExternalInput")
    b1_d = nc.dram_tensor("b1v", [128, 4], DT.float32, kind="ExternalInput")
    taps_d = nc.dram_tensor("tapsqk", [128, 4, 3], DT.float32, kind="ExternalInput")
    ones_d = nc.dram_tensor("ones_b", [128, 128], DT.bfloat16, kind="ExternalInput")
    bandq_d = nc.dram_tensor("bandq", [128, 128], DT.bfloat16, kind="ExternalInput")
    bandk_d = nc.dram_tensor("bandk", [128, 2, 128], DT.bfloat16, kind="ExternalInput")

    with _TileContext(nc) as tc, contextlib.ExitStack() as ctx:
        cpool = ctx.enter_context(tc.tile_pool(name="consts", bufs=1))
        xpool = ctx.enter_context(tc.tile_pool(name="xp", bufs=2))
        xnpool = ctx.enter_context(tc.tile_pool(name="xnp", bufs=2))
        qrawp = ctx.enter_context(tc.tile_pool(name="qraw", bufs=2))
        qcp = ctx.enter_context(tc.tile_pool(name="qcp", bufs=1))
        wp = ctx.enter_context(tc.tile_pool(name="wp", bufs=3))
        php = ctx.enter_context(tc.tile_pool(name="php", bufs=4))
        ohp = ctx.enter_context(tc.tile_pool(name="ohp", bufs=1))
        o1p = ctx.enter_context(tc.tile_pool(name="o1p", bufs=1))
        xhp = ctx.enter_context(tc.tile_pool(name="xhp", bufs=1))
        gp = ctx.enter_context(tc.tile_pool(name="gp", bufs=1))
        op = ctx.enter_context(tc.tile_pool(name="op", bufs=1))
        ps = ctx.enter_context(tc.tile_pool(name="ps", bufs=2, space="PSUM"))

        def cload(name, shape, dt, dram):
            t = cpool.tile(shape, dt, tag=name)
            nc.sync.dma_start(t[:], dram[:])
            return t

        ones_b = cload("ones_b", [128, 128], DT.bfloat16, ones_d)
        vecs = cload("vecs", [128, 2, 6], DT.float32, vec_d)
        wqk = cload("wqk", [128, 2, 512], DT.bfloat16, wqk_d)
        tapsqk = cload("tapsqk", [128, 4, 3], DT.float32, taps_d)
        wv3 = cload("wv3", [128, 2, 3, 256], DT.bfloat16, wv3_d)
        bandq = cload("bandq", [128, 128], DT.bfloat16, bandq_d)
        bandk = cload("bandk", [128, 2, 128], DT.bfloat16, bandk_d)
        wproj = cload("wproj", [128, 2, 256], DT.bfloat16, wproj_d)
        w1 = cload("w1", [128, 2, HID], DT.bfloat16, w1_d)
        w2 = cload("w2", [128, 4, 256], DT.bfloat16, w2_d)
        b1v = cload("b1v", [128, 4], DT.float32, b1_d)

        # v' composite lhsT tiles (single set; the framework serializes
        # next-block drains behind this block's attn@v reads):
        # layout (128 fk, S, pair, slot, 128); slot0 = [vA |0| 1 |0],
        # slot1 = [0| vB |0| 1].
        vpx = [cpool.tile([128, S, 4, 2, 128], DT.bfloat16,
                          name=f"vpx_{fc}", tag=f"vpx_{fc}")
               for fc in range(2)]
        for fc in range(2):
            nc.vector.memset(vpx[fc][:], 0.0)
            nc.vector.memset(vpx[fc][:, :, :, 0, 64:96], 1.0)
            nc.vector.memset(vpx[fc][:, :, :, 1, 96:128], 1.0)

        def rsqrt_rep(src_ap, out_ap, eps_ap):
            """out = 1/sqrt(src + eps), elementwise, via ACT Ln + Exp.
            src may be PSUM fp32; out is bf16 SBUF (same partitions)."""
            t = wp.tile([128, W], DT.bfloat16, name="lnt", tag="wk")
            nc.scalar.activation(t[: src_ap.shape[0]], src_ap, AF.Ln,
                                 bias=eps_ap, scale=1.0)
            nc.scalar.activation(out_ap, t[: src_ap.shape[0]], AF.Exp,
                                 scale=-0.5)

        def layernorm(src_f, g_cols, b_cols, out_aps, skip_b, skip_g,
                      relaxed):
            ve = nc.vector
            """src_f: two bf16 (128, 2048) APs (channel-major, kt split).
            Writes bf16 out_aps. LN over the 256 channels (both kt tiles).
            Fast-evicts the psum stat tiles to bf16 so psum frees early."""
            ps_s = ps.tile([128, W], DT.float32, tag="mm")
            ps_q = ps.tile([128, W], DT.float32, tag="mm")
            for kt in range(2):
                for ch in range(NCH):
                    sl = slice(ch * 512, (ch + 1) * 512)
                    nc.tensor.matmul(
                        ps_s[:, sl], ones_b[:], src_f[kt][:, sl],
                        start=(kt == 0), stop=(kt == 1), skip_group_check=True)
            for kt in range(2):
                sq = wp.tile([128, W], DT.bfloat16, name="lnsq", tag="lnsq",
                             bufs=1)
                ve.tensor_mul(sq[:], src_f[kt], src_f[kt])
                for ch in range(NCH):
                    sl = slice(ch * 512, (ch + 1) * 512)
                    nc.tensor.matmul(
                        ps_q[:, sl], ones_b[:], sq[:, sl],
                        start=(kt == 0), stop=(kt == 1), skip_group_check=True)
            mu16 = wp.tile([128, W], DT.bfloat16, name="mu16", tag="lnmu",
                           bufs=2)
            ve.tensor_scalar(mu16[:], ps_s[:], 1.0 / C, None, ALU.mult)
            q16 = wp.tile([128, W], DT.bfloat16, name="q16", tag="wk")
            nc.scalar.activation(q16[:], ps_q[:], AF.Copy, bias=0.0,
                                 scale=1.0 / C)
            mu2 = wp.tile([128, W], DT.bfloat16, name="mu2", tag="wk")
            ve.tensor_mul(mu2[:], mu16[:], mu16[:])
            var = wp.tile([128, W], DT.bfloat16, name="var", tag="wk")
            ve.tensor_sub(var[:], q16[:], mu2[:])
            rsig = wp.tile([128, W], DT.bfloat16, name="rsg1", tag="lnrs",
                           bufs=2)
            rsqrt_rep(var[:], rsig[:], vecs[:, 0, 5:6])
            for kt in range(2):
                if skip_g:
                    gr = rsig
                else:
                    gr = wp.tile([128, W], DT.bfloat16, name="lngr", tag="wk")
                    nc.vector.tensor_scalar(gr[:], rsig[:], g_cols[kt], None,
                                            ALU.mult)
                a = wp.tile([128, W], DT.bfloat16, name="lna", tag="wk")
                ve.tensor_sub(a[:], src_f[kt], mu16[:])
                if skip_b:
                    nc.vector.tensor_mul(out_aps[kt], a[:], gr[:])
                else:
                    t = wp.tile([128, W], DT.bfloat16, name="lnb", tag="wk")
                    nc.vector.tensor_mul(t[:], a[:], gr[:])
                    nc.vector.tensor_scalar(
                        out_aps[kt], t[:], b_cols[kt], None, ALU.add)

        # ---------------- staged, software-pipelined blocks ----------------
        st = [dict() for _ in range(NBLK)]

        def dma_x(blk):
            d = st[blk]
            d["x16"] = [xpool.tile([128, S, Fd], DT.bfloat16,
                                   name=f"x16_{kt}", tag=f"x16_{kt}")
                        for kt in range(2)]
            for kt in range(2):
                nc.sync.dma_start(
                    d["x16"][kt][:], x_in[:, kt, blk * S:(blk + 1) * S, :])
            d["xf"] = [t[:].rearrange("p s f -> p (s f)") for t in d["x16"]]

        def s1_ln1(blk):
            d = st[blk]
            xn = [xnpool.tile([128, S, PITCH], DT.bfloat16, name=f"xn{kt}",
                              tag=f"xn{kt}") for kt in range(2)]
            for kt in range(2):
                nc.vector.memset(xn[kt][:, :, 0:1], 0.0)
                nc.vector.memset(xn[kt][:, :, PITCH - 1:PITCH], 0.0)
            xn_data = [xn[kt][:, :, 1:1 + Fd] for kt in range(2)]
            layernorm(d["xf"], [vecs[:, kt, 0:1] for kt in range(2)],
                      [vecs[:, kt, 1:2] for kt in range(2)],
                      xn_data, skip_lnb, skip_lng, relaxed=True)
            d["xn"] = xn
            d["xn_rhs"] = xn_data

        def s2_qkv(blk):
            d = st[blk]
            d["qc"] = []
            for m in range(4):
                ps_m = ps.tile([128, W], DT.float32, tag="mm")
                for kt in range(2):
                    for ch in range(NCH):
                        nc.tensor.matmul(
                            ps_m[:, ch * 512:(ch + 1) * 512],
                            wqk[:, kt, m * 128:(m + 1) * 128],
                            d["xn_rhs"][kt][:, 2 * ch:2 * ch + 2, :],
                            start=(kt == 0), stop=(kt == 1),
                            skip_group_check=True)
                qr = qrawp.tile([128, S, PITCH], DT.bfloat16, name=f"qr{m}",
                                tag="qraw")
                nc.gpsimd.memset(qr[:, :, 0:1], 0.0)
                nc.gpsimd.memset(qr[:, :, PITCH - 1:PITCH], 0.0)
                nc.scalar.copy(qr[:, :, 1:1 + Fd],
                               ps_m[:].rearrange("p (s f) -> p s f", s=S))
                t0 = wp.tile([128, S, Fd], DT.bfloat16, name="cv0", tag="wk")
                nc.vector.tensor_scalar(
                    t0[:], qr[:, :, 0:Fd], tapsqk[:, m, 0:1], None, ALU.mult)
                t1 = wp.tile([128, S, Fd], DT.bfloat16, name="cv1", tag="wk")
                nc.vector.tensor_scalar(
                    t1[:], qr[:, :, 1:1 + Fd], tapsqk[:, m, 1:2], None,
                    ALU.mult)
                t2 = wp.tile([128, S, Fd], DT.bfloat16, name="cv2", tag="wk")
                nc.vector.tensor_scalar(
                    t2[:], qr[:, :, 2:2 + Fd], tapsqk[:, m, 2:3], None,
                    ALU.mult)
                nc.vector.tensor_add(t0[:], t0[:], t1[:])
                q = qcp.tile([128, S, Fd], DT.bfloat16, name=f"qc{m}",
                             tag=f"qc{m}")
                nc.vector.tensor_add(q[:], t0[:], t2[:])
                d["qc"].append(q)

        def s3_vprime(blk):
            d = st[blk]
            for fc in range(2):
                vp_ps = ps.tile([128, S, 256], DT.float32, tag="mm")
                for s in range(S):
                    first = True
                    for kt in range(2):
                        for tap in range(3):
                            nc.tensor.matmul(
                                vp_ps[:, s, :],
                                d["xn"][kt][:, s,
                                            fc * 128 + tap:fc * 128 + tap + 128],
                                wv3[:, kt, tap, :],
                                start=first, stop=(kt == 1 and tap == 2),
                                skip_group_check=True)
                            first = False
                vsrc = vp_ps[:].rearrange("p s (h2 two d) -> p s h2 two d",
                                          two=2, d=Dh)
                nc.scalar.copy(vpx[fc][:, :, :, 0, 0:32],
                               vsrc[:, :, :, 0, :])
                nc.vector.tensor_copy(vpx[fc][:, :, :, 1, 32:64],
                                      vsrc[:, :, :, 1, :])

        def s4_l2(blk):
            d = st[blk]
            for m in (0, 2, 1, 3):
                qf = d["qc"][m][:].rearrange("p s f -> p (s f)")
                sqm = wp.tile([128, W], DT.bfloat16, name="sqm",
                              tag="l2sq", bufs=2)
                nc.vector.tensor_mul(sqm[:], qf, qf)
                ps_n = ps.tile([128, W], DT.float32, tag="mm")
                band = bandq[:] if m < 2 else bandk[:, m - 2, :]
                for ch in range(NCH):
                    sl = slice(ch * 512, (ch + 1) * 512)
                    nc.tensor.matmul(ps_n[:, sl], band, sqm[:, sl],
                                     start=True, stop=True,
                                     skip_group_check=True)
                rsig = wp.tile([128, W], DT.bfloat16, name="rsg2", tag="lnrs",
                               bufs=2)
                if m % 2 == 0:
                    rsqrt_rep(ps_n[:], rsig[:], vecs[:, 1, 5:6])
                else:
                    sig = wp.tile([128, W], DT.bfloat16, name="sig", tag="wk")
                    nc.scalar.activation(sig[:], ps_n[:], AF.Sqrt,
                                         bias=vecs[:, 1, 5:6], scale=1.0)
                    with nc.allow_low_precision(reason="l2 rsig bf16"):
                        nc.vector.reciprocal(rsig[:], sig[:])
                nc.vector.tensor_mul(qf, qf, rsig[:])

        def s5_attn(blk):
            d = st[blk]
            qc = d["qc"]
            phat = {}
            oh = [ohp.tile([128, W], DT.bfloat16, name=f"oh{kt}",
                           tag=f"oh{kt}") for kt in range(2)]
            for h in range(H):
                ti, band = h // 4, (h % 4) * 32
                for jt in range(2):
                    ps_S = ps.tile([128, W], DT.float32, tag="mm")
                    for s in range(S):
                        nc.tensor.matmul(
                            ps_S[:, s * Fd:(s + 1) * Fd],
                            qc[2 + ti][band:band + 32, s,
                                       jt * 128:jt * 128 + 128],
                            qc[ti][band:band + 32, s, :],
                            start=True, stop=True, skip_group_check=True,
                            tile_position=(band, 0))
                    pj = php.tile([128, W], DT.bfloat16, tag="phat")
                    nc.scalar.activation(pj[:, 0:W // 2], ps_S[:, 0:W // 2],
                                         AF.Exp)
                    nc.scalar.activation(pj[:, W // 2:W], ps_S[:, W // 2:W],
                                         AF.Exp)
                    phat[(h, jt)] = pj
                if h % 2 == 1:
                    p = h // 2
                    pair_ps = ps.tile([128, W], DT.float32, tag="mm")
                    for s in range(S):
                        osl = pair_ps[:, s * Fd:(s + 1) * Fd]
                        first = True
                        for sl, hh in ((0, h - 1), (1, h)):
                            for fc in range(2):
                                nc.tensor.matmul(
                                    osl, vpx[fc][:, s, p, sl, :],
                                    phat[(hh, fc)][:, s * Fd:(s + 1) * Fd],
                                    start=first,
                                    stop=(sl == 1 and fc == 1),
                                    skip_group_check=True)
                                first = False
                    pnum = wp.tile([128, W], DT.bfloat16, name="pnum",
                                   tag="lnsq", bufs=1)
                    nc.vector.tensor_copy(pnum[:], pair_ps[:])
                    rinv = wp.tile([64, W], DT.bfloat16, tag="rinv", bufs=1)
                    with nc.allow_low_precision(reason="softmax denom bf16"):
                        nc.vector.reciprocal(rinv[:], pnum[64:128, :])
                    nc.vector.tensor_mul(
                        oh[p // 2][(p % 2) * 64:(p % 2) * 64 + 64, :],
                        pnum[0:64, :], rinv[:])
                    del phat[(h - 1, 0)], phat[(h - 1, 1)]
                    del phat[(h, 0)], phat[(h, 1)]
            d["oh"] = oh

        def s6_proj(blk):
            d = st[blk]
            d["o1"] = []
            for m2 in range(2):
                ps_y = ps.tile([128, W], DT.float32, tag="mm")
                for kt in range(2):
                    for ch in range(NCH):
                        sl = slice(ch * 512, (ch + 1) * 512)
                        nc.tensor.matmul(
                            ps_y[:, sl],
                            wproj[:, kt, m2 * 128:(m2 + 1) * 128],
                            d["oh"][kt][:, sl], start=(kt == 0),
                            stop=(kt == 1), skip_group_check=True)
                o1t = o1p.tile([128, W], DT.bfloat16, name=f"o1_{m2}",
                               tag=f"o1_{m2}")
                nc.vector.tensor_add(o1t[:], ps_y[:], d["xf"][m2])
                d["o1"].append(o1t)

        def s7_ln2(blk):
            d = st[blk]
            xh = [xhp.tile([128, W], DT.bfloat16, name=f"xh{kt}",
                           tag=f"xh{kt}") for kt in range(2)]
            layernorm([t[:] for t in d["o1"]],
                      [vecs[:, kt, 2:3] for kt in range(2)],
                      [vecs[:, kt, 3:4] for kt in range(2)],
                      [t[:] for t in xh], skip_lnb, skip_lng, relaxed=False)
            d["xh"] = xh

        def s8_mlp1(blk):
            d = st[blk]
            d["g16"] = []
            for mh in range(4):
                ps_h = ps.tile([128, W], DT.float32, tag="mm")
                for kt in range(2):
                    for ch in range(NCH):
                        sl = slice(ch * 512, (ch + 1) * 512)
                        nc.tensor.matmul(
                            ps_h[:, sl], w1[:, kt, mh * 128:(mh + 1) * 128],
                            d["xh"][kt][:][:, sl], start=(kt == 0),
                            stop=(kt == 1), skip_group_check=True)
                g = gp.tile([128, W], DT.bfloat16, name=f"g{mh}", tag=f"g{mh}")
                if skip_b1:
                    nc.scalar.activation(g[:], ps_h[:], AF.Gelu)
                else:
                    nc.scalar.activation(g[:], ps_h[:], AF.Gelu,
                                         bias=b1v[:, mh:mh + 1], scale=1.0)
                d["g16"].append(g)

        def s9_mlp2(blk):
            d = st[blk]
            for m2 in range(2):
                ps_o = ps.tile([128, W], DT.float32, tag="mm")
                for kt in range(4):
                    for ch in range(NCH):
                        sl = slice(ch * 512, (ch + 1) * 512)
                        nc.tensor.matmul(
                            ps_o[:, sl], w2[:, kt, m2 * 128:(m2 + 1) * 128],
                            d["g16"][kt][:][:, sl], start=(kt == 0),
                            stop=(kt == 3), skip_group_check=True)
                o2 = op.tile([128, S, Fd], DT.float32, name="o2", tag="o2",
                             bufs=1)
                o2f = o2[:].rearrange("p s f -> p (s f)")
                if skip_b2:
                    nc.vector.tensor_add(o2f, ps_o[:], d["o1"][m2][:])
                else:
                    t = op.tile([128, W], DT.float32, name="o2t", tag="o2t")
                    nc.vector.tensor_add(t[:], ps_o[:], d["o1"][m2][:])
                    nc.vector.tensor_scalar(
                        o2f, t[:], vecs[:, m2, 4:5], None, ALU.add)
                nc.gpsimd.dma_start(
                    out_d[:, m2, blk * S:(blk + 1) * S, :], o2[:])

        # prologue (pipeline fill: LN1 runs two blocks ahead)
        dma_x(0)
        dma_x(1)
        s1_ln1(0)
        s1_ln1(1)
        s3_vprime(0)
        s2_qkv(0)
        s4_l2(0)
        for b in range(NBLK):
            s5_attn(b)
            s6_proj(b)
            s7_ln2(b)
            if b + 1 < NBLK:
                s3_vprime(b + 1)
                s2_qkv(b + 1)
            s8_mlp1(b)
            s9_mlp2(b)
            if b + 1 < NBLK:
                s4_l2(b + 1)
                if b + 2 < NBLK:
                    dma_x(b + 2)
                    s1_ln1(b + 2)

    _split_excess_waits(nc, max_waits=1)
    return nc


def _host_prep(inputs):
    Wqkv = np.asarray(inputs["Wqkv"], np.float32)        # (C, 3C)
    dw_w = np.asarray(inputs["dw_w"], np.float32)        # (3C, 1, 3)
    taps = dw_w[:, 0, :]                                 # (3C, 3)

    wqk = np.ascontiguousarray(
        Wqkv[:, :512].reshape(2, 128, 512).transpose(1, 0, 2)).astype(BF16)
    # wv3[p, kt, tap, c_out] = Wv[kt*128+p, c_out] * taps_v[c_out, tap]
    Wv = Wqkv[:, 512:]                                   # (C, 256)
    wv3 = np.einsum("co,ot->cto", Wv, taps[512:])        # (C, 3, 256)
    wv3 = np.ascontiguousarray(
        wv3.reshape(2, 128, 3, 256).transpose(1, 0, 2, 3)).astype(BF16)

    def kt_major(w, nkt):
        K, N = w.shape
        return np.ascontiguousarray(
            w.reshape(nkt, 128, N).transpose(1, 0, 2)).astype(BF16)

    wproj = kt_major(np.asarray(inputs["Wproj"], np.float32), 2)
    w1 = kt_major(np.asarray(inputs["W1"], np.float32), 2)
    w2 = kt_major(np.asarray(inputs["W2"], np.float32), 4)

    vecs = np.zeros((128, 2, 6), np.float32)
    for kt in range(2):
        sl = slice(kt * 128, (kt + 1) * 128)
        vecs[:, kt, 0] = np.asarray(inputs["norm1_g"], np.float32)[sl]
        vecs[:, kt, 1] = np.asarray(inputs["norm1_b"], np.float32)[sl]
        vecs[:, kt, 2] = np.asarray(inputs["norm2_g"], np.float32)[sl]
        vecs[:, kt, 3] = np.asarray(inputs["norm2_b"], np.float32)[sl]
        vecs[:, kt, 4] = np.asarray(inputs["b2"], np.float32)[sl]
    vecs[:, 0, 5] = LN_EPS
    vecs[:, 1, 5] = 1e-24
    b1v = np.ascontiguousarray(
        np.asarray(inputs["b1"], np.float32).reshape(4, 128).T)

    # conv taps for the q,k channel tiles: tapsqk[p, m, tap]
    tapsqk = np.ascontiguousarray(
        taps[:512].reshape(4, 128, 3).transpose(1, 0, 2)).astype(np.float32)

    temp = np.asarray(inputs["temperature"], np.float32).reshape(H)
    bandq = np.zeros((128, 128), np.float32)
    bandk = np.zeros((128, 2, 128), np.float32)
    for k in range(128):
        for m in range(128):
            if k // 32 == m // 32:
                bandq[k, m] = 1.0
                for ti in range(2):
                    h = ti * 4 + m // 32
                    bandk[k, ti, m] = 1.0 / (temp[h] * temp[h])

    return dict(
        wqk=wqk, wv3=wv3, wproj=wproj, w1=w1, w2=w2, vecs=vecs, b1v=b1v,
        tapsqk=tapsqk,
        ones_b=np.ones((128, 128), BF16),
        bandq=bandq.astype(BF16),
        bandk=bandk.astype(BF16),
    )


_NC_CACHE = {}


def get_nc(flags=(True, True, True, True)):
    if flags not in _NC_CACHE:
        _NC_CACHE[flags] = build_nc(*flags)
    return _NC_CACHE[flags]


def _flags_from(inputs):
    z = lambda k: bool(np.all(np.asarray(inputs[k]) == 0.0))
    o = lambda k: bool(np.all(np.asarray(inputs[k]) == 1.0))
    return (z("b1"), z("b2"), z("norm1_b") and z("norm2_b"),
            o("norm1_g") and o("norm2_g"))


def make_in_maps(inputs):
    consts = _host_prep(inputs)
    x = np.asarray(inputs["x"], np.float32)  # (B, C, T, Fd)
    in_maps = []
    for core in range(NCORES):
        b, t0 = core // 2, (core % 2) * SPC
        m = dict(consts)
        xs = x[b, :, t0:t0 + SPC, :]          # (C, SPC, Fd)
        m["x16"] = np.ascontiguousarray(
            xs.reshape(2, 128, SPC, Fd).transpose(1, 0, 2, 3)).astype(BF16)
        in_maps.append(m)
    return in_maps


def assemble_out(results):
    out = np.zeros((B, C, T, Fd), np.float32)
    for core in range(NCORES):
        b, t0 = core // 2, (core % 2) * SPC
        r = results[core]["out"]              # (128, 2, SPC, Fd)
        out[b, :, t0:t0 + SPC, :] = r.transpose(1, 0, 2, 3).reshape(
            C, SPC, Fd)
    return out


def kernel(**inputs):
    nc = get_nc(_flags_from(inputs))
    in_maps = make_in_maps(inputs)
    res = run_bass_kernel_spmd(nc, in_maps, core_ids=list(range(NCORES)))
    return assemble_out(res.results)


# revision 5
# speedup vs baseline: 1.8321x; 1.0452x over previous
"""Trainium2 Bass kernel for the AxisMDTA dense-transformer block (v2).

Shapes (hardcoded): x (4, 256, 64, 256) fp32 -> out (4, 256, 64, 256) fp32.
256 independent samples of (f=256, c=256): LN -> qkv -> depthwise conv3
along f -> 8-head attn (L2-normed q/k, temperature) -> proj + residual ->
LN -> MLP(gelu) + residual.

Data-parallel over the 256 (b,t) samples across 8 cores (32 each, in 4
blocks of 8). Channel-major on-chip layout (c on partitions, (sample, f)
on the free dim).  Key cost choices vs v1:
  - depthwise conv runs on DVE (4x-rate scalar_tensor_tensor on bf16
    guarded-pitch views) instead of tripling the qkv matmul work;
  - v' (f-major v for attn@v) comes straight from 3-tap matmuls with
    xn slices as the stationary operand - no PE transposes;
  - LN/L2 statistics matmuls emit band-replicated rows so rsqrt runs
    directly on the replicated tiles (Ln+Exp on ACT) - no dense-pack
    DMAs, no Newton iterations, no broadcast matmuls;
  - attn@v processes head pairs with an in-psum ones-block that yields
    the softmax denominators, normalized during the psum drain;
  - x is pre-cast to bf16 on the host, halving DMA and enabling 4x DVE.
"""

import contextlib

import numpy as np
import ml_dtypes

import concourse.bass as bass
import concourse.mybir as mybir
import concourse.tile as tile
from concourse.vector_clock import ScopedClock
from concourse.bass_utils import run_bass_kernel_spmd

AF = mybir.ActivationFunctionType
ALU = mybir.AluOpType
DT = mybir.dt
BF16 = ml_dtypes.bfloat16

# Problem constants
B, C, T, Fd = 4, 256, 64, 256
H, Dh = 8, 32
HID = 512
NCORES = 8
SPC = (B * T) // NCORES  # 32 samples per core
LN_EPS = 1e-5
S = 8                    # samples per block
NBLK = SPC // S
W = S * Fd               # 2048 free columns per block
PITCH = Fd + 2           # guarded pitch for conv views
NCH = W // 512           # 512-col psum chunks


class _TileContext(tile.TileContext):
    """Walrus in this container caps sync-wait commands per CTRL-class
    instruction; spread the exit drain's waits across single-wait nops."""

    def _drain_and_barrier(self, tick_clock, wait_clock):
        drain_inst = self.nc.sync.drain()
        wait_clock.add_sem_waits(
            drain_inst.ins, ScopedClock({None: tick_clock.global_clock})
        )
        si = drain_inst.ins.sync_info
        waits = list(si.on_wait or []) if si else []
        if len(waits) > 1:
            si.on_wait = waits[:1]
            for w in waits[1:]:
                n = self.nc.sync.nop(nofuse=True).ins
                n.sync_info = mybir.SyncInfo(on_wait=[w], on_update=[])
        self.nc.all_engine_barrier()
        assert self.sems is not None
        popped = self.nc._tile_sem_poison_stack.pop()
        assert popped is self._sem_poison
        self.nc.clear_and_free_semaphores(list(self.sems.allocated().values()))
        self.nc.all_engine_barrier()


def _split_excess_waits(nc, max_waits=1):
    """Walrus in this container caps sync-wait commands per instruction.
    Move excess waits onto same-engine NoOps inserted just before."""
    for f in nc.m.functions:
        for bb in f.blocks:
            new_insts = []
            for inst in bb.instructions:
                si = inst.sync_info
                waits = list(si.on_wait) if si and si.on_wait else []
                if len(waits) > max_waits:
                    si.on_wait = waits[:max_waits]
                    rest = waits[max_waits:]
                    for i in range(0, len(rest), max_waits):
                        nop = mybir.InstEventSemaphore(
                            name=f"I-ws{nc.next_id()}", ins=[], outs=[])
                        nop.engine = inst.engine
                        nop.sync_info = mybir.SyncInfo(
                            on_wait=rest[i:i + max_waits], on_update=[])
                        nc.register_instruction(nop)
                        new_insts.append(nop)
                new_insts.append(inst)
            bb.instructions[:] = new_insts


def build_nc(skip_b1=True, skip_b2=True, skip_lnb=True,
             skip_lng=True):
    nc = bass.Bass()

    x_in = nc.dram_tensor("x16", [128, 2, SPC, Fd], DT.bfloat16,
                          kind="ExternalInput")
    out_d = nc.dram_tensor("out", [128, 2, SPC, Fd], DT.float32,
                           kind="ExternalOutput")
    wqk_d = nc.dram_tensor("wqk", [128, 2, 512], DT.bfloat16, kind="ExternalInput")
    wv3_d = nc.dram_tensor("wv3", [128, 2, 3, 256], DT.bfloat16, kind="ExternalInput")
    wproj_d = nc.dram_tensor("wproj", [128, 2, 256], DT.bfloat16, kind="ExternalInput")
    w1_d = nc.dram_tensor("w1", [128, 2, HID], DT.bfloat16, kind="ExternalInput")
    w2_d = nc.dram_tensor("w2", [128, 4, 256], DT.bfloat16, kind="ExternalInput")
    vec_d = nc.dram_tensor("vecs", [128, 2, 6], DT.float32, kind="ExternalInput")
    b1_d = nc.dram_tensor("b1v", [128, 4], DT.float32, kind="ExternalInput")
    taps_d = nc.dram_tensor("tapsqk", [128, 4, 3], DT.float32, kind="ExternalInput")
    ones_d = nc.dram_tensor("ones_b", [128, 128], DT.bfloat16, kind="ExternalInput")
    bandq_d = nc.dram_tensor("bandq", [128, 128], DT.bfloat16, kind="ExternalInput")
    bandk_d = nc.dram_tensor("bandk", [128, 2, 128], DT.bfloat16, kind="ExternalInput")

    with _TileContext(nc) as tc, contextlib.ExitStack() as ctx:
        cpool = ctx.enter_context(tc.tile_pool(name="consts", bufs=1))
        xpool = ctx.enter_context(tc.tile_pool(name="xp", bufs=2))
        xnpool = ctx.enter_context(tc.tile_pool(name="xnp", bufs=2))
        qrawp = ctx.enter_context(tc.tile_pool(name="qraw", bufs=2))
        qcp = ctx.enter_context(tc.tile_pool(name="qcp", bufs=1))
        wp = ctx.enter_context(tc.tile_pool(name="wp", bufs=3))
        php = ctx.enter_context(tc.tile_pool(name="php", bufs=4))
        ohp = ctx.enter_context(tc.tile_pool(name="ohp", bufs=1))
        o1p = ctx.enter_context(tc.tile_pool(name="o1p", bufs=1))
        xhp = ctx.enter_context(tc.tile_pool(name="xhp", bufs=1))
        gp = ctx.enter_context(tc.tile_pool(name="gp", bufs=1))
        op = ctx.enter_context(tc.tile_pool(name="op", bufs=1))
        ps = ctx.enter_context(tc.tile_pool(name="ps", bufs=2, space="PSUM"))

        def cload(name, shape, dt, dram):
            t = cpool.tile(shape, dt, tag=name)
            nc.sync.dma_start(t[:], dram[:])
            return t

        ones_b = cload("ones_b", [128, 128], DT.bfloat16, ones_d)
        vecs = cload("vecs", [128, 2, 6], DT.float32, vec_d)
        wqk = cload("wqk", [128, 2, 512], DT.bfloat16, wqk_d)
        tapsqk = cload("tapsqk", [128, 4, 3], DT.float32, taps_d)
        wv3 = cload("wv3", [128, 2, 3, 256], DT.bfloat16, wv3_d)
        bandq = cload("bandq", [128, 128], DT.bfloat16, bandq_d)
        bandk = cload("bandk", [128, 2, 128], DT.bfloat16, bandk_d)
        wproj = cload("wproj", [128, 2, 256], DT.bfloat16, wproj_d)
        w1 = cload("w1", [128, 2, HID], DT.bfloat16, w1_d)
        w2 = cload("w2", [128, 4, 256], DT.bfloat16, w2_d)
        b1v = cload("b1v", [128, 4], DT.float32, b1_d)

        # v' composite lhsT tiles (single set; the framework serializes
        # next-block drains behind this block's attn@v reads):
        # layout (128 fk, S, pair, slot, 128); slot0 = [vA |0| 1 |0],
        # slot1 = [0| vB |0| 1].
        vpx8 = cpool.tile([128, 2, S, 4, 2, 128], DT.float8e4,
                          name="vpx8", tag="vpx8")
        nc.vector.memset(vpx8[:], 0.0)
        nc.vector.memset(vpx8[:, :, :, :, 0, 64:96], 1.0)
        nc.vector.memset(vpx8[:, :, :, :, 1, 96:128], 1.0)

        def rsqrt_rep(src_ap, out_ap, eps_ap):
            """out = 1/sqrt(src + eps), elementwise, via ACT Ln + Exp.
            src may be PSUM fp32; out is bf16 SBUF (same partitions)."""
            t = wp.tile([128, W], DT.bfloat16, name="lnt", tag="wk")
            nc.scalar.activation(t[: src_ap.shape[0]], src_ap, AF.Ln,
                                 bias=eps_ap, scale=1.0)
            nc.scalar.activation(out_ap, t[: src_ap.shape[0]], AF.Exp,
                                 scale=-0.5)

        def layernorm(src_f, g_cols, b_cols, out_aps, skip_b, skip_g,
                      relaxed):
            ve = nc.vector
            """src_f: two bf16 (128, 2048) APs (channel-major, kt split).
            Writes bf16 out_aps. LN over the 256 channels (both kt tiles).
            Fast-evicts the psum stat tiles to bf16 so psum frees early."""
            ps_s = ps.tile([128, W], DT.float32, tag="mm")
            ps_q = ps.tile([128, W], DT.float32, tag="mm")
            for kt in range(2):
                for ch in range(NCH):
                    sl = slice(ch * 512, (ch + 1) * 512)
                    nc.tensor.matmul(
                        ps_s[:, sl], ones_b[:], src_f[kt][:, sl],
                        start=(kt == 0), stop=(kt == 1), skip_group_check=True)
            for kt in range(2):
                sq = wp.tile([128, W], DT.bfloat16, name="lnsq", tag="lnsq",
                             bufs=1)
                ve.tensor_mul(sq[:], src_f[kt], src_f[kt])
                for ch in range(NCH):
                    sl = slice(ch * 512, (ch + 1) * 512)
                    nc.tensor.matmul(
                        ps_q[:, sl], ones_b[:], sq[:, sl],
                        start=(kt == 0), stop=(kt == 1), skip_group_check=True)
            mu16 = wp.tile([128, W], DT.bfloat16, name="mu16", tag="lnmu",
                           bufs=2)
            ve.tensor_scalar(mu16[:], ps_s[:], 1.0 / C, None, ALU.mult)
            q16 = wp.tile([128, W], DT.bfloat16, name="q16", tag="wk")
            nc.scalar.activation(q16[:], ps_q[:], AF.Copy, bias=0.0,
                                 scale=1.0 / C)
            mu2 = wp.tile([128, W], DT.bfloat16, name="mu2", tag="wk")
            ve.tensor_mul(mu2[:], mu16[:], mu16[:])
            var = wp.tile([128, W], DT.bfloat16, name="var", tag="wk")
            ve.tensor_sub(var[:], q16[:], mu2[:])
            rsig = wp.tile([128, W], DT.bfloat16, name="rsg1", tag="lnrs",
                           bufs=2)
            rsqrt_rep(var[:], rsig[:], vecs[:, 0, 5:6])
            for kt in range(2):
                if skip_g:
                    gr = rsig
                else:
                    gr = wp.tile([128, W], DT.bfloat16, name="lngr", tag="wk")
                    nc.vector.tensor_scalar(gr[:], rsig[:], g_cols[kt], None,
                                            ALU.mult)
                a = wp.tile([128, W], DT.bfloat16, name="lna", tag="wk")
                ve.tensor_sub(a[:], src_f[kt], mu16[:])
                if skip_b:
                    nc.vector.tensor_mul(out_aps[kt], a[:], gr[:])
                else:
                    t = wp.tile([128, W], DT.bfloat16, name="lnb", tag="wk")
                    nc.vector.tensor_mul(t[:], a[:], gr[:])
                    nc.vector.tensor_scalar(
                        out_aps[kt], t[:], b_cols[kt], None, ALU.add)

        # ---------------- staged, software-pipelined blocks ----------------
        st = [dict() for _ in range(NBLK)]

        def dma_x(blk):
            d = st[blk]
            d["x16"] = [xpool.tile([128, S, Fd], DT.bfloat16,
                                   name=f"x16_{kt}", tag=f"x16_{kt}")
                        for kt in range(2)]
            for kt in range(2):
                nc.sync.dma_start(
                    d["x16"][kt][:], x_in[:, kt, blk * S:(blk + 1) * S, :])
            d["xf"] = [t[:].rearrange("p s f -> p (s f)") for t in d["x16"]]

        def s1_ln1(blk):
            d = st[blk]
            xn = [xnpool.tile([128, S, PITCH], DT.bfloat16, name=f"xn{kt}",
                              tag=f"xn{kt}") for kt in range(2)]
            for kt in range(2):
                nc.vector.memset(xn[kt][:, :, 0:1], 0.0)
                nc.vector.memset(xn[kt][:, :, PITCH - 1:PITCH], 0.0)
            xn_data = [xn[kt][:, :, 1:1 + Fd] for kt in range(2)]
            layernorm(d["xf"], [vecs[:, kt, 0:1] for kt in range(2)],
                      [vecs[:, kt, 1:2] for kt in range(2)],
                      xn_data, skip_lnb, skip_lng, relaxed=True)
            d["xn"] = xn
            d["xn_rhs"] = xn_data

        def s2_qkv(blk):
            d = st[blk]
            d["qc"] = []
            for m in range(4):
                ps_m = ps.tile([128, W], DT.float32, tag="mm")
                for kt in range(2):
                    for ch in range(NCH):
                        nc.tensor.matmul(
                            ps_m[:, ch * 512:(ch + 1) * 512],
                            wqk[:, kt, m * 128:(m + 1) * 128],
                            d["xn_rhs"][kt][:, 2 * ch:2 * ch + 2, :],
                            start=(kt == 0), stop=(kt == 1),
                            skip_group_check=True)
                qr = qrawp.tile([128, S, PITCH], DT.bfloat16, name=f"qr{m}",
                                tag="qraw")
                nc.gpsimd.memset(qr[:, :, 0:1], 0.0)
                nc.gpsimd.memset(qr[:, :, PITCH - 1:PITCH], 0.0)
                nc.scalar.copy(qr[:, :, 1:1 + Fd],
                               ps_m[:].rearrange("p (s f) -> p s f", s=S))
                t0 = wp.tile([128, S, Fd], DT.bfloat16, name="cv0", tag="wk")
                nc.vector.tensor_scalar(
                    t0[:], qr[:, :, 0:Fd], tapsqk[:, m, 0:1], None, ALU.mult)
                t1 = wp.tile([128, S, Fd], DT.bfloat16, name="cv1", tag="wk")
                nc.vector.tensor_scalar(
                    t1[:], qr[:, :, 1:1 + Fd], tapsqk[:, m, 1:2], None,
                    ALU.mult)
                t2 = wp.tile([128, S, Fd], DT.bfloat16, name="cv2", tag="wk")
                nc.vector.tensor_scalar(
                    t2[:], qr[:, :, 2:2 + Fd], tapsqk[:, m, 2:3], None,
                    ALU.mult)
                nc.vector.tensor_add(t0[:], t0[:], t1[:])
                q = qcp.tile([128, S, Fd], DT.bfloat16, name=f"qc{m}",
                             tag=f"qc{m}")
                nc.vector.tensor_add(q[:], t0[:], t2[:])
                d["qc"].append(q)

        def s3_vprime(blk):
            d = st[blk]
            for fc in range(2):
                vp_ps = ps.tile([128, S, 256], DT.float32, tag="mm")
                for s in range(S):
                    first = True
                    for kt in range(2):
                        for tap in range(3):
                            nc.tensor.matmul(
                                vp_ps[:, s, :],
                                d["xn"][kt][:, s,
                                            fc * 128 + tap:fc * 128 + tap + 128],
                                wv3[:, kt, tap, :],
                                start=first, stop=(kt == 1 and tap == 2),
                                skip_group_check=True)
                            first = False
                vsrc = vp_ps[:].rearrange("p s (h2 two d) -> p s h2 two d",
                                          two=2, d=Dh)
                nc.scalar.copy(vpx8[:, fc, :, :, 0, 0:32],
                               vsrc[:, :, :, 0, :])
                nc.vector.tensor_copy(vpx8[:, fc, :, :, 1, 32:64],
                                      vsrc[:, :, :, 1, :])

        def s4_l2(blk):
            d = st[blk]
            for m in (0, 2, 1, 3):
                qf = d["qc"][m][:].rearrange("p s f -> p (s f)")
                sqm = wp.tile([128, W], DT.bfloat16, name="sqm",
                              tag="l2sq", bufs=2)
                nc.vector.tensor_mul(sqm[:], qf, qf)
                ps_n = ps.tile([128, W], DT.float32, tag="mm")
                band = bandq[:] if m < 2 else bandk[:, m - 2, :]
                for ch in range(NCH):
                    sl = slice(ch * 512, (ch + 1) * 512)
                    nc.tensor.matmul(ps_n[:, sl], band, sqm[:, sl],
                                     start=True, stop=True,
                                     skip_group_check=True)
                rsig = wp.tile([128, W], DT.bfloat16, name="rsg2", tag="lnrs",
                               bufs=2)
                if m % 2 == 0:
                    rsqrt_rep(ps_n[:], rsig[:], vecs[:, 1, 5:6])
                else:
                    sig = wp.tile([128, W], DT.bfloat16, name="sig", tag="wk")
                    nc.scalar.activation(sig[:], ps_n[:], AF.Sqrt,
                                         bias=vecs[:, 1, 5:6], scale=1.0)
                    with nc.allow_low_precision(reason="l2 rsig bf16"):
                        nc.vector.reciprocal(rsig[:], sig[:])
                nc.vector.tensor_mul(qf, qf, rsig[:])

        def s5_attn(blk):
            d = st[blk]
            qc = d["qc"]
            phat = {}
            oh = [ohp.tile([128, W], DT.bfloat16, name=f"oh{kt}",
                           tag=f"oh{kt}") for kt in range(2)]
            for h in range(H):
                ti, band = h // 4, (h % 4) * 32
                pj = php.tile([128, S, 2, 256], DT.float8e4, tag="phat")
                for jt in range(2):
                    ps_S = ps.tile([128, W], DT.float32, tag="mm")
                    for s in range(S):
                        nc.tensor.matmul(
                            ps_S[:, s * Fd:(s + 1) * Fd],
                            qc[2 + ti][band:band + 32, s,
                                       jt * 128:jt * 128 + 128],
                            qc[ti][band:band + 32, s, :],
                            start=True, stop=True, skip_group_check=True,
                            tile_position=(band, 0))
                    nc.scalar.activation(
                        pj[:, 0:S // 2, jt, :],
                        ps_S[:, 0:W // 2].rearrange("p (s f) -> p s f", f=Fd),
                        AF.Exp)
                    nc.scalar.activation(
                        pj[:, S // 2:S, jt, :],
                        ps_S[:, W // 2:W].rearrange("p (s f) -> p s f", f=Fd),
                        AF.Exp)
                phat[h] = pj
                if h % 2 == 1:
                    p = h // 2
                    pair_ps = ps.tile([128, W], DT.float32, tag="mm")
                    for s in range(S):
                        osl = pair_ps[:, s * Fd:(s + 1) * Fd]
                        for sl, hh in ((0, h - 1), (1, h)):
                            nc.tensor.matmul(
                                osl, vpx8[:, :, s, p, sl, :],
                                phat[hh][:, s, :, :],
                                start=(sl == 0), stop=(sl == 1),
                                perf_mode=mybir.MatmulPerfMode.DoubleRow,
                                skip_group_check=True)
                    pnum = wp.tile([128, W], DT.bfloat16, name="pnum",
                                   tag="lnsq", bufs=1)
                    nc.vector.tensor_copy(pnum[:], pair_ps[:])
                    rinv = wp.tile([64, W], DT.bfloat16, tag="rinv", bufs=1)
                    with nc.allow_low_precision(reason="softmax denom bf16"):
                        nc.vector.reciprocal(rinv[:], pnum[64:128, :])
                    nc.vector.tensor_mul(
                        oh[p // 2][(p % 2) * 64:(p % 2) * 64 + 64, :],
                        pnum[0:64, :], rinv[:])
                    del phat[h - 1], phat[h]
            d["oh"] = oh

        def s6_proj(blk):
            d = st[blk]
            d["o1"] = []
            for m2 in range(2):
                ps_y = ps.tile([128, W], DT.float32, tag="mm")
                for kt in range(2):
                    for ch in range(NCH):
                        sl = slice(ch * 512, (ch + 1) * 512)
                        nc.tensor.matmul(
                            ps_y[:, sl],
                            wproj[:, kt, m2 * 128:(m2 + 1) * 128],
                            d["oh"][kt][:, sl], start=(kt == 0),
                            stop=(kt == 1), skip_group_check=True)
                o1t = o1p.tile([128, W], DT.bfloat16, name=f"o1_{m2}",
                               tag=f"o1_{m2}")
                nc.vector.tensor_add(o1t[:], ps_y[:], d["xf"][m2])
                d["o1"].append(o1t)

        def s7_ln2(blk):
            d = st[blk]
            xh = [xhp.tile([128, W], DT.bfloat16, name=f"xh{kt}",
                           tag=f"xh{kt}") for kt in range(2)]
            layernorm([t[:] for t in d["o1"]],
                      [vecs[:, kt, 2:3] for kt in range(2)],
                      [vecs[:, kt, 3:4] for kt in range(2)],
                      [t[:] for t in xh], skip_lnb, skip_lng, relaxed=False)
            d["xh"] = xh

        def s8_mlp1(blk):
            d = st[blk]
            d["g16"] = []
            for mh in range(4):
                ps_h = ps.tile([128, W], DT.float32, tag="mm")
                for kt in range(2):
                    for ch in range(NCH):
                        sl = slice(ch * 512, (ch + 1) * 512)
                        nc.tensor.matmul(
                            ps_h[:, sl], w1[:, kt, mh * 128:(mh + 1) * 128],
                            d["xh"][kt][:][:, sl], start=(kt == 0),
                            stop=(kt == 1), skip_group_check=True)
                g = gp.tile([128, W], DT.bfloat16, name=f"g{mh}", tag=f"g{mh}")
                if skip_b1:
                    nc.scalar.activation(g[:], ps_h[:], AF.Gelu)
                else:
                    nc.scalar.activation(g[:], ps_h[:], AF.Gelu,
                                         bias=b1v[:, mh:mh + 1], scale=1.0)
                d["g16"].append(g)

        def s9_mlp2(blk):
            d = st[blk]
            for m2 in range(2):
                ps_o = ps.tile([128, W], DT.float32, tag="mm")
                for kt in range(4):
                    for ch in range(NCH):
                        sl = slice(ch * 512, (ch + 1) * 512)
                        nc.tensor.matmul(
                            ps_o[:, sl], w2[:, kt, m2 * 128:(m2 + 1) * 128],
                            d["g16"][kt][:][:, sl], start=(kt == 0),
                            stop=(kt == 3), skip_group_check=True)
                o2 = op.tile([128, S, Fd], DT.float32, name="o2", tag="o2",
                             bufs=1)
                o2f = o2[:].rearrange("p s f -> p (s f)")
                if skip_b2:
                    nc.vector.tensor_add(o2f, ps_o[:], d["o1"][m2][:])
                else:
                    t = op.tile([128, W], DT.float32, name="o2t", tag="o2t")
                    nc.vector.tensor_add(t[:], ps_o[:], d["o1"][m2][:])
                    nc.vector.tensor_scalar(
                        o2f, t[:], vecs[:, m2, 4:5], None, ALU.add)
                nc.gpsimd.dma_start(
                    out_d[:, m2, blk * S:(blk + 1) * S, :], o2[:])

        # prologue (pipeline fill: LN1 runs two blocks ahead)
        dma_x(0)
        dma_x(1)
        s1_ln1(0)
        s1_ln1(1)
        s3_vprime(0)
        s2_qkv(0)
        s4_l2(0)
        for b in range(NBLK):
            s5_attn(b)
            s6_proj(b)
            s7_ln2(b)
            if b + 1 < NBLK:
                s3_vprime(b + 1)
                s2_qkv(b + 1)
            s8_mlp1(b)
            s9_mlp2(b)
            if b + 1 < NBLK:
                s4_l2(b + 1)
                if b + 2 < NBLK:
                    dma_x(b + 2)
                    s1_ln1(b + 2)

    _split_excess_waits(nc, max_waits=1)
    return nc


def _host_prep(inputs):
    Wqkv = np.asarray(inputs["Wqkv"], np.float32)        # (C, 3C)
    dw_w = np.asarray(inputs["dw_w"], np.float32)        # (3C, 1, 3)
    taps = dw_w[:, 0, :]                                 # (3C, 3)

    wqk = np.ascontiguousarray(
        Wqkv[:, :512].reshape(2, 128, 512).transpose(1, 0, 2)).astype(BF16)
    # wv3[p, kt, tap, c_out] = Wv[kt*128+p, c_out] * taps_v[c_out, tap]
    Wv = Wqkv[:, 512:]                                   # (C, 256)
    wv3 = np.einsum("co,ot->cto", Wv, taps[512:])        # (C, 3, 256)
    wv3 = np.ascontiguousarray(
        wv3.reshape(2, 128, 3, 256).transpose(1, 0, 2, 3)).astype(BF16)

    def kt_major(w, nkt):
        K, N = w.shape
        return np.ascontiguousarray(
            w.reshape(nkt, 128, N).transpose(1, 0, 2)).astype(BF16)

    wproj = kt_major(np.asarray(inputs["Wproj"], np.float32), 2)
    w1 = kt_major(np.asarray(inputs["W1"], np.float32), 2)
    w2 = kt_major(np.asarray(inputs["W2"], np.float32), 4)

    vecs = np.zeros((128, 2, 6), np.float32)
    for kt in range(2):
        sl = slice(kt * 128, (kt + 1) * 128)
        vecs[:, kt, 0] = np.asarray(inputs["norm1_g"], np.float32)[sl]
        vecs[:, kt, 1] = np.asarray(inputs["norm1_b"], np.float32)[sl]
        vecs[:, kt, 2] = np.asarray(inputs["norm2_g"], np.float32)[sl]
        vecs[:, kt, 3] = np.asarray(inputs["norm2_b"], np.float32)[sl]
        vecs[:, kt, 4] = np.asarray(inputs["b2"], np.float32)[sl]
    vecs[:, 0, 5] = LN_EPS
    vecs[:, 1, 5] = 1e-24
    b1v = np.ascontiguousarray(
        np.asarray(inputs["b1"], np.float32).reshape(4, 128).T)

    # conv taps for the q,k channel tiles: tapsqk[p, m, tap]
    tapsqk = np.ascontiguousarray(
        taps[:512].reshape(4, 128, 3).transpose(1, 0, 2)).astype(np.float32)

    temp = np.asarray(inputs["temperature"], np.float32).reshape(H)
    bandq = np.zeros((128, 128), np.float32)
    bandk = np.zeros((128, 2, 128), np.float32)
    for k in range(128):
        for m in range(128):
            if k // 32 == m // 32:
                bandq[k, m] = 1.0
                for ti in range(2):
                    h = ti * 4 + m // 32
                    bandk[k, ti, m] = 1.0 / (temp[h] * temp[h])

    return dict(
        wqk=wqk, wv3=wv3, wproj=wproj, w1=w1, w2=w2, vecs=vecs, b1v=b1v,
        tapsqk=tapsqk,
        ones_b=np.ones((128, 128), BF16),
        bandq=bandq.astype(BF16),
        bandk=bandk.astype(BF16),
    )


_NC_CACHE = {}


def get_nc(flags=(True, True, True, True)):
    if flags not in _NC_CACHE:
        _NC_CACHE[flags] = build_nc(*flags)
    return _NC_CACHE[flags]


def _flags_from(inputs):
    z = lambda k: bool(np.all(np.asarray(inputs[k]) == 0.0))
    o = lambda k: bool(np.all(np.asarray(inputs[k]) == 1.0))
    return (z("b1"), z("b2"), z("norm1_b") and z("norm2_b"),
            o("norm1_g") and o("norm2_g"))


def make_in_maps(inputs):
    consts = _host_prep(inputs)
    x = np.asarray(inputs["x"], np.float32)  # (B, C, T, Fd)
    in_maps = []
    for core in range(NCORES):
        b, t0 = core // 2, (core % 2) * SPC
        m = dict(consts)
        xs = x[b, :, t0:t0 + SPC, :]          # (C, SPC, Fd)
        m["x16"] = np.ascontiguousarray(
            xs.reshape(2, 128, SPC, Fd).transpose(1, 0, 2, 3)).astype(BF16)
        in_maps.append(m)
    return in_maps


def assemble_out(results):
    out = np.zeros((B, C, T, Fd), np.float32)
    for core in range(NCORES):
        b, t0 = core // 2, (core % 2) * SPC
        r = results[core]["out"]              # (128, 2, SPC, Fd)
        out[b, :, t0:t0 + SPC, :] = r.transpose(1, 0, 2, 3).reshape(
            C, SPC, Fd)
    return out


def kernel(**inputs):
    nc = get_nc(_flags_from(inputs))
    in_maps = make_in_maps(inputs)
    res = run_bass_kernel_spmd(nc, in_maps, core_ids=list(range(NCORES)))
    return assemble_out(res.results)


# revision 6
# speedup vs baseline: 1.8492x; 1.0093x over previous
"""Trainium2 Bass kernel for the AxisMDTA dense-transformer block (v2).

Shapes (hardcoded): x (4, 256, 64, 256) fp32 -> out (4, 256, 64, 256) fp32.
256 independent samples of (f=256, c=256): LN -> qkv -> depthwise conv3
along f -> 8-head attn (L2-normed q/k, temperature) -> proj + residual ->
LN -> MLP(gelu) + residual.

Data-parallel over the 256 (b,t) samples across 8 cores (32 each, in 4
blocks of 8). Channel-major on-chip layout (c on partitions, (sample, f)
on the free dim).  Key cost choices vs v1:
  - depthwise conv runs on DVE (4x-rate scalar_tensor_tensor on bf16
    guarded-pitch views) instead of tripling the qkv matmul work;
  - v' (f-major v for attn@v) comes straight from 3-tap matmuls with
    xn slices as the stationary operand - no PE transposes;
  - LN/L2 statistics matmuls emit band-replicated rows so rsqrt runs
    directly on the replicated tiles (Ln+Exp on ACT) - no dense-pack
    DMAs, no Newton iterations, no broadcast matmuls;
  - attn@v processes head pairs with an in-psum ones-block that yields
    the softmax denominators, normalized during the psum drain;
  - x is pre-cast to bf16 on the host, halving DMA and enabling 4x DVE.
"""

import contextlib

import numpy as np
import ml_dtypes

import concourse.bass as bass
import concourse.mybir as mybir
import concourse.tile as tile
from concourse.vector_clock import ScopedClock
from concourse.bass_utils import run_bass_kernel_spmd

AF = mybir.ActivationFunctionType
ALU = mybir.AluOpType
DT = mybir.dt
BF16 = ml_dtypes.bfloat16

# Problem constants
B, C, T, Fd = 4, 256, 64, 256
H, Dh = 8, 32
HID = 512
NCORES = 8
SPC = (B * T) // NCORES  # 32 samples per core
LN_EPS = 1e-5
S = 8                    # samples per block
NBLK = SPC // S
W = S * Fd               # 2048 free columns per block
PITCH = Fd + 2           # guarded pitch for conv views
NCH = W // 512           # 512-col psum chunks


class _TileContext(tile.TileContext):
    """Walrus in this container caps sync-wait commands per CTRL-class
    instruction; spread the exit drain's waits across single-wait nops."""

    def _drain_and_barrier(self, tick_clock, wait_clock):
        drain_inst = self.nc.sync.drain()
        wait_clock.add_sem_waits(
            drain_inst.ins, ScopedClock({None: tick_clock.global_clock})
        )
        si = drain_inst.ins.sync_info
        waits = list(si.on_wait or []) if si else []
        if len(waits) > 1:
            si.on_wait = waits[:1]
            for w in waits[1:]:
                n = self.nc.sync.nop(nofuse=True).ins
                n.sync_info = mybir.SyncInfo(on_wait=[w], on_update=[])
        self.nc.all_engine_barrier()
        assert self.sems is not None
        popped = self.nc._tile_sem_poison_stack.pop()
        assert popped is self._sem_poison
        self.nc.clear_and_free_semaphores(list(self.sems.allocated().values()))
        self.nc.all_engine_barrier()


def _split_excess_waits(nc, max_waits=1):
    """Walrus in this container caps sync-wait commands per instruction.
    Move excess waits onto same-engine NoOps inserted just before."""
    for f in nc.m.functions:
        for bb in f.blocks:
            new_insts = []
            for inst in bb.instructions:
                si = inst.sync_info
                waits = list(si.on_wait) if si and si.on_wait else []
                if len(waits) > max_waits:
                    si.on_wait = waits[:max_waits]
                    rest = waits[max_waits:]
                    for i in range(0, len(rest), max_waits):
                        nop = mybir.InstEventSemaphore(
                            name=f"I-ws{nc.next_id()}", ins=[], outs=[])
                        nop.engine = inst.engine
                        nop.sync_info = mybir.SyncInfo(
                            on_wait=rest[i:i + max_waits], on_update=[])
                        nc.register_instruction(nop)
                        new_insts.append(nop)
                new_insts.append(inst)
            bb.instructions[:] = new_insts


def build_nc(skip_b1=True, skip_b2=True, skip_lnb=True,
             skip_lng=True):
    nc = bass.Bass()

    x_in = nc.dram_tensor("x16", [128, 2, SPC, Fd], DT.bfloat16,
                          kind="ExternalInput")
    out_d = nc.dram_tensor("out", [128, 2, SPC, Fd], DT.float32,
                           kind="ExternalOutput")
    wqk_d = nc.dram_tensor("wqk", [128, 2, 512], DT.bfloat16, kind="ExternalInput")
    wv3_d = nc.dram_tensor("wv3", [128, 2, 3, 256], DT.bfloat16, kind="ExternalInput")
    wproj_d = nc.dram_tensor("wproj", [128, 2, 256], DT.bfloat16, kind="ExternalInput")
    w1_d = nc.dram_tensor("w1", [128, 2, HID], DT.bfloat16, kind="ExternalInput")
    w2_d = nc.dram_tensor("w2", [128, 4, 256], DT.bfloat16, kind="ExternalInput")
    vec_d = nc.dram_tensor("vecs", [128, 2, 6], DT.float32, kind="ExternalInput")
    b1_d = nc.dram_tensor("b1v", [128, 4], DT.float32, kind="ExternalInput")
    taps_d = nc.dram_tensor("tapsqk", [128, 4, 3], DT.float32, kind="ExternalInput")
    ones_d = nc.dram_tensor("ones_b", [128, 128], DT.bfloat16, kind="ExternalInput")
    bandq_d = nc.dram_tensor("bandq", [128, 128], DT.bfloat16, kind="ExternalInput")
    bandk_d = nc.dram_tensor("bandk", [128, 2, 128], DT.bfloat16, kind="ExternalInput")

    with _TileContext(nc) as tc, contextlib.ExitStack() as ctx:
        cpool = ctx.enter_context(tc.tile_pool(name="consts", bufs=1))
        xpool = ctx.enter_context(tc.tile_pool(name="xp", bufs=2))
        xnpool = ctx.enter_context(tc.tile_pool(name="xnp", bufs=2))
        qrawp = ctx.enter_context(tc.tile_pool(name="qraw", bufs=2))
        qcp = ctx.enter_context(tc.tile_pool(name="qcp", bufs=1))
        wp = ctx.enter_context(tc.tile_pool(name="wp", bufs=3))
        php = ctx.enter_context(tc.tile_pool(name="php", bufs=4))
        ohp = ctx.enter_context(tc.tile_pool(name="ohp", bufs=1))
        o1p = ctx.enter_context(tc.tile_pool(name="o1p", bufs=1))
        xhp = ctx.enter_context(tc.tile_pool(name="xhp", bufs=1))
        gp = ctx.enter_context(tc.tile_pool(name="gp", bufs=1))
        op = ctx.enter_context(tc.tile_pool(name="op", bufs=1))
        ps = ctx.enter_context(tc.tile_pool(name="ps", bufs=2, space="PSUM"))

        def cload(name, shape, dt, dram):
            t = cpool.tile(shape, dt, tag=name)
            nc.sync.dma_start(t[:], dram[:])
            return t

        ones_b = cload("ones_b", [128, 128], DT.bfloat16, ones_d)
        vecs = cload("vecs", [128, 2, 6], DT.float32, vec_d)
        wqk = cload("wqk", [128, 2, 512], DT.bfloat16, wqk_d)
        tapsqk = cload("tapsqk", [128, 4, 3], DT.float32, taps_d)
        wv3 = cload("wv3", [128, 2, 3, 256], DT.bfloat16, wv3_d)
        bandq = cload("bandq", [128, 128], DT.bfloat16, bandq_d)
        bandk = cload("bandk", [128, 2, 128], DT.bfloat16, bandk_d)
        wproj = cload("wproj", [128, 2, 256], DT.bfloat16, wproj_d)
        w1 = cload("w1", [128, 2, HID], DT.bfloat16, w1_d)
        w2 = cload("w2", [128, 4, 256], DT.bfloat16, w2_d)
        b1v = cload("b1v", [128, 4], DT.float32, b1_d)

        # v' composite lhsT tiles (single set; the framework serializes
        # next-block drains behind this block's attn@v reads):
        # layout (128 fk, S, pair, slot, 128); slot0 = [vA |0| 1 |0],
        # slot1 = [0| vB |0| 1].
        vpx8 = cpool.tile([128, 2, S, 4, 2, 128], DT.float8e4,
                          name="vpx8", tag="vpx8")
        nc.vector.memset(vpx8[:], 0.0)
        nc.vector.memset(vpx8[:, :, :, :, 0, 64:96], 1.0)
        nc.vector.memset(vpx8[:, :, :, :, 1, 96:128], 1.0)

        def rsqrt_rep(src_ap, out_ap, eps_ap):
            """out = 1/sqrt(src + eps), elementwise, via ACT Ln + Exp.
            src may be PSUM fp32; out is bf16 SBUF (same partitions)."""
            t = wp.tile([128, W], DT.bfloat16, name="lnt", tag="wk")
            nc.scalar.activation(t[: src_ap.shape[0]], src_ap, AF.Ln,
                                 bias=eps_ap, scale=1.0)
            nc.scalar.activation(out_ap, t[: src_ap.shape[0]], AF.Exp,
                                 scale=-0.5)

        def layernorm(src_f, g_cols, b_cols, out_aps, skip_b, skip_g,
                      relaxed):
            ve = nc.vector
            """src_f: two bf16 (128, 2048) APs (channel-major, kt split).
            Writes bf16 out_aps. LN over the 256 channels (both kt tiles).
            Fast-evicts the psum stat tiles to bf16 so psum frees early."""
            ps_s = ps.tile([128, W], DT.float32, tag="mm")
            ps_q = ps.tile([128, W], DT.float32, tag="mm")
            for kt in range(2):
                for ch in range(NCH):
                    sl = slice(ch * 512, (ch + 1) * 512)
                    nc.tensor.matmul(
                        ps_s[:, sl], ones_b[:], src_f[kt][:, sl],
                        start=(kt == 0), stop=(kt == 1), skip_group_check=True)
            for kt in range(2):
                sq = wp.tile([128, W], DT.bfloat16, name="lnsq", tag="lnsq",
                             bufs=1)
                ve.tensor_mul(sq[:], src_f[kt], src_f[kt])
                for ch in range(NCH):
                    sl = slice(ch * 512, (ch + 1) * 512)
                    nc.tensor.matmul(
                        ps_q[:, sl], ones_b[:], sq[:, sl],
                        start=(kt == 0), stop=(kt == 1), skip_group_check=True)
            mu16 = wp.tile([128, W], DT.bfloat16, name="mu16", tag="lnmu",
                           bufs=2)
            ve.tensor_scalar(mu16[:], ps_s[:], 1.0 / C, None, ALU.mult)
            q16 = wp.tile([128, W], DT.bfloat16, name="q16", tag="wk")
            nc.scalar.activation(q16[:], ps_q[:], AF.Copy, bias=0.0,
                                 scale=1.0 / C)
            mu2 = wp.tile([128, W], DT.bfloat16, name="mu2", tag="wk")
            ve.tensor_mul(mu2[:], mu16[:], mu16[:])
            var = wp.tile([128, W], DT.bfloat16, name="var", tag="wk")
            ve.tensor_sub(var[:], q16[:], mu2[:])
            rsig = wp.tile([128, W], DT.bfloat16, name="rsg1", tag="lnrs",
                           bufs=2)
            rsqrt_rep(var[:], rsig[:], vecs[:, 0, 5:6])
            for kt in range(2):
                if skip_g:
                    gr = rsig
                else:
                    gr = wp.tile([128, W], DT.bfloat16, name="lngr", tag="wk")
                    nc.vector.tensor_scalar(gr[:], rsig[:], g_cols[kt], None,
                                            ALU.mult)
                a = wp.tile([128, W], DT.bfloat16, name="lna", tag="wk")
                ve.tensor_sub(a[:], src_f[kt], mu16[:])
                if skip_b:
                    nc.vector.tensor_mul(out_aps[kt], a[:], gr[:])
                else:
                    t = wp.tile([128, W], DT.bfloat16, name="lnb", tag="wk")
                    nc.vector.tensor_mul(t[:], a[:], gr[:])
                    nc.vector.tensor_scalar(
                        out_aps[kt], t[:], b_cols[kt], None, ALU.add)

        # ---------------- staged, software-pipelined blocks ----------------
        st = [dict() for _ in range(NBLK)]

        def dma_x(blk):
            d = st[blk]
            d["x16"] = [xpool.tile([128, S, Fd], DT.bfloat16,
                                   name=f"x16_{kt}", tag=f"x16_{kt}")
                        for kt in range(2)]
            for kt in range(2):
                nc.sync.dma_start(
                    d["x16"][kt][:], x_in[:, kt, blk * S:(blk + 1) * S, :])
            d["xf"] = [t[:].rearrange("p s f -> p (s f)") for t in d["x16"]]

        def s1_ln1(blk):
            d = st[blk]
            xn = [xnpool.tile([128, S, PITCH], DT.bfloat16, name=f"xn{kt}",
                              tag=f"xn{kt}") for kt in range(2)]
            for kt in range(2):
                nc.vector.memset(xn[kt][:, :, 0:1], 0.0)
                nc.vector.memset(xn[kt][:, :, PITCH - 1:PITCH], 0.0)
            xn_data = [xn[kt][:, :, 1:1 + Fd] for kt in range(2)]
            layernorm(d["xf"], [vecs[:, kt, 0:1] for kt in range(2)],
                      [vecs[:, kt, 1:2] for kt in range(2)],
                      xn_data, skip_lnb, skip_lng, relaxed=True)
            d["xn"] = xn
            d["xn_rhs"] = xn_data

        def s2_qkv(blk):
            d = st[blk]
            d["qc"] = []
            for m in range(4):
                ps_m = ps.tile([128, W], DT.float32, tag="mm")
                for kt in range(2):
                    for ch in range(NCH):
                        nc.tensor.matmul(
                            ps_m[:, ch * 512:(ch + 1) * 512],
                            wqk[:, kt, m * 128:(m + 1) * 128],
                            d["xn_rhs"][kt][:, 2 * ch:2 * ch + 2, :],
                            start=(kt == 0), stop=(kt == 1),
                            skip_group_check=True)
                qr = qrawp.tile([128, S, PITCH], DT.bfloat16, name=f"qr{m}",
                                tag="qraw")
                nc.gpsimd.memset(qr[:, :, 0:1], 0.0)
                nc.gpsimd.memset(qr[:, :, PITCH - 1:PITCH], 0.0)
                nc.scalar.copy(qr[:, :, 1:1 + Fd],
                               ps_m[:].rearrange("p (s f) -> p s f", s=S))
                t0 = wp.tile([128, S, Fd], DT.bfloat16, name="cv0", tag="wk")
                nc.vector.tensor_scalar(
                    t0[:], qr[:, :, 0:Fd], tapsqk[:, m, 0:1], None, ALU.mult)
                t1 = wp.tile([128, S, Fd], DT.bfloat16, name="cv1", tag="wk")
                nc.vector.tensor_scalar(
                    t1[:], qr[:, :, 1:1 + Fd], tapsqk[:, m, 1:2], None,
                    ALU.mult)
                t2 = wp.tile([128, S, Fd], DT.bfloat16, name="cv2", tag="wk")
                nc.vector.tensor_scalar(
                    t2[:], qr[:, :, 2:2 + Fd], tapsqk[:, m, 2:3], None,
                    ALU.mult)
                nc.vector.tensor_add(t0[:], t0[:], t1[:])
                q = qcp.tile([128, S, Fd], DT.bfloat16, name=f"qc{m}",
                             tag=f"qc{m}")
                nc.vector.tensor_add(q[:], t0[:], t2[:])
                d["qc"].append(q)

        def s3_vprime(blk):
            d = st[blk]
            for fc in range(2):
                vp_ps = ps.tile([128, S, 256], DT.float32, tag="mm")
                for s in range(S):
                    first = True
                    for kt in range(2):
                        for tap in range(3):
                            nc.tensor.matmul(
                                vp_ps[:, s, :],
                                d["xn"][kt][:, s,
                                            fc * 128 + tap:fc * 128 + tap + 128],
                                wv3[:, kt, tap, :],
                                start=first, stop=(kt == 1 and tap == 2),
                                skip_group_check=True)
                            first = False
                vsrc = vp_ps[:].rearrange("p s (h2 two d) -> p s h2 two d",
                                          two=2, d=Dh)
                nc.scalar.copy(vpx8[:, fc, :, :, 0, 0:32],
                               vsrc[:, :, :, 0, :])
                nc.vector.tensor_copy(vpx8[:, fc, :, :, 1, 32:64],
                                      vsrc[:, :, :, 1, :])

        def s4_l2(blk):
            d = st[blk]
            for m in (0, 2, 1, 3):
                qf = d["qc"][m][:].rearrange("p s f -> p (s f)")
                sqm = wp.tile([128, W], DT.bfloat16, name="sqm",
                              tag="l2sq", bufs=2)
                nc.vector.tensor_mul(sqm[:], qf, qf)
                ps_n = ps.tile([128, W], DT.float32, tag="mm")
                band = bandq[:] if m < 2 else bandk[:, m - 2, :]
                for ch in range(NCH):
                    sl = slice(ch * 512, (ch + 1) * 512)
                    nc.tensor.matmul(ps_n[:, sl], band, sqm[:, sl],
                                     start=True, stop=True,
                                     skip_group_check=True)
                rsig = wp.tile([128, W], DT.bfloat16, name="rsg2", tag="lnrs",
                               bufs=2)
                if m % 2 == 0:
                    rsqrt_rep(ps_n[:], rsig[:], vecs[:, 1, 5:6])
                else:
                    sig = wp.tile([128, W], DT.bfloat16, name="sig", tag="wk")
                    nc.scalar.activation(sig[:], ps_n[:], AF.Sqrt,
                                         bias=vecs[:, 1, 5:6], scale=1.0)
                    with nc.allow_low_precision(reason="l2 rsig bf16"):
                        nc.vector.reciprocal(rsig[:], sig[:])
                nc.vector.tensor_mul(qf, qf, rsig[:])

        def s5_attn(blk):
            d = st[blk]
            qc = d["qc"]
            phat = {}
            oh = [ohp.tile([128, W], DT.bfloat16, name=f"oh{kt}",
                           tag=f"oh{kt}") for kt in range(2)]
            for h in range(H):
                ti, band = h // 4, (h % 4) * 32
                pj = php.tile([128, S, 2, 256], DT.float8e4, tag="phat")
                for jt in range(2):
                    ps_S = ps.tile([128, W], DT.float32, tag="mm")
                    for s in range(S):
                        nc.tensor.matmul(
                            ps_S[:, s * Fd:(s + 1) * Fd],
                            qc[2 + ti][band:band + 32, s,
                                       jt * 128:jt * 128 + 128],
                            qc[ti][band:band + 32, s, :],
                            start=True, stop=True, skip_group_check=True,
                            tile_position=(band, 0))
                    nc.scalar.activation(
                        pj[:, :, jt, :],
                        ps_S[:].rearrange("p (s f) -> p s f", f=Fd),
                        AF.Exp)
                phat[h] = pj
                if h % 2 == 1:
                    p = h // 2
                    pair_ps = ps.tile([128, W], DT.float32, tag="mm")
                    for s in range(S):
                        osl = pair_ps[:, s * Fd:(s + 1) * Fd]
                        for sl, hh in ((0, h - 1), (1, h)):
                            nc.tensor.matmul(
                                osl, vpx8[:, :, s, p, sl, :],
                                phat[hh][:, s, :, :],
                                start=(sl == 0), stop=(sl == 1),
                                perf_mode=mybir.MatmulPerfMode.DoubleRow,
                                skip_group_check=True)
                    pnum = wp.tile([128, W], DT.bfloat16, name="pnum",
                                   tag="lnsq", bufs=1)
                    nc.vector.tensor_copy(pnum[:], pair_ps[:])
                    rinv = wp.tile([64, W], DT.bfloat16, tag="rinv", bufs=1)
                    with nc.allow_low_precision(reason="softmax denom bf16"):
                        nc.vector.reciprocal(rinv[:], pnum[64:128, :])
                    nc.vector.tensor_mul(
                        oh[p // 2][(p % 2) * 64:(p % 2) * 64 + 64, :],
                        pnum[0:64, :], rinv[:])
                    del phat[h - 1], phat[h]
            d["oh"] = oh

        def s6_proj(blk):
            d = st[blk]
            d["o1"] = []
            for m2 in range(2):
                ps_y = ps.tile([128, W], DT.float32, tag="mm")
                for kt in range(2):
                    for ch in range(NCH):
                        sl = slice(ch * 512, (ch + 1) * 512)
                        nc.tensor.matmul(
                            ps_y[:, sl],
                            wproj[:, kt, m2 * 128:(m2 + 1) * 128],
                            d["oh"][kt][:, sl], start=(kt == 0),
                            stop=(kt == 1), skip_group_check=True)
                o1t = o1p.tile([128, W], DT.bfloat16, name=f"o1_{m2}",
                               tag=f"o1_{m2}")
                nc.vector.tensor_add(o1t[:], ps_y[:], d["xf"][m2])
                d["o1"].append(o1t)

        def s7_ln2(blk):
            d = st[blk]
            xh = [xhp.tile([128, W], DT.bfloat16, name=f"xh{kt}",
                           tag=f"xh{kt}") for kt in range(2)]
            layernorm([t[:] for t in d["o1"]],
                      [vecs[:, kt, 2:3] for kt in range(2)],
                      [vecs[:, kt, 3:4] for kt in range(2)],
                      [t[:] for t in xh], skip_lnb, skip_lng, relaxed=False)
            d["xh"] = xh

        def s8_mlp1(blk):
            d = st[blk]
            d["g16"] = []
            for mh in range(4):
                ps_h = ps.tile([128, W], DT.float32, tag="mm")
                for kt in range(2):
                    for ch in range(NCH):
                        sl = slice(ch * 512, (ch + 1) * 512)
                        nc.tensor.matmul(
                            ps_h[:, sl], w1[:, kt, mh * 128:(mh + 1) * 128],
                            d["xh"][kt][:][:, sl], start=(kt == 0),
                            stop=(kt == 1), skip_group_check=True)
                g = gp.tile([128, W], DT.bfloat16, name=f"g{mh}", tag=f"g{mh}")
                if skip_b1:
                    nc.scalar.activation(g[:], ps_h[:], AF.Gelu)
                else:
                    nc.scalar.activation(g[:], ps_h[:], AF.Gelu,
                                         bias=b1v[:, mh:mh + 1], scale=1.0)
                d["g16"].append(g)

        def s9_mlp2(blk):
            d = st[blk]
            for m2 in range(2):
                ps_o = ps.tile([128, W], DT.float32, tag="mm")
                for kt in range(4):
                    for ch in range(NCH):
                        sl = slice(ch * 512, (ch + 1) * 512)
                        nc.tensor.matmul(
                            ps_o[:, sl], w2[:, kt, m2 * 128:(m2 + 1) * 128],
                            d["g16"][kt][:][:, sl], start=(kt == 0),
                            stop=(kt == 3), skip_group_check=True)
                o2 = op.tile([128, S, Fd], DT.float32, name="o2", tag="o2",
                             bufs=1)
                o2f = o2[:].rearrange("p s f -> p (s f)")
                if skip_b2:
                    nc.vector.tensor_add(o2f, ps_o[:], d["o1"][m2][:])
                else:
                    t = op.tile([128, W], DT.float32, name="o2t", tag="o2t")
                    nc.vector.tensor_add(t[:], ps_o[:], d["o1"][m2][:])
                    nc.vector.tensor_scalar(
                        o2f, t[:], vecs[:, m2, 4:5], None, ALU.add)
                nc.gpsimd.dma_start(
                    out_d[:, m2, blk * S:(blk + 1) * S, :], o2[:])

        # prologue (pipeline fill: LN1 runs two blocks ahead)
        dma_x(0)
        dma_x(1)
        s1_ln1(0)
        s1_ln1(1)
        s3_vprime(0)
        s2_qkv(0)
        s4_l2(0)
        for b in range(NBLK):
            s5_attn(b)
            s6_proj(b)
            s7_ln2(b)
            if b + 1 < NBLK:
                s3_vprime(b + 1)
                s2_qkv(b + 1)
            s8_mlp1(b)
            s9_mlp2(b)
            if b + 1 < NBLK:
                s4_l2(b + 1)
                if b + 2 < NBLK:
                    dma_x(b + 2)
                    s1_ln1(b + 2)

    _split_excess_waits(nc, max_waits=1)
    return nc


def _host_prep(inputs):
    Wqkv = np.asarray(inputs["Wqkv"], np.float32)        # (C, 3C)
    dw_w = np.asarray(inputs["dw_w"], np.float32)        # (3C, 1, 3)
    taps = dw_w[:, 0, :]                                 # (3C, 3)

    wqk = np.ascontiguousarray(
        Wqkv[:, :512].reshape(2, 128, 512).transpose(1, 0, 2)).astype(BF16)
    # wv3[p, kt, tap, c_out] = Wv[kt*128+p, c_out] * taps_v[c_out, tap]
    Wv = Wqkv[:, 512:]                                   # (C, 256)
    wv3 = np.einsum("co,ot->cto", Wv, taps[512:])        # (C, 3, 256)
    wv3 = np.ascontiguousarray(
        wv3.reshape(2, 128, 3, 256).transpose(1, 0, 2, 3)).astype(BF16)

    def kt_major(w, nkt):
        K, N = w.shape
        return np.ascontiguousarray(
            w.reshape(nkt, 128, N).transpose(1, 0, 2)).astype(BF16)

    wproj = kt_major(np.asarray(inputs["Wproj"], np.float32), 2)
    w1 = kt_major(np.asarray(inputs["W1"], np.float32), 2)
    w2 = kt_major(np.asarray(inputs["W2"], np.float32), 4)

    vecs = np.zeros((128, 2, 6), np.float32)
    for kt in range(2):
        sl = slice(kt * 128, (kt + 1) * 128)
        vecs[:, kt, 0] = np.asarray(inputs["norm1_g"], np.float32)[sl]
        vecs[:, kt, 1] = np.asarray(inputs["norm1_b"], np.float32)[sl]
        vecs[:, kt, 2] = np.asarray(inputs["norm2_g"], np.float32)[sl]
        vecs[:, kt, 3] = np.asarray(inputs["norm2_b"], np.float32)[sl]
        vecs[:, kt, 4] = np.asarray(inputs["b2"], np.float32)[sl]
    vecs[:, 0, 5] = LN_EPS
    vecs[:, 1, 5] = 1e-24
    b1v = np.ascontiguousarray(
        np.asarray(inputs["b1"], np.float32).reshape(4, 128).T)

    # conv taps for the q,k channel tiles: tapsqk[p, m, tap]
    tapsqk = np.ascontiguousarray(
        taps[:512].reshape(4, 128, 3).transpose(1, 0, 2)).astype(np.float32)

    temp = np.asarray(inputs["temperature"], np.float32).reshape(H)
    bandq = np.zeros((128, 128), np.float32)
    bandk = np.zeros((128, 2, 128), np.float32)
    for k in range(128):
        for m in range(128):
            if k // 32 == m // 32:
                bandq[k, m] = 1.0
                for ti in range(2):
                    h = ti * 4 + m // 32
                    bandk[k, ti, m] = 1.0 / (temp[h] * temp[h])

    return dict(
        wqk=wqk, wv3=wv3, wproj=wproj, w1=w1, w2=w2, vecs=vecs, b1v=b1v,
        tapsqk=tapsqk,
        ones_b=np.ones((128, 128), BF16),
        bandq=bandq.astype(BF16),
        bandk=bandk.astype(BF16),
    )


_NC_CACHE = {}


def get_nc(flags=(True, True, True, True)):
    if flags not in _NC_CACHE:
        _NC_CACHE[flags] = build_nc(*flags)
    return _NC_CACHE[flags]


def _flags_from(inputs):
    z = lambda k: bool(np.all(np.asarray(inputs[k]) == 0.0))
    o = lambda k: bool(np.all(np.asarray(inputs[k]) == 1.0))
    return (z("b1"), z("b2"), z("norm1_b") and z("norm2_b"),
            o("norm1_g") and o("norm2_g"))


def make_in_maps(inputs):
    consts = _host_prep(inputs)
    x = np.asarray(inputs["x"], np.float32)  # (B, C, T, Fd)
    in_maps = []
    for core in range(NCORES):
        b, t0 = core // 2, (core % 2) * SPC
        m = dict(consts)
        xs = x[b, :, t0:t0 + SPC, :]          # (C, SPC, Fd)
        m["x16"] = np.ascontiguousarray(
            xs.reshape(2, 128, SPC, Fd).transpose(1, 0, 2, 3)).astype(BF16)
        in_maps.append(m)
    return in_maps


def assemble_out(results):
    out = np.zeros((B, C, T, Fd), np.float32)
    for core in range(NCORES):
        b, t0 = core // 2, (core % 2) * SPC
        r = results[core]["out"]              # (128, 2, SPC, Fd)
        out[b, :, t0:t0 + SPC, :] = r.transpose(1, 0, 2, 3).reshape(
            C, SPC, Fd)
    return out


def kernel(**inputs):
    nc = get_nc(_flags_from(inputs))
    in_maps = make_in_maps(inputs)
    res = run_bass_kernel_spmd(nc, in_maps, core_ids=list(range(NCORES)))
    return assemble_out(res.results)


# revision 7
# speedup vs baseline: 1.8636x; 1.0078x over previous
"""Trainium2 Bass kernel for the AxisMDTA dense-transformer block (v2).

Shapes (hardcoded): x (4, 256, 64, 256) fp32 -> out (4, 256, 64, 256) fp32.
256 independent samples of (f=256, c=256): LN -> qkv -> depthwise conv3
along f -> 8-head attn (L2-normed q/k, temperature) -> proj + residual ->
LN -> MLP(gelu) + residual.

Data-parallel over the 256 (b,t) samples across 8 cores (32 each, in 4
blocks of 8). Channel-major on-chip layout (c on partitions, (sample, f)
on the free dim).  Key cost choices vs v1:
  - depthwise conv runs on DVE (4x-rate scalar_tensor_tensor on bf16
    guarded-pitch views) instead of tripling the qkv matmul work;
  - v' (f-major v for attn@v) comes straight from 3-tap matmuls with
    xn slices as the stationary operand - no PE transposes;
  - LN/L2 statistics matmuls emit band-replicated rows so rsqrt runs
    directly on the replicated tiles (Ln+Exp on ACT) - no dense-pack
    DMAs, no Newton iterations, no broadcast matmuls;
  - attn@v processes head pairs with an in-psum ones-block that yields
    the softmax denominators, normalized during the psum drain;
  - x is pre-cast to bf16 on the host, halving DMA and enabling 4x DVE.
"""

import contextlib

import numpy as np
import ml_dtypes

import concourse.bass as bass
import concourse.mybir as mybir
import concourse.tile as tile
from concourse.vector_clock import ScopedClock
from concourse.bass_utils import run_bass_kernel_spmd

AF = mybir.ActivationFunctionType
ALU = mybir.AluOpType
DT = mybir.dt
BF16 = ml_dtypes.bfloat16

# Problem constants
B, C, T, Fd = 4, 256, 64, 256
H, Dh = 8, 32
HID = 512
NCORES = 8
SPC = (B * T) // NCORES  # 32 samples per core
LN_EPS = 1e-5
S = 8                    # samples per block
NBLK = SPC // S
W = S * Fd               # 2048 free columns per block
PITCH = Fd + 2           # guarded pitch for conv views
NCH = W // 512           # 512-col psum chunks


class _TileContext(tile.TileContext):
    """Walrus in this container caps sync-wait commands per CTRL-class
    instruction; spread the exit drain's waits across single-wait nops."""

    def _drain_and_barrier(self, tick_clock, wait_clock):
        drain_inst = self.nc.sync.drain()
        wait_clock.add_sem_waits(
            drain_inst.ins, ScopedClock({None: tick_clock.global_clock})
        )
        si = drain_inst.ins.sync_info
        waits = list(si.on_wait or []) if si else []
        if len(waits) > 1:
            si.on_wait = waits[:1]
            for w in waits[1:]:
                n = self.nc.sync.nop(nofuse=True).ins
                n.sync_info = mybir.SyncInfo(on_wait=[w], on_update=[])
        self.nc.all_engine_barrier()
        assert self.sems is not None
        popped = self.nc._tile_sem_poison_stack.pop()
        assert popped is self._sem_poison
        self.nc.clear_and_free_semaphores(list(self.sems.allocated().values()))
        self.nc.all_engine_barrier()


def _split_excess_waits(nc, max_waits=1):
    """Walrus in this container caps sync-wait commands per instruction.
    Move excess waits onto same-engine NoOps inserted just before."""
    for f in nc.m.functions:
        for bb in f.blocks:
            new_insts = []
            for inst in bb.instructions:
                si = inst.sync_info
                waits = list(si.on_wait) if si and si.on_wait else []
                if len(waits) > max_waits:
                    si.on_wait = waits[:max_waits]
                    rest = waits[max_waits:]
                    for i in range(0, len(rest), max_waits):
                        nop = mybir.InstEventSemaphore(
                            name=f"I-ws{nc.next_id()}", ins=[], outs=[])
                        nop.engine = inst.engine
                        nop.sync_info = mybir.SyncInfo(
                            on_wait=rest[i:i + max_waits], on_update=[])
                        nc.register_instruction(nop)
                        new_insts.append(nop)
                new_insts.append(inst)
            bb.instructions[:] = new_insts


def build_nc(skip_b1=True, skip_b2=True, skip_lnb=True,
             skip_lng=True):
    nc = bass.Bass()

    x_in = nc.dram_tensor("x16", [128, 2, SPC, Fd], DT.bfloat16,
                          kind="ExternalInput")
    out_d = nc.dram_tensor("out", [128, 2, SPC, Fd], DT.float32,
                           kind="ExternalOutput")
    wqk_d = nc.dram_tensor("wqk", [128, 2, 512], DT.bfloat16, kind="ExternalInput")
    wv3_d = nc.dram_tensor("wv3", [128, 2, 3, 256], DT.bfloat16, kind="ExternalInput")
    wproj_d = nc.dram_tensor("wproj", [128, 2, 256], DT.bfloat16, kind="ExternalInput")
    w1_d = nc.dram_tensor("w1", [128, 2, HID], DT.bfloat16, kind="ExternalInput")
    w2_d = nc.dram_tensor("w2", [128, 4, 256], DT.bfloat16, kind="ExternalInput")
    vec_d = nc.dram_tensor("vecs", [128, 2, 6], DT.float32, kind="ExternalInput")
    b1_d = nc.dram_tensor("b1v", [128, 4], DT.float32, kind="ExternalInput")
    taps_d = nc.dram_tensor("tapsqk", [128, 4, 3], DT.float32, kind="ExternalInput")
    ones_d = nc.dram_tensor("ones_b", [128, 128], DT.bfloat16, kind="ExternalInput")
    bandq_d = nc.dram_tensor("bandq", [128, 128], DT.bfloat16, kind="ExternalInput")
    bandk_d = nc.dram_tensor("bandk", [128, 2, 128], DT.bfloat16, kind="ExternalInput")

    with _TileContext(nc) as tc, contextlib.ExitStack() as ctx:
        cpool = ctx.enter_context(tc.tile_pool(name="consts", bufs=1))
        xpool = ctx.enter_context(tc.tile_pool(name="xp", bufs=2))
        xnpool = ctx.enter_context(tc.tile_pool(name="xnp", bufs=2))
        qrawp = ctx.enter_context(tc.tile_pool(name="qraw", bufs=2))
        qcp = ctx.enter_context(tc.tile_pool(name="qcp", bufs=1))
        wp = ctx.enter_context(tc.tile_pool(name="wp", bufs=3))
        php = ctx.enter_context(tc.tile_pool(name="php", bufs=4))
        ohp = ctx.enter_context(tc.tile_pool(name="ohp", bufs=1))
        o1p = ctx.enter_context(tc.tile_pool(name="o1p", bufs=1))
        xhp = ctx.enter_context(tc.tile_pool(name="xhp", bufs=1))
        gp = ctx.enter_context(tc.tile_pool(name="gp", bufs=1))
        op = ctx.enter_context(tc.tile_pool(name="op", bufs=1))
        ps = ctx.enter_context(tc.tile_pool(name="ps", bufs=2, space="PSUM"))

        def cload(name, shape, dt, dram):
            t = cpool.tile(shape, dt, tag=name)
            nc.sync.dma_start(t[:], dram[:])
            return t

        ones_b = cload("ones_b", [128, 128], DT.bfloat16, ones_d)
        vecs = cload("vecs", [128, 2, 6], DT.float32, vec_d)
        wqk = cload("wqk", [128, 2, 512], DT.bfloat16, wqk_d)
        tapsqk = cload("tapsqk", [128, 4, 3], DT.float32, taps_d)
        wv3 = cload("wv3", [128, 2, 3, 256], DT.bfloat16, wv3_d)
        bandq = cload("bandq", [128, 128], DT.bfloat16, bandq_d)
        bandk = cload("bandk", [128, 2, 128], DT.bfloat16, bandk_d)
        wproj = cload("wproj", [128, 2, 256], DT.bfloat16, wproj_d)
        w1 = cload("w1", [128, 2, HID], DT.bfloat16, w1_d)
        w2 = cload("w2", [128, 4, 256], DT.bfloat16, w2_d)
        b1v = cload("b1v", [128, 4], DT.float32, b1_d)

        # v' composite lhsT tiles (single set; the framework serializes
        # next-block drains behind this block's attn@v reads):
        # layout (128 fk, S, pair, slot, 128); slot0 = [vA |0| 1 |0],
        # slot1 = [0| vB |0| 1].
        vpx8 = cpool.tile([128, 2, S, 4, 2, 128], DT.float8e4,
                          name="vpx8", tag="vpx8")
        nc.vector.memset(vpx8[:], 0.0)
        nc.vector.memset(vpx8[:, :, :, :, 0, 64:96], 1.0)
        nc.vector.memset(vpx8[:, :, :, :, 1, 96:128], 1.0)

        def rsqrt_rep(src_ap, out_ap, eps_ap):
            """out = 1/sqrt(src + eps), elementwise, via ACT Ln + Exp.
            src may be PSUM fp32; out is bf16 SBUF (same partitions)."""
            t = wp.tile([128, W], DT.bfloat16, name="lnt", tag="wk")
            nc.scalar.activation(t[: src_ap.shape[0]], src_ap, AF.Ln,
                                 bias=eps_ap, scale=1.0)
            nc.scalar.activation(out_ap, t[: src_ap.shape[0]], AF.Exp,
                                 scale=-0.5)

        def layernorm(src_f, g_cols, b_cols, out_aps, skip_b, skip_g,
                      relaxed):
            ve = nc.vector
            """src_f: two bf16 (128, 2048) APs (channel-major, kt split).
            Writes bf16 out_aps. LN over the 256 channels (both kt tiles).
            Fast-evicts the psum stat tiles to bf16 so psum frees early."""
            ps_s = ps.tile([128, W], DT.float32, tag="mm")
            ps_q = ps.tile([128, W], DT.float32, tag="mm")
            for kt in range(2):
                for ch in range(NCH):
                    sl = slice(ch * 512, (ch + 1) * 512)
                    nc.tensor.matmul(
                        ps_s[:, sl], ones_b[:], src_f[kt][:, sl],
                        start=(kt == 0), stop=(kt == 1), skip_group_check=True)
            for kt in range(2):
                sq = wp.tile([128, W], DT.bfloat16, name="lnsq", tag="lnsq",
                             bufs=1)
                ve.tensor_mul(sq[:], src_f[kt], src_f[kt])
                for ch in range(NCH):
                    sl = slice(ch * 512, (ch + 1) * 512)
                    nc.tensor.matmul(
                        ps_q[:, sl], ones_b[:], sq[:, sl],
                        start=(kt == 0), stop=(kt == 1), skip_group_check=True)
            mu16 = wp.tile([128, W], DT.bfloat16, name="mu16", tag="lnmu",
                           bufs=2)
            ve.tensor_scalar(mu16[:], ps_s[:], 1.0 / C, None, ALU.mult)
            q16 = wp.tile([128, W], DT.bfloat16, name="q16", tag="wk")
            nc.scalar.activation(q16[:], ps_q[:], AF.Copy, bias=0.0,
                                 scale=1.0 / C)
            mu2 = wp.tile([128, W], DT.bfloat16, name="mu2", tag="wk")
            ve.tensor_mul(mu2[:], mu16[:], mu16[:])
            var = wp.tile([128, W], DT.bfloat16, name="var", tag="wk")
            ve.tensor_sub(var[:], q16[:], mu2[:])
            rsig = wp.tile([128, W], DT.bfloat16, name="rsg1", tag="lnrs",
                           bufs=2)
            rsqrt_rep(var[:], rsig[:], vecs[:, 0, 5:6])
            for kt in range(2):
                if skip_g:
                    gr = rsig
                else:
                    gr = wp.tile([128, W], DT.bfloat16, name="lngr", tag="wk")
                    nc.vector.tensor_scalar(gr[:], rsig[:], g_cols[kt], None,
                                            ALU.mult)
                a = wp.tile([128, W], DT.bfloat16, name="lna", tag="wk")
                ve.tensor_sub(a[:], src_f[kt], mu16[:])
                if skip_b:
                    nc.vector.tensor_mul(out_aps[kt], a[:], gr[:])
                else:
                    t = wp.tile([128, W], DT.bfloat16, name="lnb", tag="wk")
                    nc.vector.tensor_mul(t[:], a[:], gr[:])
                    nc.vector.tensor_scalar(
                        out_aps[kt], t[:], b_cols[kt], None, ALU.add)

        # ---------------- staged, software-pipelined blocks ----------------
        st = [dict() for _ in range(NBLK)]

        def dma_x(blk):
            d = st[blk]
            d["x16"] = [xpool.tile([128, S, Fd], DT.bfloat16,
                                   name=f"x16_{kt}", tag=f"x16_{kt}")
                        for kt in range(2)]
            for kt in range(2):
                nc.sync.dma_start(
                    d["x16"][kt][:], x_in[:, kt, blk * S:(blk + 1) * S, :])
            d["xf"] = [t[:].rearrange("p s f -> p (s f)") for t in d["x16"]]

        def s1_ln1(blk):
            d = st[blk]
            xn = [xnpool.tile([128, S, PITCH], DT.bfloat16, name=f"xn{kt}",
                              tag=f"xn{kt}") for kt in range(2)]
            for kt in range(2):
                nc.vector.memset(xn[kt][:, :, 0:1], 0.0)
                nc.vector.memset(xn[kt][:, :, PITCH - 1:PITCH], 0.0)
            xn_data = [xn[kt][:, :, 1:1 + Fd] for kt in range(2)]
            layernorm(d["xf"], [vecs[:, kt, 0:1] for kt in range(2)],
                      [vecs[:, kt, 1:2] for kt in range(2)],
                      xn_data, skip_lnb, skip_lng, relaxed=True)
            d["xn"] = xn
            d["xn_rhs"] = xn_data

        def s2_qkv(blk):
            d = st[blk]
            d["qc"] = []
            for m in range(4):
                ps_m = ps.tile([128, W], DT.float32, tag="mm")
                for kt in range(2):
                    for ch in range(NCH):
                        nc.tensor.matmul(
                            ps_m[:, ch * 512:(ch + 1) * 512],
                            wqk[:, kt, m * 128:(m + 1) * 128],
                            d["xn_rhs"][kt][:, 2 * ch:2 * ch + 2, :],
                            start=(kt == 0), stop=(kt == 1),
                            skip_group_check=True)
                qr = qrawp.tile([128, S, PITCH], DT.bfloat16, name=f"qr{m}",
                                tag="qraw")
                nc.gpsimd.memset(qr[:, :, 0:1], 0.0)
                nc.gpsimd.memset(qr[:, :, PITCH - 1:PITCH], 0.0)
                nc.scalar.copy(qr[:, :, 1:1 + Fd],
                               ps_m[:].rearrange("p (s f) -> p s f", s=S))
                t0 = wp.tile([128, S, Fd], DT.bfloat16, name="cv0", tag="wk")
                nc.vector.tensor_scalar(
                    t0[:], qr[:, :, 0:Fd], tapsqk[:, m, 0:1], None, ALU.mult)
                t1 = wp.tile([128, S, Fd], DT.bfloat16, name="cv1", tag="wk")
                nc.vector.tensor_scalar(
                    t1[:], qr[:, :, 1:1 + Fd], tapsqk[:, m, 1:2], None,
                    ALU.mult)
                t2 = wp.tile([128, S, Fd], DT.bfloat16, name="cv2", tag="wk")
                nc.vector.tensor_scalar(
                    t2[:], qr[:, :, 2:2 + Fd], tapsqk[:, m, 2:3], None,
                    ALU.mult)
                nc.vector.tensor_add(t0[:], t0[:], t1[:])
                q = qcp.tile([128, S, Fd], DT.bfloat16, name=f"qc{m}",
                             tag=f"qc{m}")
                nc.vector.tensor_add(q[:], t0[:], t2[:])
                d["qc"].append(q)

        def s3_vprime(blk):
            d = st[blk]
            for fc in range(2):
                vp_ps = ps.tile([128, S, 256], DT.float32, tag="mm")
                for s in range(S):
                    first = True
                    for kt in range(2):
                        for tap in range(3):
                            nc.tensor.matmul(
                                vp_ps[:, s, :],
                                d["xn"][kt][:, s,
                                            fc * 128 + tap:fc * 128 + tap + 128],
                                wv3[:, kt, tap, :],
                                start=first, stop=(kt == 1 and tap == 2),
                                skip_group_check=True)
                            first = False
                vsrc = vp_ps[:].rearrange("p s (h2 two d) -> p s h2 two d",
                                          two=2, d=Dh)
                nc.scalar.copy(vpx8[:, fc, :, :, 0, 0:32],
                               vsrc[:, :, :, 0, :])
                nc.vector.tensor_copy(vpx8[:, fc, :, :, 1, 32:64],
                                      vsrc[:, :, :, 1, :])

        def s4_l2(blk):
            d = st[blk]
            for m in (0, 2, 1, 3):
                qf = d["qc"][m][:].rearrange("p s f -> p (s f)")
                sqm = wp.tile([128, W], DT.bfloat16, name="sqm",
                              tag="l2sq", bufs=2)
                nc.vector.tensor_mul(sqm[:], qf, qf)
                ps_n = ps.tile([128, W], DT.float32, tag="mm")
                band = bandq[:] if m < 2 else bandk[:, m - 2, :]
                for ch in range(NCH):
                    sl = slice(ch * 512, (ch + 1) * 512)
                    nc.tensor.matmul(ps_n[:, sl], band, sqm[:, sl],
                                     start=True, stop=True,
                                     skip_group_check=True)
                rsig = wp.tile([128, W], DT.bfloat16, name="rsg2", tag="lnrs",
                               bufs=2)
                rsqrt_rep(ps_n[:], rsig[:], vecs[:, 1, 5:6])
                nc.vector.tensor_mul(qf, qf, rsig[:])

        def s5_attn(blk):
            d = st[blk]
            qc = d["qc"]
            phat = {}
            oh = [ohp.tile([128, W], DT.bfloat16, name=f"oh{kt}",
                           tag=f"oh{kt}") for kt in range(2)]
            for h in range(H):
                ti, band = h // 4, (h % 4) * 32
                pj = php.tile([128, S, 2, 256], DT.float8e4, tag="phat")
                for jt in range(2):
                    ps_S = ps.tile([128, W], DT.float32, tag="mm")
                    for s in range(S):
                        nc.tensor.matmul(
                            ps_S[:, s * Fd:(s + 1) * Fd],
                            qc[2 + ti][band:band + 32, s,
                                       jt * 128:jt * 128 + 128],
                            qc[ti][band:band + 32, s, :],
                            start=True, stop=True, skip_group_check=True,
                            tile_position=(band, 0))
                    nc.scalar.activation(
                        pj[:, :, jt, :],
                        ps_S[:].rearrange("p (s f) -> p s f", f=Fd),
                        AF.Exp)
                phat[h] = pj
                if h % 2 == 1:
                    p = h // 2
                    pair_ps = ps.tile([128, W], DT.float32, tag="mm")
                    for s in range(S):
                        osl = pair_ps[:, s * Fd:(s + 1) * Fd]
                        for sl, hh in ((0, h - 1), (1, h)):
                            nc.tensor.matmul(
                                osl, vpx8[:, :, s, p, sl, :],
                                phat[hh][:, s, :, :],
                                start=(sl == 0), stop=(sl == 1),
                                perf_mode=mybir.MatmulPerfMode.DoubleRow,
                                skip_group_check=True)
                    pnum = wp.tile([128, W], DT.bfloat16, name="pnum",
                                   tag="lnsq", bufs=1)
                    nc.vector.tensor_copy(pnum[:], pair_ps[:])
                    rinv = wp.tile([64, W], DT.bfloat16, tag="rinv", bufs=1)
                    with nc.allow_low_precision(reason="softmax denom bf16"):
                        nc.vector.reciprocal(rinv[:], pnum[64:128, :])
                    nc.vector.tensor_mul(
                        oh[p // 2][(p % 2) * 64:(p % 2) * 64 + 64, :],
                        pnum[0:64, :], rinv[:])
                    del phat[h - 1], phat[h]
            d["oh"] = oh

        def s6_proj(blk):
            d = st[blk]
            d["o1"] = []
            for m2 in range(2):
                ps_y = ps.tile([128, W], DT.float32, tag="mm")
                for kt in range(2):
                    for ch in range(NCH):
                        sl = slice(ch * 512, (ch + 1) * 512)
                        nc.tensor.matmul(
                            ps_y[:, sl],
                            wproj[:, kt, m2 * 128:(m2 + 1) * 128],
                            d["oh"][kt][:, sl], start=(kt == 0),
                            stop=(kt == 1), skip_group_check=True)
                o1t = o1p.tile([128, W], DT.bfloat16, name=f"o1_{m2}",
                               tag=f"o1_{m2}")
                nc.vector.tensor_add(o1t[:], ps_y[:], d["xf"][m2])
                d["o1"].append(o1t)

        def s7_ln2(blk):
            d = st[blk]
            xh = [xhp.tile([128, W], DT.bfloat16, name=f"xh{kt}",
                           tag=f"xh{kt}") for kt in range(2)]
            layernorm([t[:] for t in d["o1"]],
                      [vecs[:, kt, 2:3] for kt in range(2)],
                      [vecs[:, kt, 3:4] for kt in range(2)],
                      [t[:] for t in xh], skip_lnb, skip_lng, relaxed=False)
            d["xh"] = xh

        def s8_mlp1(blk):
            d = st[blk]
            d["g16"] = []
            for mh in range(4):
                ps_h = ps.tile([128, W], DT.float32, tag="mm")
                for kt in range(2):
                    for ch in range(NCH):
                        sl = slice(ch * 512, (ch + 1) * 512)
                        nc.tensor.matmul(
                            ps_h[:, sl], w1[:, kt, mh * 128:(mh + 1) * 128],
                            d["xh"][kt][:][:, sl], start=(kt == 0),
                            stop=(kt == 1), skip_group_check=True)
                g = gp.tile([128, W], DT.bfloat16, name=f"g{mh}", tag=f"g{mh}")
                if skip_b1:
                    nc.scalar.activation(g[:], ps_h[:], AF.Gelu)
                else:
                    nc.scalar.activation(g[:], ps_h[:], AF.Gelu,
                                         bias=b1v[:, mh:mh + 1], scale=1.0)
                d["g16"].append(g)

        def s9_mlp2(blk):
            d = st[blk]
            for m2 in range(2):
                ps_o = ps.tile([128, W], DT.float32, tag="mm")
                for kt in range(4):
                    for ch in range(NCH):
                        sl = slice(ch * 512, (ch + 1) * 512)
                        nc.tensor.matmul(
                            ps_o[:, sl], w2[:, kt, m2 * 128:(m2 + 1) * 128],
                            d["g16"][kt][:][:, sl], start=(kt == 0),
                            stop=(kt == 3), skip_group_check=True)
                o2 = op.tile([128, S, Fd], DT.float32, name="o2", tag="o2",
                             bufs=1)
                o2f = o2[:].rearrange("p s f -> p (s f)")
                if skip_b2:
                    nc.vector.tensor_add(o2f, ps_o[:], d["o1"][m2][:])
                else:
                    t = op.tile([128, W], DT.float32, name="o2t", tag="o2t")
                    nc.vector.tensor_add(t[:], ps_o[:], d["o1"][m2][:])
                    nc.vector.tensor_scalar(
                        o2f, t[:], vecs[:, m2, 4:5], None, ALU.add)
                nc.gpsimd.dma_start(
                    out_d[:, m2, blk * S:(blk + 1) * S, :], o2[:])

        # prologue (pipeline fill: LN1 runs two blocks ahead)
        dma_x(0)
        dma_x(1)
        s1_ln1(0)
        s1_ln1(1)
        s3_vprime(0)
        s2_qkv(0)
        s4_l2(0)
        for b in range(NBLK):
            s5_attn(b)
            s6_proj(b)
            s7_ln2(b)
            if b + 1 < NBLK:
                s3_vprime(b + 1)
                s2_qkv(b + 1)
            s8_mlp1(b)
            s9_mlp2(b)
            if b + 1 < NBLK:
                s4_l2(b + 1)
                if b + 2 < NBLK:
                    dma_x(b + 2)
                    s1_ln1(b + 2)

    _split_excess_waits(nc, max_waits=1)
    return nc


def _host_prep(inputs):
    Wqkv = np.asarray(inputs["Wqkv"], np.float32)        # (C, 3C)
    dw_w = np.asarray(inputs["dw_w"], np.float32)        # (3C, 1, 3)
    taps = dw_w[:, 0, :]                                 # (3C, 3)

    wqk = np.ascontiguousarray(
        Wqkv[:, :512].reshape(2, 128, 512).transpose(1, 0, 2)).astype(BF16)
    # wv3[p, kt, tap, c_out] = Wv[kt*128+p, c_out] * taps_v[c_out, tap]
    Wv = Wqkv[:, 512:]                                   # (C, 256)
    wv3 = np.einsum("co,ot->cto", Wv, taps[512:])        # (C, 3, 256)
    wv3 = np.ascontiguousarray(
        wv3.reshape(2, 128, 3, 256).transpose(1, 0, 2, 3)).astype(BF16)

    def kt_major(w, nkt):
        K, N = w.shape
        return np.ascontiguousarray(
            w.reshape(nkt, 128, N).transpose(1, 0, 2)).astype(BF16)

    wproj = kt_major(np.asarray(inputs["Wproj"], np.float32), 2)
    w1 = kt_major(np.asarray(inputs["W1"], np.float32), 2)
    w2 = kt_major(np.asarray(inputs["W2"], np.float32), 4)

    vecs = np.zeros((128, 2, 6), np.float32)
    for kt in range(2):
        sl = slice(kt * 128, (kt + 1) * 128)
        vecs[:, kt, 0] = np.asarray(inputs["norm1_g"], np.float32)[sl]
        vecs[:, kt, 1] = np.asarray(inputs["norm1_b"], np.float32)[sl]
        vecs[:, kt, 2] = np.asarray(inputs["norm2_g"], np.float32)[sl]
        vecs[:, kt, 3] = np.asarray(inputs["norm2_b"], np.float32)[sl]
        vecs[:, kt, 4] = np.asarray(inputs["b2"], np.float32)[sl]
    vecs[:, 0, 5] = LN_EPS
    vecs[:, 1, 5] = 1e-24
    b1v = np.ascontiguousarray(
        np.asarray(inputs["b1"], np.float32).reshape(4, 128).T)

    # conv taps for the q,k channel tiles: tapsqk[p, m, tap]
    tapsqk = np.ascontiguousarray(
        taps[:512].reshape(4, 128, 3).transpose(1, 0, 2)).astype(np.float32)

    temp = np.asarray(inputs["temperature"], np.float32).reshape(H)
    bandq = np.zeros((128, 128), np.float32)
    bandk = np.zeros((128, 2, 128), np.float32)
    for k in range(128):
        for m in range(128):
            if k // 32 == m // 32:
                bandq[k, m] = 1.0
                for ti in range(2):
                    h = ti * 4 + m // 32
                    bandk[k, ti, m] = 1.0 / (temp[h] * temp[h])

    return dict(
        wqk=wqk, wv3=wv3, wproj=wproj, w1=w1, w2=w2, vecs=vecs, b1v=b1v,
        tapsqk=tapsqk,
        ones_b=np.ones((128, 128), BF16),
        bandq=bandq.astype(BF16),
        bandk=bandk.astype(BF16),
    )


_NC_CACHE = {}


def get_nc(flags=(True, True, True, True)):
    if flags not in _NC_CACHE:
        _NC_CACHE[flags] = build_nc(*flags)
    return _NC_CACHE[flags]


def _flags_from(inputs):
    z = lambda k: bool(np.all(np.asarray(inputs[k]) == 0.0))
    o = lambda k: bool(np.all(np.asarray(inputs[k]) == 1.0))
    return (z("b1"), z("b2"), z("norm1_b") and z("norm2_b"),
            o("norm1_g") and o("norm2_g"))


def make_in_maps(inputs):
    consts = _host_prep(inputs)
    x = np.asarray(inputs["x"], np.float32)  # (B, C, T, Fd)
    in_maps = []
    for core in range(NCORES):
        b, t0 = core // 2, (core % 2) * SPC
        m = dict(consts)
        xs = x[b, :, t0:t0 + SPC, :]          # (C, SPC, Fd)
        m["x16"] = np.ascontiguousarray(
            xs.reshape(2, 128, SPC, Fd).transpose(1, 0, 2, 3)).astype(BF16)
        in_maps.append(m)
    return in_maps


def assemble_out(results):
    out = np.zeros((B, C, T, Fd), np.float32)
    for core in range(NCORES):
        b, t0 = core // 2, (core % 2) * SPC
        r = results[core]["out"]              # (128, 2, SPC, Fd)
        out[b, :, t0:t0 + SPC, :] = r.transpose(1, 0, 2, 3).reshape(
            C, SPC, Fd)
    return out


def kernel(**inputs):
    nc = get_nc(_flags_from(inputs))
    in_maps = make_in_maps(inputs)
    res = run_bass_kernel_spmd(nc, in_maps, core_ids=list(range(NCORES)))
    return assemble_out(res.results)
